# revision 1
# baseline (speedup 1.0000x reference)
"""Trainium2 Bass kernel for nn_Attention_77043123355775.

Sharded GQA causal attention with RoPE: 8 NeuronCores as 2-way data
parallel (batch) x 4-way tensor parallel (heads). Each core computes its
4 Q heads / 2 KV heads for one batch entry and a partial output
projection (x[b] @ W)^T; the host sums the 4 partials per batch.

All matmuls use bf16 hi/lo splitting (3 matmuls per logical fp32 matmul),
giving ~1e-5 relative error at ~3x bf16 matmul cost. Scores are computed
transposed (k on partitions) so the kernel needs no on-chip transposes.
"""
import math
import os
import sys

for _p in ("/opt/trn_rl_repo",):
    if _p not in sys.path:
        sys.path.insert(0, _p)

import ml_dtypes
import numpy as np

import concourse.bass as bass
import concourse.mybir as mybir
import concourse.tile as tile

from concourse.tile import add_dep_helper

dt = mybir.dt
AF = mybir.ActivationFunctionType


def build_attention_nc(S=2048, D=2048, NQ=4, NKV=2, HD=128, TC=512):
    assert HD == 128
    C = D // 128          # contraction chunks over features
    TB = S // 128         # 128-token blocks
    NTC = S // TC         # token chunks
    DB = D // 128         # output feature blocks
    CO = NQ * HD // 128   # contraction chunks for wo (= NQ)
    REP = NQ // NKV
    CH = C // 2           # c-chunks per x half-tile
    scale = 1.0 / math.sqrt(HD)

    nc = bass.Bass()

    xh = nc.dram_tensor("xh", [D, S], dt.bfloat16, kind="ExternalInput")
    xl = nc.dram_tensor("xl", [D, S], dt.bfloat16, kind="ExternalInput")
    wqp = nc.dram_tensor("wqp", [D, NQ * 2 * HD], dt.bfloat16, kind="ExternalInput")
    wkp = nc.dram_tensor("wkp", [D, NKV * 2 * HD], dt.bfloat16, kind="ExternalInput")
    wvp = nc.dram_tensor("wvp", [D, NKV * 2 * HD], dt.bfloat16, kind="ExternalInput")
    woh = nc.dram_tensor("woh", [NQ * HD, D], dt.bfloat16, kind="ExternalInput")
    wol = nc.dram_tensor("wol", [NQ * HD, D], dt.bfloat16, kind="ExternalInput")
    csT = nc.dram_tensor("csT", [HD, S], dt.float32, kind="ExternalInput")
    masks = nc.dram_tensor("masks", [4 * 128, TC], dt.bfloat16, kind="ExternalInput")
    outT = nc.dram_tensor("outT", [D, S], dt.float32, kind="ExternalOutput")

    with tile.TileContext(nc) as tc:
        with (
            tc.tile_pool(name="const", bufs=1) as constp,
            tc.tile_pool(name="tabs", bufs=1) as tabp,
            tc.tile_pool(name="acts", bufs=1) as actp,
            tc.tile_pool(name="chunkacts", bufs=1) as cap,
            tc.tile_pool(name="wstream", bufs=2) as wsp,
            tc.tile_pool(name="xstream", bufs=2) as xsp,
            tc.tile_pool(name="wo", bufs=1) as wop,
            tc.tile_pool(name="scratch", bufs=3) as scr,
            tc.tile_pool(name="psum", bufs=1, space="PSUM") as psp,
        ):
            ones_t = constp.tile([128, 1], dt.bfloat16, tag="ones")
            nc.vector.memset(ones_t[:], 1.0)
            ones_row = constp.tile([1, 128], dt.float32, tag="ones_row")
            nc.vector.memset(ones_row[:], 1.0)

            cs_t = tabp.tile([HD, S], dt.float32, tag="cs")
            nc.gpsimd.dma_start(cs_t[:], csT[:])
            cos_t = cs_t[0:HD // 2, :]
            sin_t = cs_t[HD // 2:HD, :]
            mask_t = [tabp.tile([128, TC], dt.bfloat16, tag=f"mask{i}", name=f"mask{i}") for i in range(4)]

            # wo resident; loaded on the gpsimd ring as 2 big DMAs, deferred
            # into the chunk-0 attention section
            woh_t = wop.tile([128, CO * D], dt.bfloat16, tag="woh")
            wol_t = wop.tile([128, CO * D], dt.bfloat16, tag="wol")

            # K/V persist for the full sequence (written chunk by chunk)
            kth = [actp.tile([128, S], dt.bfloat16, tag=f"kth{h}", name=f"kth{h}") for h in range(NKV)]
            ktl = [actp.tile([128, S], dt.bfloat16, tag=f"ktl{h}", name=f"ktl{h}") for h in range(NKV)]
            vh_t = [actp.tile([128, NKV * HD], dt.bfloat16, tag=f"vh{b}", name=f"vh{b}") for b in range(TB)]
            vl_t = [actp.tile([128, NKV * HD], dt.bfloat16, tag=f"vl{b}", name=f"vl{b}") for b in range(TB)]

            for tci in range(NTC):
                ts = slice(tci * TC, (tci + 1) * TC)
                qth = [cap.tile([128, TC], dt.bfloat16, tag=f"qth{h}", name=f"qth{h}_{tci}") for h in range(NQ)]
                qtl = [cap.tile([128, TC], dt.bfloat16, tag=f"qtl{h}", name=f"qtl{h}_{tci}") for h in range(NQ)]
                oth = [cap.tile([128, TC], dt.bfloat16, tag=f"oth{h}", name=f"oth{h}_{tci}") for h in range(NQ)]
                otl = [cap.tile([128, TC], dt.bfloat16, tag=f"otl{h}", name=f"otl{h}_{tci}") for h in range(NQ)]

                # ---- big-DMA input streams for chunk tci ----
                # wqk head 0 first on the sync ring so PE can start ASAP
                wqk_t = []
                for h in range(NQ + NKV):
                    whl = wsp.tile([128, C * 2 * HD], dt.bfloat16, tag="wqk_s",
                                   bufs=3, name=f"wqk_{tci}_{h}")
                    wqk_t.append(whl)
                wsrcs = [wqp] * NQ + [wkp] * NKV
                wcols = [h * 2 * HD for h in range(NQ)] + [h * 2 * HD for h in range(NKV)]

                wqk_dma = {}

                def dma_wqk(h):
                    src = wsrcs[h][:, wcols[h]:wcols[h] + 2 * HD]
                    wqk_dma[h] = nc.sync.dma_start(
                        wqk_t[h].rearrange("p (c n) -> p c n", c=C),
                        src.rearrange("(c p) n -> p c n", p=128),
                    )

                # Ring plan per chunk (pseudo-DMAs block their issuing
                # engine for the whole transfer): sync carries wqk + x-hi,
                # scalar carries x-lo + wv, gpsimd carries stores + consts.
                # x is loaded in quarters interleaved with wqk so PE starts
                # on the first quarter instead of waiting for half/full x.
                CQ = max(C // 4, 1)
                NG = C // CQ
                xh_g = []
                xl_g = []
                for g in range(NG):
                    rs = slice(g * CQ * 128, (g + 1) * CQ * 128)
                    th = xsp.tile([128, CQ * TC], dt.bfloat16, tag="xh", bufs=NG, name=f"xh_{tci}_{g}")
                    tl = xsp.tile([128, CQ * TC], dt.bfloat16, tag="xl", bufs=NG, name=f"xl_{tci}_{g}")
                    if g == 0:
                        dma_wqk(0)
                    nc.sync.dma_start(
                        th.rearrange("p (c n) -> p c n", c=CQ),
                        xh[rs, ts].rearrange("(c p) n -> p c n", p=128),
                    )
                    nc.scalar.dma_start(
                        tl.rearrange("p (c n) -> p c n", c=CQ),
                        xl[rs, ts].rearrange("(c p) n -> p c n", p=128),
                    )
                    if 1 + g < NQ + NKV:
                        dma_wqk(1 + g)
                    xh_g.append(th)
                    xl_g.append(tl)
                for h in range(NG + 1, NQ + NKV):
                    dma_wqk(h)
                CW = CQ
                # wv halves: gpsimd ring at chunk 0 (keeps the startup
                # burst off the HWDGE rings), scalar ring afterwards
                wv_ring = nc.gpsimd if tci == 0 else nc.scalar
                wv_g = []
                for g in range(2):
                    rs = slice(g * CH * 128, (g + 1) * CH * 128)
                    t = wsp.tile([128, CH * 2 * NKV * HD], dt.bfloat16, tag="wv_s",
                                 bufs=2, name=f"wv_{tci}_{g}")
                    wv_ring.dma_start(
                        t.rearrange("p (c n) -> p c n", c=CH),
                        wvp[rs, :].rearrange("(c p) n -> p c n", p=128),
                    )
                    wv_g.append(t)
                if tci == 0:
                    for i in range(4):
                        nc.gpsimd.dma_start(mask_t[i][:], masks[i * 128:(i + 1) * 128, :])

                def xh_c(c):
                    return xh_g[c // CW][:, (c % CW) * TC:(c % CW + 1) * TC]

                def xl_c(c):
                    return xl_g[c // CW][:, (c % CW) * TC:(c % CW + 1) * TC]

                # ---- QKV projections + RoPE + split ----
                first_mm = {}
                for h in range(NQ + NKV):
                    is_q = h < NQ
                    ps = psp.tile([128, TC], dt.float32, tag="mm", bufs=2)
                    n_mm = 3 * C
                    i_mm = 0
                    for c in range(C):
                        wht = wqk_t[h][:, c * 2 * HD:c * 2 * HD + HD]
                        wlt = wqk_t[h][:, c * 2 * HD + HD:(c + 1) * 2 * HD]
                        for lhsT, rhs in ((wht, xh_c(c)), (wht, xl_c(c)), (wlt, xh_c(c))):
                            mm = nc.tensor.matmul(
                                ps[:], lhsT, rhs,
                                start=(i_mm == 0), stop=(i_mm == n_mm - 1),
                            )
                            if i_mm == 0:
                                first_mm[h] = mm
                            i_mm += 1
                    # chunk 0: pace the weight prefetch two heads ahead so
                    # the startup HBM burst stays small (all 8 cores share
                    # chip bandwidth at packet granularity)
                    if tci == 0 and h + 2 in wqk_dma:
                        add_dep_helper(wqk_dma[h + 2].ins, first_mm[h].ins,
                                       reason="startup prefetch throttle")
                    # RoPE in f32 from PSUM -> scratch
                    rot = scr.tile([128, TC], dt.float32, tag="rope", bufs=2)
                    t0 = scr.tile([128, TC], dt.float32, tag="ropetmp", bufs=1)
                    cs = cos_t[:, ts]
                    sn = sin_t[:, ts]
                    xr = ps[0:64, :]
                    xi = ps[64:128, :]
                    nc.vector.tensor_tensor(rot[0:64, :], xr, cs, mybir.AluOpType.mult)
                    nc.vector.tensor_tensor(t0[0:64, :], xi, sn, mybir.AluOpType.mult)
                    nc.vector.tensor_tensor(rot[0:64, :], rot[0:64, :], t0[0:64, :], mybir.AluOpType.subtract)
                    nc.vector.tensor_tensor(rot[64:128, :], xr, sn, mybir.AluOpType.mult)
                    nc.vector.tensor_tensor(t0[64:128, :], xi, cs, mybir.AluOpType.mult)
                    nc.vector.tensor_tensor(rot[64:128, :], rot[64:128, :], t0[64:128, :], mybir.AluOpType.add)
                    if is_q:
                        dsth, dstl = qth[h][:], qtl[h][:]
                    else:
                        dsth, dstl = kth[h - NQ][:, ts], ktl[h - NQ][:, ts]
                    nc.vector.tensor_copy(dsth, rot[:])
                    nc.vector.tensor_tensor(dstl, rot[:], dsth, mybir.AluOpType.subtract)

                # V projection
                for tb in range(TC // 128):
                    tbg = tci * (TC // 128) + tb
                    ps = psp.tile([128, NKV * HD], dt.float32, tag="mm", bufs=2)
                    n_mm = 3 * C
                    i_mm = 0
                    for c in range(C):
                        xh_s = xh_c(c)[:, tb * 128:(tb + 1) * 128]
                        xl_s = xl_c(c)[:, tb * 128:(tb + 1) * 128]
                        g, cc = c // CH, c % CH
                        vht = wv_g[g][:, cc * 2 * NKV * HD:cc * 2 * NKV * HD + NKV * HD]
                        vlt = wv_g[g][:, cc * 2 * NKV * HD + NKV * HD:(cc + 1) * 2 * NKV * HD]
                        for lhsT, rhs in ((xh_s, vht), (xh_s, vlt), (xl_s, vht)):
                            nc.tensor.matmul(
                                ps[:], lhsT, rhs,
                                start=(i_mm == 0), stop=(i_mm == n_mm - 1),
                            )
                            i_mm += 1
                    nc.vector.tensor_copy(vh_t[tbg][:], ps[:])
                    nc.vector.tensor_tensor(vl_t[tbg][:], ps[:], vh_t[tbg][:], mybir.AluOpType.subtract)

                # ---- attention for q-chunk tci (keys 0..(tci+1)*TC) ----
                if tci == 0:
                    nc.gpsimd.dma_start(
                        woh_t.rearrange("p (c n) -> p c n", c=CO),
                        woh.rearrange("(c p) n -> p c n", p=128),
                    )
                    nc.gpsimd.dma_start(
                        wol_t.rearrange("p (c n) -> p c n", c=CO),
                        wol.rearrange("(c p) n -> p c n", p=128),
                    )
                qc = tci
                nkb = (qc + 1) * (TC // 128)
                pending_norm = []

                def emit_norm(h, ot_ps, sum_ps):
                    rec = scr.tile([1, TC], dt.float32, tag="rec", bufs=1, name=f"rec_{tci}_{h}")
                    nc.vector.reciprocal(rec[:], sum_ps[:])
                    bc_ps = psp.tile([128, TC], dt.float32, tag="bcast", bufs=1, name=f"bc_{tci}_{h}")
                    nc.tensor.matmul(bc_ps[:], ones_row[:], rec[:], start=True, stop=True)
                    recb = scr.tile([128, TC], dt.float32, tag="recb", bufs=1, name=f"recb_{tci}_{h}")
                    nc.scalar.copy(recb[:], bc_ps[:])
                    otn = scr.tile([128, TC], dt.float32, tag="otn", bufs=2, name=f"otn_{tci}_{h}")
                    nc.vector.tensor_tensor(otn[:], ot_ps[:], recb[:], mybir.AluOpType.mult)
                    nc.vector.tensor_copy(oth[h][:], otn[:])
                    nc.vector.tensor_tensor(otl[h][:], otn[:], oth[h][:], mybir.AluOpType.subtract)

                # Two-stage software pipeline over all (head, block)
                # pairs: scores/exp/split lead PV by LAG blocks so the PE
                # never waits on the ACT/DVE probs chain at head starts.
                LAG = 4
                blocks = [(h, kb) for h in range(NQ) for kb in range(nkb)]
                head_ps = {}
                head_sum = {}

                def emit_scores(h, kb):
                    kv = h // REP
                    d = kb * 128 - qc * TC
                    ks = slice(kb * 128, (kb + 1) * 128)
                    q0 = max(d, 0)
                    sc_ps = psp.tile([128, TC], dt.float32, tag="mm", bufs=2,
                                     name=f"sc_{tci}_{h}_{kb}")
                    mms = (
                        (kth[kv][:, ks], qth[h][:, q0:TC]),
                        (kth[kv][:, ks], qtl[h][:, q0:TC]),
                        (ktl[kv][:, ks], qth[h][:, q0:TC]),
                    )
                    for i_mm, (lhsT, rhs) in enumerate(mms):
                        nc.tensor.matmul(sc_ps[:, q0:TC], lhsT, rhs, start=(i_mm == 0), stop=(i_mm == 2))
                    pf = scr.tile([128, TC], dt.float32, tag="pf", bufs=3,
                                  name=f"pf_{tci}_{h}_{kb}")
                    nc.scalar.activation(pf[:, q0:TC], sc_ps[:, q0:TC], AF.Exp, bias=0.0, scale=scale)
                    if d >= 0:
                        nc.vector.tensor_tensor(pf[:, q0:TC], pf[:, q0:TC], mask_t[d // 128][:, q0:TC], mybir.AluOpType.mult)
                    ph = scr.tile([128, TC], dt.bfloat16, tag="ph", bufs=LAG + 2,
                                  name=f"ph_{tci}_{h}_{kb}")
                    pl = scr.tile([128, TC], dt.bfloat16, tag="pl", bufs=LAG + 2,
                                  name=f"pl_{tci}_{h}_{kb}")
                    nc.scalar.copy(ph[:, q0:TC], pf[:, q0:TC])
                    nc.vector.tensor_tensor(pl[:, q0:TC], pf[:, q0:TC], ph[:, q0:TC], mybir.AluOpType.subtract)
                    return ph, pl

                def emit_pv(h, kb, ph, pl):
                    kv = h // REP
                    vcol = kv * HD
                    d = kb * 128 - qc * TC
                    q0 = max(d, 0)
                    if kb == 0:
                        head_ps[h] = (
                            psp.tile([128, TC], dt.float32, tag="otps", bufs=3,
                                     name=f"ot_{tci}_{h}"),
                            psp.tile([1, TC], dt.float32, tag="sums", bufs=2,
                                     name=f"sum_{tci}_{h}"),
                        )
                    ot_ps, sum_ps = head_ps[h]
                    vh_s = vh_t[kb][:, vcol:vcol + HD]
                    vl_s = vl_t[kb][:, vcol:vcol + HD]
                    pv = ((vh_s, ph[:, q0:TC]), (vh_s, pl[:, q0:TC]), (vl_s, ph[:, q0:TC]))
                    for i_mm, (lhsT, rhs) in enumerate(pv):
                        nc.tensor.matmul(
                            ot_ps[:, q0:TC], lhsT, rhs,
                            start=(kb == 0 and i_mm == 0),
                            stop=(kb == nkb - 1 and i_mm == 2),
                        )
                    for i_mm, rhs in enumerate((ph[:, q0:TC], pl[:, q0:TC])):
                        nc.tensor.matmul(
                            sum_ps[:, q0:TC], ones_t[:], rhs,
                            start=(kb == 0 and i_mm == 0),
                            stop=(kb == nkb - 1 and i_mm == 1),
                        )
                    if kb == nkb - 1:
                        pending_norm.append((h, ot_ps, sum_ps))
                        if len(pending_norm) > 1:
                            emit_norm(*pending_norm.pop(0))

                probs_q = []
                for h, kb in blocks:
                    probs_q.append((h, kb, emit_scores(h, kb)))
                    if len(probs_q) > LAG:
                        hh, kk, (ph, pl) = probs_q.pop(0)
                        emit_pv(hh, kk, ph, pl)
                for hh, kk, (ph, pl) in probs_q:
                    emit_pv(hh, kk, ph, pl)

                # ---- output projection for token-chunk tci ----
                for db in range(DB):
                    ds_ = slice(db * 128, (db + 1) * 128)
                    ps = psp.tile([128, TC], dt.float32, tag="mm", bufs=2)
                    n_mm = 3 * CO
                    i_mm = 0
                    for c in range(CO):
                        # the last head's normalization drains here, covered
                        # by the first db's head-0..2 matmuls
                        if db == 0 and c == CO - 1 and pending_norm:
                            for args in pending_norm:
                                emit_norm(*args)
                            pending_norm = []
                        wh_s = woh_t[:, c * D + db * 128:c * D + (db + 1) * 128]
                        wl_s = wol_t[:, c * D + db * 128:c * D + (db + 1) * 128]
                        for lhsT, rhs in (
                            (wh_s, oth[c][:]),
                            (wh_s, otl[c][:]),
                            (wl_s, oth[c][:]),
                        ):
                            nc.tensor.matmul(
                                ps[:], lhsT, rhs,
                                start=(i_mm == 0), stop=(i_mm == n_mm - 1),
                            )
                            i_mm += 1
                    o3 = scr.tile([128, TC], dt.float32, tag="o3", bufs=2)
                    nc.scalar.copy(o3[:], ps[:])
                    eng = nc.sync if tci == NTC - 1 else nc.gpsimd
                    eng.dma_start(outT[ds_, ts], o3[:])

    return nc


# ---------------------------------------------------------------------------
# walrus in this container refuses >1 sem wait per instruction ("Too many
# sync wait commands"). Hoist excess waits onto same-engine NoOps inserted
# immediately before the instruction - program order on the engine queue
# preserves the sync semantics.
def split_multiwait_insts(nc, max_waits=1):
    n_split = 0
    for bb in nc.main_func.blocks:
        insts = bb.instructions
        i = 0
        while i < len(insts):
            ins = insts[i]
            si = getattr(ins, "sync_info", None)
            if si is not None and si.on_wait and len(si.on_wait) > max_waits:
                waits = list(si.on_wait)
                head, tail = waits[:-max_waits], waits[-max_waits:]
                nops = []
                for j in range(0, len(head), max_waits):
                    nop = mybir.InstNoOp(name=f"{ins.name}-ws{j}", ins=[], outs=[])
                    nop.engine = ins.engine
                    nop.sync_info = mybir.SyncInfo(
                        on_wait=head[j:j + max_waits], on_update=[])
                    nops.append(nop)
                ins.sync_info = mybir.SyncInfo(
                    on_wait=tail, on_update=list(si.on_update or []))
                insts[i:i] = nops
                i += len(nops)
                n_split += 1
            i += 1
    return n_split


# ---------------------------------------------------------------------------
# Host-side shard preparation / gather
BF16 = ml_dtypes.bfloat16


BF16 = ml_dtypes.bfloat16


def _split(a):
    h = a.astype(BF16)
    l = (a.astype(np.float32) - h.astype(np.float32)).astype(BF16)
    return h, l


def rope_tables(S, HD):
    inv = 1.0 / (10000.0 ** (np.arange(0, HD, 2, dtype=np.float32) / HD))
    t = np.arange(S, dtype=np.float32)
    f = np.outer(t, inv).astype(np.float32)  # [S, HD//2]
    return np.ascontiguousarray(np.cos(f).T), np.ascontiguousarray(np.sin(f).T)


def causal_masks(TC):
    # masks[dd][k, qrel] = 1 if k + dd*128 <= qrel else 0
    out = np.zeros((4 * 128, TC), BF16)
    k = np.arange(128)[:, None]
    q = np.arange(TC)[None, :]
    for dd in range(4):
        out[dd * 128:(dd + 1) * 128] = (k + dd * 128 <= q).astype(BF16)
    return out


def rope_perm(HD):
    # new row i (i < HD//2) = old 2i; new row HD//2+i = old 2i+1
    return np.concatenate([np.arange(0, HD, 2), np.arange(1, HD, 2)])


def make_in_maps(x, wq, wk, wv, wo, *, n_batch_shards, n_head_shards,
                 NQ_TOT, NKV_TOT, HD, TC):
    """Returns list of in_maps, one per core (batch-major: core = b*G + g)."""
    B, S, D = x.shape
    G = n_head_shards
    NQ = NQ_TOT // G
    NKV = NKV_TOT // G
    perm = rope_perm(HD)
    cosT, sinT = rope_tables(S, HD)
    csT = np.concatenate([cosT, sinT], axis=0)  # [HD, S]
    masks = causal_masks(TC)

    # Per-batch xT splits (shared across head shards)
    xt = {}
    for b in range(B):
        xT = np.ascontiguousarray(x[b].T)  # [D, S]
        xt[b] = _split(xT)

    def _pack_per_head(wT_h, wT_l, n_heads):
        # [D, n_heads*HD] hi/lo -> [D, n_heads*2*HD] with per-head [hi | lo]
        D_ = wT_h.shape[0]
        out = np.empty((D_, n_heads * 2 * HD), BF16)
        for hh in range(n_heads):
            out[:, hh * 2 * HD:hh * 2 * HD + HD] = wT_h[:, hh * HD:(hh + 1) * HD]
            out[:, hh * 2 * HD + HD:(hh + 1) * 2 * HD] = wT_l[:, hh * HD:(hh + 1) * HD]
        return out

    # Per-headgroup weight shards
    wshard = {}
    for g in range(G):
        qrows = slice(g * NQ * HD, (g + 1) * NQ * HD)
        kvrows = slice(g * NKV * HD, (g + 1) * NKV * HD)
        wq_g = wq[qrows, :].copy()      # [NQ*HD, D]
        wk_g = wk[kvrows, :].copy()
        wv_g = wv[kvrows, :].copy()
        # RoPE permutation of output rows, per head
        for hh in range(NQ):
            blk = wq_g[hh * HD:(hh + 1) * HD]
            wq_g[hh * HD:(hh + 1) * HD] = blk[perm]
        for hh in range(NKV):
            blk = wk_g[hh * HD:(hh + 1) * HD]
            wk_g[hh * HD:(hh + 1) * HD] = blk[perm]
        wqT = np.ascontiguousarray(wq_g.T)   # [D, NQ*HD]
        wkT = np.ascontiguousarray(wk_g.T)
        wvT = np.ascontiguousarray(wv_g.T)
        woT = np.ascontiguousarray(wo[:, qrows].T)  # [NQ*HD, D]
        wqp = _pack_per_head(*_split(wqT), NQ)
        wkp = _pack_per_head(*_split(wkT), NKV)
        wvh_, wvl_ = _split(wvT)
        wvp = np.concatenate([wvh_, wvl_], axis=1)  # [D, 2*NKV*HD] hi-all|lo-all
        wshard[g] = (wqp, wkp, wvp, _split(woT))

    in_maps = []
    for b in range(n_batch_shards):
        for g in range(G):
            wqp, wkp, wvp, (woh, wol) = wshard[g]
            xh, xl = xt[b]
            in_maps.append({
                "xh": xh, "xl": xl,
                "wqp": wqp, "wkp": wkp, "wvp": wvp,
                "woh": woh, "wol": wol,
                "csT": csT,
                "masks": masks,
            })
    return in_maps


def combine_outputs(outTs, B, G):
    """outTs: list of [D, S] partials, core order b*G+g. Returns [B, S, D]."""
    outs = []
    for b in range(B):
        acc = outTs[b * G].astype(np.float32).copy()
        for g in range(1, G):
            acc += outTs[b * G + g]
        outs.append(acc.T)  # [S, D]
    return np.stack(outs)


_NC_CACHE = {}


def _get_nc(S, D, NQ, NKV, HD, TC):
    key = (S, D, NQ, NKV, HD, TC)
    if key not in _NC_CACHE:
        nc = build_attention_nc(S=S, D=D, NQ=NQ, NKV=NKV, HD=HD, TC=TC)
        split_multiwait_insts(nc)
        _NC_CACHE[key] = nc
    return _NC_CACHE[key]


def kernel(**inputs):
    x = np.asarray(inputs["x"], dtype=np.float32)
    wq = np.asarray(inputs["wq"], dtype=np.float32)
    wk = np.asarray(inputs["wk"], dtype=np.float32)
    wv = np.asarray(inputs["wv"], dtype=np.float32)
    wo = np.asarray(inputs["wo"], dtype=np.float32)

    B, S, D = x.shape          # (2, 2048, 2048)
    NQ_TOT = wq.shape[0] // 128
    NKV_TOT = wk.shape[0] // 128
    HD = 128
    TC = 512
    G = 4                      # head shards
    NQ, NKV = NQ_TOT // G, NKV_TOT // G

    nc = _get_nc(S, D, NQ, NKV, HD, TC)
    in_maps = make_in_maps(
        x, wq, wk, wv, wo,
        n_batch_shards=B, n_head_shards=G,
        NQ_TOT=NQ_TOT, NKV_TOT=NKV_TOT, HD=HD, TC=TC,
    )

    from concourse.bass_utils import run_bass_kernel_spmd

    trace = os.environ.get("BASS_ATTN_TRACE") == "1"
    res = run_bass_kernel_spmd(nc, in_maps, list(range(len(in_maps))), trace=trace)
    kernel.last_results = res
    outTs = [r["outT"] for r in res.results]
    return combine_outputs(outTs, B, G).astype(np.float32)



# revision 2
# speedup vs baseline: 1.8533x; 1.8533x over previous
"""Trainium2 Bass kernel for nn_Attention_77043123355775.

Sharded GQA causal attention with RoPE: 8 NeuronCores as 2-way data
parallel (batch) x 4-way tensor parallel (heads). Each core computes its
4 Q heads / 2 KV heads for one batch entry and a partial output
projection (x[b] @ W)^T; the host sums the 4 partials per batch.

All matmuls run in plain bf16 with fp32 PSUM accumulation (the 2e-2
rel-err budget has ~20x headroom over bf16 rounding noise). Weights are
fully SBUF-resident, loaded once. Scores are computed transposed (k on
partitions) so the kernel needs no on-chip transposes.
"""
import math
import os
import sys

for _p in ("/opt/trn_rl_repo",):
    if _p not in sys.path:
        sys.path.insert(0, _p)

import ml_dtypes
import numpy as np

import concourse.bass as bass
import concourse.mybir as mybir
import concourse.tile as tile

dt = mybir.dt
AF = mybir.ActivationFunctionType


def build_attention_nc(S=2048, D=2048, NQ=4, NKV=2, HD=128, TC=512):
    assert HD == 128
    C = D // 128          # contraction chunks over features
    TB = S // 128         # 128-token blocks
    NTC = S // TC         # token chunks
    DB = D // 128         # output feature blocks
    CO = NQ * HD // 128   # contraction chunks for wo (= NQ)
    REP = NQ // NKV
    CQ = C // 4           # c-chunks per x quarter-tile
    scale = 1.0 / math.sqrt(HD)

    nc = bass.Bass()

    xt = nc.dram_tensor("xt", [D, S], dt.bfloat16, kind="ExternalInput")
    wqp = nc.dram_tensor("wqp", [D, NQ * HD], dt.bfloat16, kind="ExternalInput")
    wkp = nc.dram_tensor("wkp", [D, NKV * HD], dt.bfloat16, kind="ExternalInput")
    wvp = nc.dram_tensor("wvp", [D, NKV * HD], dt.bfloat16, kind="ExternalInput")
    woh = nc.dram_tensor("woh", [NQ * HD, D], dt.bfloat16, kind="ExternalInput")
    csT = nc.dram_tensor("csT", [HD, S], dt.float32, kind="ExternalInput")
    masks = nc.dram_tensor("masks", [4 * 128, TC], dt.bfloat16, kind="ExternalInput")
    outT = nc.dram_tensor("outT", [D, S], dt.float32, kind="ExternalOutput")

    with tile.TileContext(nc) as tc:
        with (
            tc.tile_pool(name="const", bufs=1) as constp,
            tc.tile_pool(name="tabs", bufs=1) as tabp,
            tc.tile_pool(name="wts", bufs=1) as wtp,
            tc.tile_pool(name="acts", bufs=1) as actp,
            tc.tile_pool(name="chunkacts", bufs=1) as cap,
            tc.tile_pool(name="xstream", bufs=8) as xsp,
            tc.tile_pool(name="scratch", bufs=3) as scr,
            tc.tile_pool(name="psum", bufs=1, space="PSUM") as psp,
        ):
            ones_t = constp.tile([128, 1], dt.bfloat16, tag="ones")
            nc.vector.memset(ones_t[:], 1.0)
            ones_row = constp.tile([1, 128], dt.bfloat16, tag="ones_row")
            nc.vector.memset(ones_row[:], 1.0)

            # ---- resident tables / weights ----
            # sync ring: wq first (PE's first dependency), then x chunk-0
            # halves, then wk/wv/wo. gpsimd ring: cs, masks, x chunk-0 tail.
            wq_t = wtp.tile([128, C * NQ * HD], dt.bfloat16, tag="wq")
            wk_t = wtp.tile([128, C * NKV * HD], dt.bfloat16, tag="wk")
            wv_t = wtp.tile([128, C * NKV * HD], dt.bfloat16, tag="wv")
            wo_t = wtp.tile([128, CO * D], dt.bfloat16, tag="wo")

            nc.sync.dma_start(
                wq_t.rearrange("p (c n) -> p c n", c=C),
                wqp.rearrange("(c p) n -> p c n", p=128),
            )

            cs_t = tabp.tile([HD, S], dt.float32, tag="cs")
            nc.gpsimd.dma_start(cs_t[:], csT[:])
            cos_t = cs_t[0:HD // 2, :]
            sin_t = cs_t[HD // 2:HD, :]
            mask_t = [tabp.tile([128, TC], dt.bfloat16, tag=f"mask{i}", name=f"mask{i}") for i in range(4)]
            for i in range(4):
                nc.gpsimd.dma_start(mask_t[i][:], masks[i * 128:(i + 1) * 128, :])

            # x quarter tiles per chunk; chunk 0 split across the two rings
            xq_tiles = {}

            def emit_x_dmas(tci, rings):
                ts_ = slice(tci * TC, (tci + 1) * TC)
                tiles = []
                for g in range(4):
                    rs = slice(g * CQ * 128, (g + 1) * CQ * 128)
                    t = xsp.tile([128, CQ * TC], dt.bfloat16, tag="xq",
                                 name=f"x_{tci}_{g}")
                    rings[g].dma_start(
                        t.rearrange("p (c n) -> p c n", c=CQ),
                        xt[rs, ts_].rearrange("(c p) n -> p c n", p=128),
                    )
                    tiles.append(t)
                xq_tiles[tci] = tiles

            emit_x_dmas(0, [nc.sync, nc.sync, nc.gpsimd, nc.gpsimd])
            nc.sync.dma_start(
                wk_t.rearrange("p (c n) -> p c n", c=C),
                wkp.rearrange("(c p) n -> p c n", p=128),
            )
            nc.sync.dma_start(
                wv_t.rearrange("p (c n) -> p c n", c=C),
                wvp.rearrange("(c p) n -> p c n", p=128),
            )
            nc.sync.dma_start(
                wo_t.rearrange("p (c n) -> p c n", c=CO),
                woh.rearrange("(c p) n -> p c n", p=128),
            )

            # K/V persist for the full sequence (written chunk by chunk)
            kt = [actp.tile([128, S], dt.bfloat16, tag=f"kt{h}", name=f"kt{h}") for h in range(NKV)]
            vt = [actp.tile([128, NKV * HD], dt.bfloat16, tag=f"vt{b}", name=f"vt{b}") for b in range(TB)]

            for tci in range(NTC):
                ts = slice(tci * TC, (tci + 1) * TC)
                qt = [cap.tile([128, TC], dt.bfloat16, tag=f"qt{h}", name=f"qt{h}_{tci}") for h in range(NQ)]
                ot = [cap.tile([128, TC], dt.bfloat16, tag=f"ot{h}", name=f"ot{h}_{tci}") for h in range(NQ)]

                # prefetch next chunk's x on the sync ring
                if tci + 1 < NTC:
                    emit_x_dmas(tci + 1, [nc.sync] * 4)

                xg = xq_tiles.pop(tci)

                def x_c(c):
                    return xg[c // CQ][:, (c % CQ) * TC:(c % CQ + 1) * TC]

                # ---- QKV projections + RoPE ----
                for h in range(NQ + NKV):
                    is_q = h < NQ
                    ps = psp.tile([128, TC], dt.float32, tag="mm", bufs=2)
                    if is_q:
                        wsl = [wq_t[:, c * NQ * HD + h * HD:c * NQ * HD + (h + 1) * HD] for c in range(C)]
                    else:
                        hk = h - NQ
                        wsl = [wk_t[:, c * NKV * HD + hk * HD:c * NKV * HD + (hk + 1) * HD] for c in range(C)]
                    for c in range(C):
                        nc.tensor.matmul(
                            ps[:], wsl[c], x_c(c),
                            start=(c == 0), stop=(c == C - 1),
                        )
                    # RoPE in f32 from PSUM, final ops write bf16 dest
                    rot = scr.tile([128, TC], dt.float32, tag="rope", bufs=2)
                    t0 = scr.tile([128, TC], dt.float32, tag="ropetmp", bufs=1)
                    cs = cos_t[:, ts]
                    sn = sin_t[:, ts]
                    xr = ps[0:64, :]
                    xi = ps[64:128, :]
                    dsth = qt[h][:] if is_q else kt[h - NQ][:, ts]
                    nc.vector.tensor_tensor(rot[0:64, :], xr, cs, mybir.AluOpType.mult)
                    nc.vector.tensor_tensor(t0[0:64, :], xi, sn, mybir.AluOpType.mult)
                    nc.vector.tensor_tensor(dsth[0:64, :], rot[0:64, :], t0[0:64, :], mybir.AluOpType.subtract)
                    nc.vector.tensor_tensor(rot[64:128, :], xr, sn, mybir.AluOpType.mult)
                    nc.vector.tensor_tensor(t0[64:128, :], xi, cs, mybir.AluOpType.mult)
                    nc.vector.tensor_tensor(dsth[64:128, :], rot[64:128, :], t0[64:128, :], mybir.AluOpType.add)

                # V projection
                for tb in range(TC // 128):
                    tbg = tci * (TC // 128) + tb
                    ps = psp.tile([128, NKV * HD], dt.float32, tag="mm", bufs=2)
                    for c in range(C):
                        xsl = x_c(c)[:, tb * 128:(tb + 1) * 128]
                        nc.tensor.matmul(
                            ps[:], xsl, wv_t[:, c * NKV * HD:(c + 1) * NKV * HD],
                            start=(c == 0), stop=(c == C - 1),
                        )
                    nc.scalar.copy(vt[tbg][:], ps[:])

                # ---- attention for q-chunk tci (keys 0..(tci+1)*TC) ----
                qc = tci
                nkb = (qc + 1) * (TC // 128)
                pending_norm = []

                def emit_norm(h, ot_ps, sum_ps):
                    rec = scr.tile([1, TC], dt.float32, tag="rec", bufs=1, name=f"rec_{tci}_{h}")
                    nc.vector.reciprocal(rec[:], sum_ps[:])
                    rech = scr.tile([1, TC], dt.bfloat16, tag="rech", bufs=1, name=f"rech_{tci}_{h}")
                    recl = scr.tile([1, TC], dt.bfloat16, tag="recl", bufs=1, name=f"recl_{tci}_{h}")
                    nc.scalar.copy(rech[:], rec[:])
                    nc.vector.tensor_tensor(recl[:], rec[:], rech[:], mybir.AluOpType.subtract)
                    bc_ps = psp.tile([128, TC], dt.float32, tag="bcast", bufs=1, name=f"bc_{tci}_{h}")
                    nc.tensor.matmul(bc_ps[:], ones_row[:], rech[:], start=True, stop=False)
                    nc.tensor.matmul(bc_ps[:], ones_row[:], recl[:], start=False, stop=True)
                    recb = scr.tile([128, TC], dt.float32, tag="recb", bufs=1, name=f"recb_{tci}_{h}")
                    nc.scalar.copy(recb[:], bc_ps[:])
                    nc.vector.tensor_tensor(ot[h][:], ot_ps[:], recb[:], mybir.AluOpType.mult)

                # Two-stage software pipeline over all (head, block) pairs:
                # scores/exp lead PV by LAG blocks so the PE never waits on
                # the ACT exp chain at head starts.
                LAG = 4
                blocks = [(h, kb) for h in range(NQ) for kb in range(nkb)]
                head_ps = {}

                def emit_scores(h, kb):
                    kv = h // REP
                    d = kb * 128 - qc * TC
                    ks = slice(kb * 128, (kb + 1) * 128)
                    q0 = max(d, 0)
                    sc_ps = psp.tile([128, TC], dt.float32, tag="mm", bufs=2,
                                     name=f"sc_{tci}_{h}_{kb}")
                    nc.tensor.matmul(sc_ps[:, q0:TC], kt[kv][:, ks], qt[h][:, q0:TC],
                                     start=True, stop=True)
                    ph = scr.tile([128, TC], dt.bfloat16, tag="ph", bufs=LAG + 2,
                                  name=f"ph_{tci}_{h}_{kb}")
                    nc.scalar.activation(ph[:, q0:TC], sc_ps[:, q0:TC], AF.Exp,
                                         bias=0.0, scale=scale)
                    if d >= 0:
                        nc.vector.tensor_tensor(ph[:, q0:TC], ph[:, q0:TC],
                                                mask_t[d // 128][:, q0:TC],
                                                mybir.AluOpType.mult)
                    return ph

                def emit_pv(h, kb, ph):
                    kv = h // REP
                    vcol = kv * HD
                    d = kb * 128 - qc * TC
                    q0 = max(d, 0)
                    if kb == 0:
                        head_ps[h] = (
                            psp.tile([128, TC], dt.float32, tag="otps", bufs=3,
                                     name=f"ot_{tci}_{h}"),
                            psp.tile([1, TC], dt.float32, tag="sums", bufs=2,
                                     name=f"sum_{tci}_{h}"),
                        )
                    ot_ps, sum_ps = head_ps[h]
                    nc.tensor.matmul(
                        ot_ps[:, q0:TC], vt[kb][:, vcol:vcol + HD], ph[:, q0:TC],
                        start=(kb == 0), stop=(kb == nkb - 1),
                    )
                    nc.tensor.matmul(
                        sum_ps[:, q0:TC], ones_t[:], ph[:, q0:TC],
                        start=(kb == 0), stop=(kb == nkb - 1),
                    )
                    if kb == nkb - 1:
                        pending_norm.append((h, ot_ps, sum_ps))
                        if len(pending_norm) > 1:
                            emit_norm(*pending_norm.pop(0))

                probs_q = []
                for h, kb in blocks:
                    probs_q.append((h, kb, emit_scores(h, kb)))
                    if len(probs_q) > LAG:
                        hh, kk, ph = probs_q.pop(0)
                        emit_pv(hh, kk, ph)
                for hh, kk, ph in probs_q:
                    emit_pv(hh, kk, ph)

                # ---- output projection for token-chunk tci ----
                for db in range(DB):
                    ds_ = slice(db * 128, (db + 1) * 128)
                    ps = psp.tile([128, TC], dt.float32, tag="mm", bufs=2)
                    for c in range(CO):
                        # the last head's normalization drains here, covered
                        # by the first db's head-0..2 matmuls
                        if db == 0 and c == CO - 1 and pending_norm:
                            for args in pending_norm:
                                emit_norm(*args)
                            pending_norm = []
                        nc.tensor.matmul(
                            ps[:], wo_t[:, c * D + db * 128:c * D + (db + 1) * 128], ot[c][:],
                            start=(c == 0), stop=(c == CO - 1),
                        )
                    o3 = scr.tile([128, TC], dt.float32, tag="o3", bufs=2)
                    nc.scalar.copy(o3[:], ps[:])
                    if tci == NTC - 1:
                        eng = nc.sync if db % 2 == 0 else nc.gpsimd
                    else:
                        eng = nc.gpsimd
                    eng.dma_start(outT[ds_, ts], o3[:])

    return nc


# ---------------------------------------------------------------------------
# walrus in this container refuses >1 sem wait per instruction ("Too many
# sync wait commands"). Hoist excess waits onto same-engine NoOps inserted
# immediately before the instruction - program order on the engine queue
# preserves the sync semantics.
def split_multiwait_insts(nc, max_waits=1):
    n_split = 0
    for bb in nc.main_func.blocks:
        insts = bb.instructions
        i = 0
        while i < len(insts):
            ins = insts[i]
            si = getattr(ins, "sync_info", None)
            if si is not None and si.on_wait and len(si.on_wait) > max_waits:
                waits = list(si.on_wait)
                head, tail = waits[:-max_waits], waits[-max_waits:]
                nops = []
                for j in range(0, len(head), max_waits):
                    nop = mybir.InstNoOp(name=f"{ins.name}-ws{j}", ins=[], outs=[])
                    nop.engine = ins.engine
                    nop.sync_info = mybir.SyncInfo(
                        on_wait=head[j:j + max_waits], on_update=[])
                    nops.append(nop)
                ins.sync_info = mybir.SyncInfo(
                    on_wait=tail, on_update=list(si.on_update or []))
                insts[i:i] = nops
                i += len(nops)
                n_split += 1
            i += 1
    return n_split


# ---------------------------------------------------------------------------
# Host-side shard preparation / gather
BF16 = ml_dtypes.bfloat16


def rope_tables(S, HD):
    inv = 1.0 / (10000.0 ** (np.arange(0, HD, 2, dtype=np.float32) / HD))
    t = np.arange(S, dtype=np.float32)
    f = np.outer(t, inv).astype(np.float32)  # [S, HD//2]
    return np.ascontiguousarray(np.cos(f).T), np.ascontiguousarray(np.sin(f).T)


def causal_masks(TC):
    # masks[dd][k, qrel] = 1 if k + dd*128 <= qrel else 0
    out = np.zeros((4 * 128, TC), BF16)
    k = np.arange(128)[:, None]
    q = np.arange(TC)[None, :]
    for dd in range(4):
        out[dd * 128:(dd + 1) * 128] = (k + dd * 128 <= q).astype(BF16)
    return out


def rope_perm(HD):
    # new row i (i < HD//2) = old 2i; new row HD//2+i = old 2i+1
    return np.concatenate([np.arange(0, HD, 2), np.arange(1, HD, 2)])


def make_in_maps(x, wq, wk, wv, wo, *, n_batch_shards, n_head_shards,
                 NQ_TOT, NKV_TOT, HD, TC):
    """Returns list of in_maps, one per core (batch-major: core = b*G + g)."""
    B, S, D = x.shape
    G = n_head_shards
    NQ = NQ_TOT // G
    NKV = NKV_TOT // G
    perm = rope_perm(HD)
    cosT, sinT = rope_tables(S, HD)
    csT = np.concatenate([cosT, sinT], axis=0)  # [HD, S]
    masks = causal_masks(TC)

    # Per-batch xT (shared across head shards)
    xtb = {}
    for b in range(B):
        xtb[b] = np.ascontiguousarray(x[b].T).astype(BF16)  # [D, S]

    # Per-headgroup weight shards
    wshard = {}
    for g in range(G):
        qrows = slice(g * NQ * HD, (g + 1) * NQ * HD)
        kvrows = slice(g * NKV * HD, (g + 1) * NKV * HD)
        wq_g = wq[qrows, :].copy()      # [NQ*HD, D]
        wk_g = wk[kvrows, :].copy()
        wv_g = wv[kvrows, :].copy()
        # RoPE permutation of output rows, per head
        for hh in range(NQ):
            blk = wq_g[hh * HD:(hh + 1) * HD]
            wq_g[hh * HD:(hh + 1) * HD] = blk[perm]
        for hh in range(NKV):
            blk = wk_g[hh * HD:(hh + 1) * HD]
            wk_g[hh * HD:(hh + 1) * HD] = blk[perm]
        wqT = np.ascontiguousarray(wq_g.T).astype(BF16)   # [D, NQ*HD]
        wkT = np.ascontiguousarray(wk_g.T).astype(BF16)
        wvT = np.ascontiguousarray(wv_g.T).astype(BF16)
        woT = np.ascontiguousarray(wo[:, qrows].T).astype(BF16)  # [NQ*HD, D]
        wshard[g] = (wqT, wkT, wvT, woT)

    in_maps = []
    for b in range(n_batch_shards):
        for g in range(G):
            wqT, wkT, wvT, woT = wshard[g]
            in_maps.append({
                "xt": xtb[b],
                "wqp": wqT, "wkp": wkT, "wvp": wvT, "woh": woT,
                "csT": csT,
                "masks": masks,
            })
    return in_maps


def combine_outputs(outTs, B, G):
    """outTs: list of [D, S] partials, core order b*G+g. Returns [B, S, D]."""
    outs = []
    for b in range(B):
        acc = outTs[b * G].astype(np.float32).copy()
        for g in range(1, G):
            acc += outTs[b * G + g]
        outs.append(acc.T)  # [S, D]
    return np.stack(outs)


_NC_CACHE = {}


def _get_nc(S, D, NQ, NKV, HD, TC):
    key = (S, D, NQ, NKV, HD, TC)
    if key not in _NC_CACHE:
        nc = build_attention_nc(S=S, D=D, NQ=NQ, NKV=NKV, HD=HD, TC=TC)
        split_multiwait_insts(nc)
        _NC_CACHE[key] = nc
    return _NC_CACHE[key]


def kernel(**inputs):
    x = np.asarray(inputs["x"], dtype=np.float32)
    wq = np.asarray(inputs["wq"], dtype=np.float32)
    wk = np.asarray(inputs["wk"], dtype=np.float32)
    wv = np.asarray(inputs["wv"], dtype=np.float32)
    wo = np.asarray(inputs["wo"], dtype=np.float32)

    B, S, D = x.shape          # (2, 2048, 2048)
    NQ_TOT = wq.shape[0] // 128
    NKV_TOT = wk.shape[0] // 128
    HD = 128
    TC = 512
    G = 4                      # head shards
    NQ, NKV = NQ_TOT // G, NKV_TOT // G

    nc = _get_nc(S, D, NQ, NKV, HD, TC)
    in_maps = make_in_maps(
        x, wq, wk, wv, wo,
        n_batch_shards=B, n_head_shards=G,
        NQ_TOT=NQ_TOT, NKV_TOT=NKV_TOT, HD=HD, TC=TC,
    )

    from concourse.bass_utils import run_bass_kernel_spmd

    trace = os.environ.get("BASS_ATTN_TRACE") == "1"
    res = run_bass_kernel_spmd(nc, in_maps, list(range(len(in_maps))), trace=trace)
    kernel.last_results = res
    outTs = [r["outT"] for r in res.results]
    return combine_outputs(outTs, B, G).astype(np.float32)


# revision 15
# speedup vs baseline: 2.2784x; 1.2294x over previous
"""Trainium2 Bass kernel for nn_Attention_77043123355775.

Sharded GQA causal attention with RoPE: 8 NeuronCores as 2-way data
parallel (batch) x 4-way tensor parallel (heads). Each core computes its
4 Q heads / 2 KV heads for one batch entry and a partial output
projection (x[b] @ W)^T; the host sums the 4 partials per batch.

All matmuls run in plain bf16 with fp32 PSUM accumulation (the 2e-2
rel-err budget has ~20x headroom over bf16 rounding noise). Weights are
fully SBUF-resident, loaded once. Scores are computed transposed (k on
partitions) so the kernel needs no on-chip transposes.
"""
import math
import os
import sys

for _p in ("/opt/trn_rl_repo",):
    if _p not in sys.path:
        sys.path.insert(0, _p)

import ml_dtypes
import numpy as np

import concourse.bass as bass
import concourse.mybir as mybir
import concourse.tile as tile

dt = mybir.dt
AF = mybir.ActivationFunctionType


def build_attention_nc(S=2048, D=2048, NQ=4, NKV=2, HD=128, TC=512):
    assert HD == 128
    C = D // 128          # contraction chunks over features
    TB = S // 128         # 128-token blocks
    NTC = S // TC         # token chunks
    DB = D // 128         # output feature blocks
    CO = NQ * HD // 128   # contraction chunks for wo (= NQ)
    REP = NQ // NKV
    CQ = C // 4           # c-chunks per x quarter-tile
    scale = 1.0 / math.sqrt(HD)

    nc = bass.Bass()

    xt = nc.dram_tensor("xt", [D, S], dt.bfloat16, kind="ExternalInput")
    wqp = nc.dram_tensor("wqp", [D, NQ * HD], dt.bfloat16, kind="ExternalInput")
    wkp = nc.dram_tensor("wkp", [D, NKV * HD], dt.bfloat16, kind="ExternalInput")
    wvp = nc.dram_tensor("wvp", [D, NKV * HD], dt.bfloat16, kind="ExternalInput")
    woh = nc.dram_tensor("woh", [NQ * HD, D], dt.bfloat16, kind="ExternalInput")
    csT = nc.dram_tensor("csT", [HD, S], dt.bfloat16, kind="ExternalInput")
    masks = nc.dram_tensor("masks", [4 * 128, TC], dt.bfloat16, kind="ExternalInput")
    outT = nc.dram_tensor("outT", [D, S], dt.bfloat16, kind="ExternalOutput")

    with tile.TileContext(nc) as tc:
        with (
            tc.tile_pool(name="const", bufs=1) as constp,
            tc.tile_pool(name="tabs", bufs=1) as tabp,
            tc.tile_pool(name="wts", bufs=1) as wtp,
            tc.tile_pool(name="acts", bufs=1) as actp,
            tc.tile_pool(name="chunkacts", bufs=1) as cap,
            tc.tile_pool(name="xstream", bufs=8) as xsp,
            tc.tile_pool(name="scratch", bufs=3) as scr,
            tc.tile_pool(name="psum", bufs=1, space="PSUM") as psp,
        ):
            ones_t = constp.tile([128, 1], dt.bfloat16, tag="ones")
            nc.vector.memset(ones_t[:], 1.0)
            ones_row = constp.tile([1, 128], dt.bfloat16, tag="ones_row")
            nc.vector.memset(ones_row[:], 1.0)

            # ---- resident tables / weights ----
            # Weights load in c-quarter tiles so the first QKV chains start
            # after ~1MB instead of the full 6MB. sync ring: wq q0, x q0,
            # wq q1-3, wk, wv, wo. gpsimd ring: x q1-3, cs, masks.
            def wtile_quarters(name, src, ncols, nq):
                cq = C // nq
                tiles = []
                for g in range(nq):
                    t = wtp.tile([128, cq * ncols], dt.bfloat16, tag=f"{name}{g}")
                    tiles.append(t)
                return tiles

            def emit_w_dma(name, src, ncols, tiles, g):
                cq = C // len(tiles)
                rs = slice(g * cq * 128, (g + 1) * cq * 128)
                nc.sync.dma_start(
                    tiles[g].rearrange("p (c n) -> p c n", c=cq),
                    src[rs, :].rearrange("(c p) n -> p c n", p=128),
                )

            wq_ts = wtile_quarters("wq", wqp, NQ * HD, 4)
            wk_ts = wtile_quarters("wk", wkp, NKV * HD, 2)
            wv_ts = wtile_quarters("wv", wvp, NKV * HD, 2)

            def wq_sl(c, h):
                return wq_ts[c // CQ][:, (c % CQ) * NQ * HD + h * HD:(c % CQ) * NQ * HD + (h + 1) * HD]

            def wk_sl(c, h):
                ch = C // 2
                return wk_ts[c // ch][:, (c % ch) * NKV * HD + h * HD:(c % ch) * NKV * HD + (h + 1) * HD]

            def wv_sl(c):
                ch = C // 2
                return wv_ts[c // ch][:, (c % ch) * NKV * HD:(c % ch + 1) * NKV * HD]

            wo_t = wtp.tile([128, CO * D], dt.bfloat16, tag="wo")

            cs_t = tabp.tile([HD, S], dt.bfloat16, tag="cs")
            cos_t = cs_t[0:HD // 2, :]
            sin_t = cs_t[HD // 2:HD, :]
            mask_t = [tabp.tile([128, TC], dt.bfloat16, tag=f"mask{i}", name=f"mask{i}") for i in range(4)]

            # x quarter tiles per chunk; chunk 0 split across the two rings
            xq_tiles = {}

            def emit_x_dmas(tci, rings):
                ts_ = slice(tci * TC, (tci + 1) * TC)
                tiles = []
                for g in range(4):
                    rs = slice(g * CQ * 128, (g + 1) * CQ * 128)
                    t = xsp.tile([128, CQ * TC], dt.bfloat16, tag="xq",
                                 name=f"x_{tci}_{g}")
                    rings[g].dma_start(
                        t.rearrange("p (c n) -> p c n", c=CQ),
                        xt[rs, ts_].rearrange("(c p) n -> p c n", p=128),
                    )
                    tiles.append(t)
                xq_tiles[tci] = tiles

            # startup interleave: first matmul needs wq q0 + x q0 only
            emit_w_dma("wq", wqp, NQ * HD, wq_ts, 0)
            emit_x_dmas(0, [nc.sync, nc.gpsimd, nc.gpsimd, nc.gpsimd])
            for g in range(1, 4):
                emit_w_dma("wq", wqp, NQ * HD, wq_ts, g)
            nc.gpsimd.dma_start(cs_t[:], csT[:])
            for i in range(4):
                nc.gpsimd.dma_start(mask_t[i][:], masks[i * 128:(i + 1) * 128, :])
            for g in range(2):
                emit_w_dma("wk", wkp, NKV * HD, wk_ts, g)
            for g in range(2):
                emit_w_dma("wv", wvp, NKV * HD, wv_ts, g)
            nc.sync.dma_start(
                wo_t.rearrange("p (c n) -> p c n", c=CO),
                woh.rearrange("(c p) n -> p c n", p=128),
            )

            # K/V persist for the full sequence (written chunk by chunk)
            kt = [actp.tile([128, S], dt.bfloat16, tag=f"kt{h}", name=f"kt{h}") for h in range(NKV)]
            vt = [actp.tile([128, NKV * HD], dt.bfloat16, tag=f"vt{b}", name=f"vt{b}") for b in range(TB)]

            for tci in range(NTC):
                ts = slice(tci * TC, (tci + 1) * TC)
                qt = [cap.tile([128, TC], dt.bfloat16, tag=f"qt{h}", name=f"qt{h}_{tci}") for h in range(NQ)]
                ot = [cap.tile([128, TC], dt.bfloat16, tag=f"ot{h}", name=f"ot{h}_{tci}") for h in range(NQ)]

                # prefetch next chunk's x on the sync ring
                if tci + 1 < NTC:
                    emit_x_dmas(tci + 1, [nc.sync] * 4)

                xg = xq_tiles.pop(tci)

                def x_c(c):
                    return xg[c // CQ][:, (c % CQ) * TC:(c % CQ + 1) * TC]

                # ---- QKV projections + RoPE ----
                for h in range(NQ + NKV):
                    is_q = h < NQ
                    ps = psp.tile([128, TC], dt.float32, tag="mm", bufs=2)
                    if is_q:
                        wsl = [wq_sl(c, h) for c in range(C)]
                    else:
                        wsl = [wk_sl(c, h - NQ) for c in range(C)]
                    for c in range(C):
                        nc.tensor.matmul(
                            ps[:], wsl[c], x_c(c),
                            start=(c == 0), stop=(c == C - 1),
                        )
                    # RoPE from PSUM, bf16 intermediates, bf16 dest
                    rot = scr.tile([128, TC], dt.bfloat16, tag="rope", bufs=2)
                    t0 = scr.tile([128, TC], dt.bfloat16, tag="ropetmp", bufs=1)
                    cs = cos_t[:, ts]
                    sn = sin_t[:, ts]
                    xr = ps[0:64, :]
                    xi = ps[64:128, :]
                    dsth = qt[h][:] if is_q else kt[h - NQ][:, ts]
                    nc.vector.tensor_tensor(rot[0:64, :], xr, cs, mybir.AluOpType.mult)
                    nc.vector.tensor_tensor(t0[0:64, :], xi, sn, mybir.AluOpType.mult)
                    nc.vector.tensor_tensor(dsth[0:64, :], rot[0:64, :], t0[0:64, :], mybir.AluOpType.subtract)
                    nc.vector.tensor_tensor(rot[64:128, :], xr, sn, mybir.AluOpType.mult)
                    nc.vector.tensor_tensor(t0[64:128, :], xi, cs, mybir.AluOpType.mult)
                    nc.vector.tensor_tensor(dsth[64:128, :], rot[64:128, :], t0[64:128, :], mybir.AluOpType.add)

                # V projection
                for tb in range(TC // 128):
                    tbg = tci * (TC // 128) + tb
                    ps = psp.tile([128, NKV * HD], dt.float32, tag="mm", bufs=2)
                    for c in range(C):
                        xsl = x_c(c)[:, tb * 128:(tb + 1) * 128]
                        nc.tensor.matmul(
                            ps[:], xsl, wv_sl(c),
                            start=(c == 0), stop=(c == C - 1),
                        )
                    nc.scalar.copy(vt[tbg][:], ps[:])

                # ---- attention for q-chunk tci (keys 0..(tci+1)*TC) ----
                qc = tci
                nkb = (qc + 1) * (TC // 128)
                pending_norm = []

                def emit_norm(h, ot_ps, sum_ps):
                    # 1/x as exp(-ln(x)) on the scalar engine: InstReciprocal
                    # on DVE is ~13 cyc/elem on one lane (3.3us per [1,TC]).
                    lnt = scr.tile([1, TC], dt.float32, tag="lnt", bufs=2, name=f"ln_{tci}_{h}")
                    nc.scalar.activation(lnt[:], sum_ps[:], AF.Ln)
                    rec = scr.tile([1, TC], dt.float32, tag="rec", bufs=2, name=f"rec_{tci}_{h}")
                    nc.scalar.activation(rec[:], lnt[:], AF.Exp, bias=0.0, scale=-1.0)
                    rech = scr.tile([1, TC], dt.bfloat16, tag="rech", bufs=2, name=f"rech_{tci}_{h}")
                    recl = scr.tile([1, TC], dt.bfloat16, tag="recl", bufs=2, name=f"recl_{tci}_{h}")
                    nc.scalar.copy(rech[:], rec[:])
                    nc.vector.tensor_tensor(recl[:], rec[:], rech[:], mybir.AluOpType.subtract)
                    bc_ps = psp.tile([128, TC], dt.float32, tag="bcast", bufs=1, name=f"bc_{tci}_{h}")
                    nc.tensor.matmul(bc_ps[:], ones_row[:], rech[:], start=True, stop=False)
                    nc.tensor.matmul(bc_ps[:], ones_row[:], recl[:], start=False, stop=True)
                    recb = scr.tile([128, TC], dt.float32, tag="recb", bufs=2, name=f"recb_{tci}_{h}")
                    nc.scalar.copy(recb[:], bc_ps[:])
                    nc.vector.tensor_tensor(ot[h][:], ot_ps[:], recb[:], mybir.AluOpType.mult)

                # Two-stage software pipeline over all (head, block) pairs:
                # scores/exp lead PV by LAG blocks so the PE never waits on
                # the ACT exp chain at head starts.
                LAG = 4
                blocks = [(h, kb) for h in range(NQ) for kb in range(nkb)]
                head_ps = {}

                def emit_scores(h, kb):
                    kv = h // REP
                    d = kb * 128 - qc * TC
                    ks = slice(kb * 128, (kb + 1) * 128)
                    q0 = max(d, 0)
                    sc_ps = psp.tile([128, TC], dt.float32, tag="mm", bufs=2,
                                     name=f"sc_{tci}_{h}_{kb}")
                    nc.tensor.matmul(sc_ps[:, q0:TC], kt[kv][:, ks], qt[h][:, q0:TC],
                                     start=True, stop=True)
                    ph = scr.tile([128, TC], dt.bfloat16, tag="ph", bufs=LAG + 2,
                                  name=f"ph_{tci}_{h}_{kb}")
                    nc.scalar.activation(ph[:, q0:TC], sc_ps[:, q0:TC], AF.Exp,
                                         bias=0.0, scale=scale)
                    if d >= 0:
                        nc.vector.tensor_tensor(ph[:, q0:TC], ph[:, q0:TC],
                                                mask_t[d // 128][:, q0:TC],
                                                mybir.AluOpType.mult)
                    return ph

                def emit_pv(h, kb, ph):
                    kv = h // REP
                    vcol = kv * HD
                    d = kb * 128 - qc * TC
                    q0 = max(d, 0)
                    if kb == 0:
                        head_ps[h] = (
                            psp.tile([128, TC], dt.float32, tag="otps", bufs=3,
                                     name=f"ot_{tci}_{h}"),
                            psp.tile([1, TC], dt.float32, tag="sums", bufs=2,
                                     name=f"sum_{tci}_{h}"),
                        )
                    ot_ps, sum_ps = head_ps[h]
                    nc.tensor.matmul(
                        ot_ps[:, q0:TC], vt[kb][:, vcol:vcol + HD], ph[:, q0:TC],
                        start=(kb == 0), stop=(kb == nkb - 1),
                    )
                    nc.tensor.matmul(
                        sum_ps[:, q0:TC], ones_t[:], ph[:, q0:TC],
                        start=(kb == 0), stop=(kb == nkb - 1),
                    )
                    if kb == nkb - 1:
                        pending_norm.append((h, ot_ps, sum_ps))
                        if len(pending_norm) > 1:
                            emit_norm(*pending_norm.pop(0))

                probs_q = []
                for h, kb in blocks:
                    probs_q.append((h, kb, emit_scores(h, kb)))
                    if len(probs_q) > LAG:
                        hh, kk, ph = probs_q.pop(0)
                        emit_pv(hh, kk, ph)
                for hh, kk, ph in probs_q:
                    emit_pv(hh, kk, ph)

                # ---- output projection for token-chunk tci ----
                for db in range(DB):
                    ds_ = slice(db * 128, (db + 1) * 128)
                    ps = psp.tile([128, TC], dt.float32, tag="mm", bufs=2)
                    for c in range(CO):
                        # the last head's normalization drains here, covered
                        # by the first db's head-0..2 matmuls
                        if db == 0 and c == CO - 1 and pending_norm:
                            for args in pending_norm:
                                emit_norm(*args)
                            pending_norm = []
                        nc.tensor.matmul(
                            ps[:], wo_t[:, c * D + db * 128:c * D + (db + 1) * 128], ot[c][:],
                            start=(c == 0), stop=(c == CO - 1),
                        )
                    o3 = scr.tile([128, TC], dt.bfloat16, tag="o3", bufs=4)
                    nc.scalar.copy(o3[:], ps[:])
                    eng = nc.sync if db % 2 == 0 else nc.gpsimd
                    eng.dma_start(outT[ds_, ts], o3[:])

    return nc


# ---------------------------------------------------------------------------
# walrus in this container refuses >1 sem wait per instruction ("Too many
# sync wait commands"). Hoist excess waits onto same-engine NoOps inserted
# immediately before the instruction - program order on the engine queue
# preserves the sync semantics.
def split_multiwait_insts(nc, max_waits=1):
    n_split = 0
    for bb in nc.main_func.blocks:
        insts = bb.instructions
        i = 0
        while i < len(insts):
            ins = insts[i]
            si = getattr(ins, "sync_info", None)
            if si is not None and si.on_wait and len(si.on_wait) > max_waits:
                waits = list(si.on_wait)
                head, tail = waits[:-max_waits], waits[-max_waits:]
                nops = []
                for j in range(0, len(head), max_waits):
                    nop = mybir.InstNoOp(name=f"{ins.name}-ws{j}", ins=[], outs=[])
                    nop.engine = ins.engine
                    nop.sync_info = mybir.SyncInfo(
                        on_wait=head[j:j + max_waits], on_update=[])
                    nops.append(nop)
                ins.sync_info = mybir.SyncInfo(
                    on_wait=tail, on_update=list(si.on_update or []))
                insts[i:i] = nops
                i += len(nops)
                n_split += 1
            i += 1
    return n_split


# ---------------------------------------------------------------------------
# Host-side shard preparation / gather
BF16 = ml_dtypes.bfloat16


def rope_tables(S, HD):
    inv = 1.0 / (10000.0 ** (np.arange(0, HD, 2, dtype=np.float32) / HD))
    t = np.arange(S, dtype=np.float32)
    f = np.outer(t, inv).astype(np.float32)  # [S, HD//2]
    return np.ascontiguousarray(np.cos(f).T), np.ascontiguousarray(np.sin(f).T)


def causal_masks(TC):
    # masks[dd][k, qrel] = 1 if k + dd*128 <= qrel else 0
    out = np.zeros((4 * 128, TC), BF16)
    k = np.arange(128)[:, None]
    q = np.arange(TC)[None, :]
    for dd in range(4):
        out[dd * 128:(dd + 1) * 128] = (k + dd * 128 <= q).astype(BF16)
    return out


def rope_perm(HD):
    # new row i (i < HD//2) = old 2i; new row HD//2+i = old 2i+1
    return np.concatenate([np.arange(0, HD, 2), np.arange(1, HD, 2)])


def make_in_maps(x, wq, wk, wv, wo, *, n_batch_shards, n_head_shards,
                 NQ_TOT, NKV_TOT, HD, TC):
    """Returns list of in_maps, one per core (batch-major: core = b*G + g)."""
    B, S, D = x.shape
    G = n_head_shards
    NQ = NQ_TOT // G
    NKV = NKV_TOT // G
    perm = rope_perm(HD)
    cosT, sinT = rope_tables(S, HD)
    csT = np.concatenate([cosT, sinT], axis=0).astype(BF16)  # [HD, S]
    masks = causal_masks(TC)

    # Per-batch xT (shared across head shards)
    xtb = {}
    for b in range(B):
        xtb[b] = np.ascontiguousarray(x[b].T).astype(BF16)  # [D, S]

    # Per-headgroup weight shards
    wshard = {}
    for g in range(G):
        qrows = slice(g * NQ * HD, (g + 1) * NQ * HD)
        kvrows = slice(g * NKV * HD, (g + 1) * NKV * HD)
        wq_g = wq[qrows, :].copy()      # [NQ*HD, D]
        wk_g = wk[kvrows, :].copy()
        wv_g = wv[kvrows, :].copy()
        # RoPE permutation of output rows, per head
        for hh in range(NQ):
            blk = wq_g[hh * HD:(hh + 1) * HD]
            wq_g[hh * HD:(hh + 1) * HD] = blk[perm]
        for hh in range(NKV):
            blk = wk_g[hh * HD:(hh + 1) * HD]
            wk_g[hh * HD:(hh + 1) * HD] = blk[perm]
        wqT = np.ascontiguousarray(wq_g.T).astype(BF16)   # [D, NQ*HD]
        wkT = np.ascontiguousarray(wk_g.T).astype(BF16)
        wvT = np.ascontiguousarray(wv_g.T).astype(BF16)
        woT = np.ascontiguousarray(wo[:, qrows].T).astype(BF16)  # [NQ*HD, D]
        wshard[g] = (wqT, wkT, wvT, woT)

    in_maps = []
    for b in range(n_batch_shards):
        for g in range(G):
            wqT, wkT, wvT, woT = wshard[g]
            in_maps.append({
                "xt": xtb[b],
                "wqp": wqT, "wkp": wkT, "wvp": wvT, "woh": woT,
                "csT": csT,
                "masks": masks,
            })
    return in_maps


def combine_outputs(outTs, B, G):
    """outTs: list of [D, S] partials, core order b*G+g. Returns [B, S, D]."""
    outs = []
    for b in range(B):
        acc = outTs[b * G].astype(np.float32).copy()
        for g in range(1, G):
            acc += outTs[b * G + g]
        outs.append(acc.T)  # [S, D]
    return np.stack(outs)


_NC_CACHE = {}


def _get_nc(S, D, NQ, NKV, HD, TC):
    key = (S, D, NQ, NKV, HD, TC)
    if key not in _NC_CACHE:
        nc = build_attention_nc(S=S, D=D, NQ=NQ, NKV=NKV, HD=HD, TC=TC)
        split_multiwait_insts(nc)
        _NC_CACHE[key] = nc
    return _NC_CACHE[key]


def kernel(**inputs):
    x = np.asarray(inputs["x"], dtype=np.float32)
    wq = np.asarray(inputs["wq"], dtype=np.float32)
    wk = np.asarray(inputs["wk"], dtype=np.float32)
    wv = np.asarray(inputs["wv"], dtype=np.float32)
    wo = np.asarray(inputs["wo"], dtype=np.float32)

    B, S, D = x.shape          # (2, 2048, 2048)
    NQ_TOT = wq.shape[0] // 128
    NKV_TOT = wk.shape[0] // 128
    HD = 128
    TC = 512
    G = 4                      # head shards
    NQ, NKV = NQ_TOT // G, NKV_TOT // G

    nc = _get_nc(S, D, NQ, NKV, HD, TC)
    in_maps = make_in_maps(
        x, wq, wk, wv, wo,
        n_batch_shards=B, n_head_shards=G,
        NQ_TOT=NQ_TOT, NKV_TOT=NKV_TOT, HD=HD, TC=TC,
    )

    from concourse.bass_utils import run_bass_kernel_spmd

    trace = os.environ.get("BASS_ATTN_TRACE") == "1"
    res = run_bass_kernel_spmd(nc, in_maps, list(range(len(in_maps))), trace=trace)
    kernel.last_results = res
    outTs = [r["outT"] for r in res.results]
    return combine_outputs(outTs, B, G).astype(np.float32)


# revision 21
# speedup vs baseline: 2.4835x; 1.0900x over previous
"""Trainium2 Bass kernel for nn_Attention_77043123355775.

Sharded GQA causal attention with RoPE: 8 NeuronCores as 2-way data
parallel (batch) x 4-way tensor parallel (heads). Each core computes its
4 Q heads / 2 KV heads for one batch entry and a partial output
projection (x[b] @ W)^T; the host sums the 4 partials per batch.

All matmuls run in plain bf16 with fp32 PSUM accumulation (the 2e-2
rel-err budget has ~20x headroom over bf16 rounding noise). Weights are
fully SBUF-resident. The PE instruction stream interleaves next-chunk
QKV/V projection chains and prev-chunk output-projection groups between
attention blocks, so the PE never waits on the scalar-engine exp chain
and stays in its high DVFS p-state.
"""
import math
import os
import sys

for _p in ("/opt/trn_rl_repo",):
    if _p not in sys.path:
        sys.path.insert(0, _p)

import ml_dtypes
import numpy as np

import concourse.bass as bass
import concourse.mybir as mybir
import concourse.tile as tile

dt = mybir.dt
AF = mybir.ActivationFunctionType


def build_attention_nc(S=2048, D=2048, NQ=4, NKV=2, HD=128, TC=512):
    assert HD == 128
    C = D // 128          # contraction chunks over features
    TB = S // 128         # 128-token blocks
    NTC = S // TC         # token chunks
    DB = D // 128         # output feature blocks
    CO = NQ * HD // 128   # contraction chunks for wo (= NQ)
    REP = NQ // NKV
    CQ = C // 4           # c-chunks per x quarter-tile
    NTB = TC // 128       # token blocks per chunk
    scale = 1.0 / math.sqrt(HD)

    nc = bass.Bass()

    xt = nc.dram_tensor("xt", [D, S], dt.bfloat16, kind="ExternalInput")
    wqp = nc.dram_tensor("wqp", [D, NQ * HD], dt.bfloat16, kind="ExternalInput")
    wkp = nc.dram_tensor("wkp", [D, NKV * HD], dt.bfloat16, kind="ExternalInput")
    wvp = nc.dram_tensor("wvp", [D, NKV * HD], dt.bfloat16, kind="ExternalInput")
    woh = nc.dram_tensor("woh", [NQ * HD, D], dt.bfloat16, kind="ExternalInput")
    csT = nc.dram_tensor("csT", [HD, S], dt.bfloat16, kind="ExternalInput")
    masks = nc.dram_tensor("masks", [4 * 128, TC], dt.bfloat16, kind="ExternalInput")
    outT = nc.dram_tensor("outT", [D, S], dt.bfloat16, kind="ExternalOutput")

    with tile.TileContext(nc) as tc:
        with (
            tc.tile_pool(name="const", bufs=1) as constp,
            tc.tile_pool(name="tabs", bufs=1) as tabp,
            tc.tile_pool(name="wts", bufs=1) as wtp,
            tc.tile_pool(name="acts", bufs=1) as actp,
            tc.tile_pool(name="chunkacts", bufs=1) as cap,
            tc.tile_pool(name="xstream", bufs=8) as xsp,
            tc.tile_pool(name="scratch", bufs=3) as scr,
            tc.tile_pool(name="psum", bufs=1, space="PSUM") as psp,
        ):
            ones_t = constp.tile([128, 1], dt.bfloat16, tag="ones")
            nc.vector.memset(ones_t[:], 1.0)
            ones_row = constp.tile([1, 128], dt.bfloat16, tag="ones_row")
            nc.vector.memset(ones_row[:], 1.0)

            # ---- resident tables / weights (c-quartered for fine deps) ----
            def emit_w_dma(tiles, src, g, ring):
                cq = C // len(tiles)
                rs = slice(g * cq * 128, (g + 1) * cq * 128)
                ring.dma_start(
                    tiles[g].rearrange("p (c n) -> p c n", c=cq),
                    src[rs, :].rearrange("(c p) n -> p c n", p=128),
                )

            wq_ts = [wtp.tile([128, CQ * NQ * HD], dt.bfloat16, tag=f"wq{g}", name=f"wq{g}") for g in range(4)]
            wk_ts = [wtp.tile([128, (C // 2) * NKV * HD], dt.bfloat16, tag=f"wk{g}", name=f"wk{g}") for g in range(2)]
            wv_ts = [wtp.tile([128, (C // 2) * NKV * HD], dt.bfloat16, tag=f"wv{g}", name=f"wv{g}") for g in range(2)]
            wo_t = wtp.tile([128, CO * D], dt.bfloat16, tag="wo")

            def wq_sl(c, h):
                return wq_ts[c // CQ][:, (c % CQ) * NQ * HD + h * HD:(c % CQ) * NQ * HD + (h + 1) * HD]

            def wk_sl(c, h):
                ch = C // 2
                return wk_ts[c // ch][:, (c % ch) * NKV * HD + h * HD:(c % ch) * NKV * HD + (h + 1) * HD]

            def wv_sl(c):
                ch = C // 2
                return wv_ts[c // ch][:, (c % ch) * NKV * HD:(c % ch + 1) * NKV * HD]

            cs_t = tabp.tile([HD, S], dt.bfloat16, tag="cs")
            cos_t = cs_t[0:HD // 2, :]
            sin_t = cs_t[HD // 2:HD, :]
            mask_t = [tabp.tile([128, TC], dt.bfloat16, tag=f"mask{i}", name=f"mask{i}") for i in range(4)]

            xq_tiles = {}

            def emit_x_dmas(tci, rings):
                ts_ = slice(tci * TC, (tci + 1) * TC)
                tiles = []
                for g in range(4):
                    rs = slice(g * CQ * 128, (g + 1) * CQ * 128)
                    t = xsp.tile([128, CQ * TC], dt.bfloat16, tag="xq",
                                 name=f"x_{tci}_{g}")
                    rings[g].dma_start(
                        t.rearrange("p (c n) -> p c n", c=CQ),
                        xt[rs, ts_].rearrange("(c p) n -> p c n", p=128),
                    )
                    tiles.append(t)
                xq_tiles[tci] = tiles

            # startup: first QKV chain needs wq q0 + x q0 only. Spread the
            # burst over sync + scalar + gpsimd rings (scalar is idle here).
            emit_w_dma(wq_ts, wqp, 0, nc.sync)
            emit_x_dmas(0, [nc.sync, nc.scalar, nc.gpsimd, nc.gpsimd])
            emit_w_dma(wq_ts, wqp, 1, nc.sync)
            emit_w_dma(wq_ts, wqp, 2, nc.sync)
            emit_w_dma(wq_ts, wqp, 3, nc.sync)
            emit_w_dma(wk_ts, wkp, 0, nc.scalar)
            emit_w_dma(wk_ts, wkp, 1, nc.scalar)
            nc.gpsimd.dma_start(cs_t[:], csT[:])
            emit_w_dma(wv_ts, wvp, 0, nc.scalar)
            emit_w_dma(wv_ts, wvp, 1, nc.scalar)
            for i in range(4):
                nc.gpsimd.dma_start(mask_t[i][:], masks[i * 128:(i + 1) * 128, :])
            nc.sync.dma_start(
                wo_t.rearrange("p (c n) -> p c n", c=CO),
                woh.rearrange("(c p) n -> p c n", p=128),
            )

            # K/V persist per 512-chunk / 128-block (no cross-chunk tiles,
            # so interleaved next-chunk RoPE writes never alias attention
            # reads at the dep tracker's granularity)
            ktc = [[actp.tile([128, TC], dt.bfloat16, tag=f"kt{h}_{j}", name=f"kt{h}_{j}")
                    for j in range(NTC)] for h in range(NKV)]
            vt = [actp.tile([128, NKV * HD], dt.bfloat16, tag=f"vt{b}", name=f"vt{b}") for b in range(TB)]
            qt_all = {}
            ot_all = {}
            for tci in range(NTC):
                qt_all[tci] = [cap.tile([128, TC], dt.bfloat16, tag=f"qt{h}_{tci % 2}", name=f"qt{h}_{tci}") for h in range(NQ)]
                ot_all[tci] = [cap.tile([128, TC], dt.bfloat16, tag=f"ot{h}_{tci % 2}", name=f"ot{h}_{tci}") for h in range(NQ)]

            # ---------------- unit generators ----------------
            def rope_epilogue(tci, h, ps):
                ts_ = slice(tci * TC, (tci + 1) * TC)
                rot = scr.tile([128, TC], dt.bfloat16, tag="rope", bufs=2)
                t0 = scr.tile([128, TC], dt.bfloat16, tag="ropetmp", bufs=1)
                cs = cos_t[:, ts_]
                sn = sin_t[:, ts_]
                xr = ps[0:64, :]
                xi = ps[64:128, :]
                dsth = qt_all[tci][h][:] if h < NQ else ktc[h - NQ][tci][:]
                nc.vector.tensor_tensor(rot[0:64, :], xr, cs, mybir.AluOpType.mult)
                nc.vector.tensor_tensor(t0[0:64, :], xi, sn, mybir.AluOpType.mult)
                nc.vector.tensor_tensor(dsth[0:64, :], rot[0:64, :], t0[0:64, :], mybir.AluOpType.subtract)
                nc.vector.tensor_tensor(rot[64:128, :], xr, sn, mybir.AluOpType.mult)
                nc.vector.tensor_tensor(t0[64:128, :], xi, cs, mybir.AluOpType.mult)
                nc.vector.tensor_tensor(dsth[64:128, :], rot[64:128, :], t0[64:128, :], mybir.AluOpType.add)

            def qkv_units(tci, ptag, pbufs, group=4):
                """Closures emitting `group` matmuls of a QKV/V chain each
                (coarser units keep chain psum lifetimes short)."""
                units = []
                state = {}

                def x_c(c):
                    xg = xq_tiles[tci]
                    return xg[c // CQ][:, (c % CQ) * TC:(c % CQ + 1) * TC]

                def qk_seg(h, c0):
                    if c0 == 0:
                        state[h] = psp.tile([128, TC], dt.float32, tag=ptag,
                                            bufs=pbufs, name=f"qkv_{tci}_{h}")
                    ps = state[h]
                    for c in range(c0, c0 + group):
                        wsl = wq_sl(c, h) if h < NQ else wk_sl(c, h - NQ)
                        nc.tensor.matmul(ps[:], wsl, x_c(c),
                                         start=(c == 0), stop=(c == C - 1))
                    if c0 + group == C:
                        rope_epilogue(tci, h, ps)

                def v_seg(tb, c0):
                    key = "v", tb
                    if c0 == 0:
                        state[key] = psp.tile([128, NKV * HD], dt.float32,
                                              tag=ptag, bufs=pbufs,
                                              name=f"v_{tci}_{tb}")
                    ps = state[key]
                    for c in range(c0, c0 + group):
                        nc.tensor.matmul(ps[:], x_c(c)[:, tb * 128:(tb + 1) * 128],
                                         wv_sl(c),
                                         start=(c == 0), stop=(c == C - 1))
                    if c0 + group == C:
                        nc.scalar.copy(vt[tci * NTB + tb][:], ps[:])

                for h in range(NQ + NKV):
                    for c0 in range(0, C, group):
                        units.append(lambda h=h, c0=c0: qk_seg(h, c0))
                for tb in range(NTB):
                    for c0 in range(0, C, group):
                        units.append(lambda tb=tb, c0=c0: v_seg(tb, c0))
                return units

            def op_units(tci, ptag, pbufs):
                """One closure per output-projection db group (4 matmuls +
                copy + store)."""
                ts_ = slice(tci * TC, (tci + 1) * TC)
                ot = ot_all[tci]
                units = []
                for db in range(DB):
                    def u(db=db):
                        ps = psp.tile([128, TC], dt.float32, tag=ptag, bufs=pbufs,
                                      name=f"op_{tci}_{db}")
                        for c in range(CO):
                            nc.tensor.matmul(
                                ps[:], wo_t[:, c * D + db * 128:c * D + (db + 1) * 128],
                                ot[c][:],
                                start=(c == 0), stop=(c == CO - 1),
                            )
                        o3 = scr.tile([128, TC], dt.bfloat16, tag="o3", bufs=4)
                        nc.scalar.copy(o3[:], ps[:])
                        eng = nc.sync if db % 2 == 0 else nc.gpsimd
                        eng.dma_start(outT[db * 128:(db + 1) * 128, ts_], o3[:])
                    units.append(u)
                return units

            # ---------------- attention emission ----------------
            def emit_attention(tci, filler):
                """Emits scores/exp/PV blocks for q-chunk tci, draining
                `filler` closures between blocks to keep the PE busy."""
                qc = tci
                qt = qt_all[tci]
                ot = ot_all[tci]
                nkb = (qc + 1) * NTB
                blocks = [(h, kb) for h in range(NQ) for kb in range(nkb)]
                nb = len(blocks)
                head_ps = {}
                pending_a = []   # (h, ot_ps, sum_ps) awaiting norm part b
                LAG = 4

                def norm_a(h):
                    _, sum_ps = head_ps[h]
                    # copy psum->sbuf on the (lightly loaded) vector queue so
                    # the single sums bank frees before the next head needs it
                    sumc = scr.tile([1, TC], dt.float32, tag="sumc", bufs=2, name=f"sumc_{tci}_{h}")
                    nc.vector.tensor_copy(sumc[:], sum_ps[:])
                    lnt = scr.tile([1, TC], dt.float32, tag="lnt", bufs=2, name=f"ln_{tci}_{h}")
                    nc.scalar.activation(lnt[:], sumc[:], AF.Ln)
                    rec = scr.tile([1, TC], dt.float32, tag="rec", bufs=2, name=f"rec_{tci}_{h}")
                    nc.scalar.activation(rec[:], lnt[:], AF.Exp, bias=0.0, scale=-1.0)
                    rech = scr.tile([1, TC], dt.bfloat16, tag="rech", bufs=2, name=f"rech_{tci}_{h}")
                    recl = scr.tile([1, TC], dt.bfloat16, tag="recl", bufs=2, name=f"recl_{tci}_{h}")
                    nc.scalar.copy(rech[:], rec[:])
                    nc.vector.tensor_tensor(recl[:], rec[:], rech[:], mybir.AluOpType.subtract)
                    return rech, recl

                def norm_b(h, rech, recl):
                    ot_ps, _ = head_ps[h]
                    bc_ps = psp.tile([128, TC], dt.float32, tag="op", bufs=1, name=f"bc_{tci}_{h}")
                    nc.tensor.matmul(bc_ps[:], ones_row[:], rech[:], start=True, stop=False)
                    nc.tensor.matmul(bc_ps[:], ones_row[:], recl[:], start=False, stop=True)
                    recb = scr.tile([128, TC], dt.float32, tag="recb", bufs=2, name=f"recb_{tci}_{h}")
                    nc.scalar.copy(recb[:], bc_ps[:])
                    nc.vector.tensor_tensor(ot[h][:], ot_ps[:], recb[:], mybir.AluOpType.mult)

                def emit_scores(h, kb):
                    kv = h // REP
                    d = kb * 128 - qc * TC
                    q0 = max(d, 0)
                    sc_ps = psp.tile([128, TC], dt.float32, tag="sc", bufs=2,
                                     name=f"sc_{tci}_{h}_{kb}")
                    ksl = ktc[kv][kb // NTB][:, (kb % NTB) * 128:(kb % NTB + 1) * 128]
                    nc.tensor.matmul(sc_ps[:, q0:TC], ksl, qt[h][:, q0:TC],
                                     start=True, stop=True)
                    ph = scr.tile([128, TC], dt.bfloat16, tag="ph", bufs=LAG + 2,
                                  name=f"ph_{tci}_{h}_{kb}")
                    nc.scalar.activation(ph[:, q0:TC], sc_ps[:, q0:TC], AF.Exp,
                                         bias=0.0, scale=scale)
                    if d >= 0:
                        nc.vector.tensor_tensor(ph[:, q0:TC], ph[:, q0:TC],
                                                mask_t[d // 128][:, q0:TC],
                                                mybir.AluOpType.mult)
                    return ph

                def emit_pv(h, kb, ph):
                    kv = h // REP
                    vcol = kv * HD
                    d = kb * 128 - qc * TC
                    q0 = max(d, 0)
                    if kb == 0:
                        head_ps[h] = (
                            psp.tile([128, TC], dt.float32, tag="otps", bufs=2,
                                     name=f"ot_{tci}_{h}"),
                            psp.tile([1, TC], dt.float32, tag="sums", bufs=1,
                                     name=f"sum_{tci}_{h}"),
                        )
                    ot_ps, sum_ps = head_ps[h]
                    nc.tensor.matmul(
                        ot_ps[:, q0:TC], vt[kb][:, vcol:vcol + HD], ph[:, q0:TC],
                        start=(kb == 0), stop=(kb == nkb - 1),
                    )
                    nc.tensor.matmul(
                        sum_ps[:, q0:TC], ones_t[:], ph[:, q0:TC],
                        start=(kb == 0), stop=(kb == nkb - 1),
                    )
                    if kb == nkb - 1:
                        pending_a.append((h, *norm_a(h)))

                # filler pacing: spread all filler units evenly over blocks
                nf = len(filler)
                drained = 0
                probs_q = []
                for bi, (h, kb) in enumerate(blocks):
                    probs_q.append((h, kb, emit_scores(h, kb)))
                    # norms deferred by one block so scalar/vector prep is
                    # ready before the PE broadcast matmul
                    if len(pending_a) > 1 or (pending_a and kb >= 2):
                        norm_b(*pending_a.pop(0))
                    want = (bi + 1) * nf // nb
                    while drained < want:
                        filler[drained]()
                        drained += 1
                    if len(probs_q) > LAG:
                        hh, kk, ph = probs_q.pop(0)
                        emit_pv(hh, kk, ph)
                for hh, kk, ph in probs_q:
                    emit_pv(hh, kk, ph)
                while drained < nf:
                    filler[drained]()
                    drained += 1
                for args in pending_a:
                    norm_b(*args)

            # ---------------- schedule ----------------
            # QKV0 standalone; attn(t) interleaves QKV(t+1) and outproj(t-1);
            # outproj(NTC-1) standalone.
            for u in qkv_units(0, "sc", 2):
                u()
            for tci in range(NTC):
                if tci + 1 < NTC:
                    emit_x_dmas(tci + 1, [nc.sync, nc.sync, nc.gpsimd, nc.gpsimd])
                filler = []
                if tci > 0:
                    filler += op_units(tci - 1, "op", 1)
                if tci + 1 < NTC:
                    filler += qkv_units(tci + 1, "fill", 2)
                emit_attention(tci, filler)
                xq_tiles.pop(tci)
            for u in op_units(NTC - 1, "sc", 2):
                u()

    return nc


# ---------------------------------------------------------------------------
# walrus in this container refuses >1 sem wait per instruction ("Too many
# sync wait commands"). Hoist excess waits onto same-engine NoOps inserted
# immediately before the instruction - program order on the engine queue
# preserves the sync semantics.
def split_multiwait_insts(nc, max_waits=1):
    n_split = 0
    for bb in nc.main_func.blocks:
        insts = bb.instructions
        i = 0
        while i < len(insts):
            ins = insts[i]
            si = getattr(ins, "sync_info", None)
            if si is not None and si.on_wait and len(si.on_wait) > max_waits:
                waits = list(si.on_wait)
                head, tail = waits[:-max_waits], waits[-max_waits:]
                nops = []
                for j in range(0, len(head), max_waits):
                    nop = mybir.InstNoOp(name=f"{ins.name}-ws{j}", ins=[], outs=[])
                    nop.engine = ins.engine
                    nop.sync_info = mybir.SyncInfo(
                        on_wait=head[j:j + max_waits], on_update=[])
                    nops.append(nop)
                ins.sync_info = mybir.SyncInfo(
                    on_wait=tail, on_update=list(si.on_update or []))
                insts[i:i] = nops
                i += len(nops)
                n_split += 1
            i += 1
    return n_split


# ---------------------------------------------------------------------------
# Host-side shard preparation / gather
BF16 = ml_dtypes.bfloat16


def rope_tables(S, HD):
    inv = 1.0 / (10000.0 ** (np.arange(0, HD, 2, dtype=np.float32) / HD))
    t = np.arange(S, dtype=np.float32)
    f = np.outer(t, inv).astype(np.float32)  # [S, HD//2]
    return np.ascontiguousarray(np.cos(f).T), np.ascontiguousarray(np.sin(f).T)


def causal_masks(TC):
    # masks[dd][k, qrel] = 1 if k + dd*128 <= qrel else 0
    out = np.zeros((4 * 128, TC), BF16)
    k = np.arange(128)[:, None]
    q = np.arange(TC)[None, :]
    for dd in range(4):
        out[dd * 128:(dd + 1) * 128] = (k + dd * 128 <= q).astype(BF16)
    return out


def rope_perm(HD):
    # new row i (i < HD//2) = old 2i; new row HD//2+i = old 2i+1
    return np.concatenate([np.arange(0, HD, 2), np.arange(1, HD, 2)])


def make_in_maps(x, wq, wk, wv, wo, *, n_batch_shards, n_head_shards,
                 NQ_TOT, NKV_TOT, HD, TC):
    """Returns list of in_maps, one per core (batch-major: core = b*G + g)."""
    B, S, D = x.shape
    G = n_head_shards
    NQ = NQ_TOT // G
    NKV = NKV_TOT // G
    perm = rope_perm(HD)
    cosT, sinT = rope_tables(S, HD)
    csT = np.concatenate([cosT, sinT], axis=0).astype(BF16)  # [HD, S]
    masks = causal_masks(TC)

    # Per-batch xT (shared across head shards)
    xtb = {}
    for b in range(B):
        xtb[b] = np.ascontiguousarray(x[b].T).astype(BF16)  # [D, S]

    # Per-headgroup weight shards
    wshard = {}
    for g in range(G):
        qrows = slice(g * NQ * HD, (g + 1) * NQ * HD)
        kvrows = slice(g * NKV * HD, (g + 1) * NKV * HD)
        wq_g = wq[qrows, :].copy()      # [NQ*HD, D]
        wk_g = wk[kvrows, :].copy()
        wv_g = wv[kvrows, :].copy()
        # RoPE permutation of output rows, per head
        for hh in range(NQ):
            blk = wq_g[hh * HD:(hh + 1) * HD]
            wq_g[hh * HD:(hh + 1) * HD] = blk[perm]
        for hh in range(NKV):
            blk = wk_g[hh * HD:(hh + 1) * HD]
            wk_g[hh * HD:(hh + 1) * HD] = blk[perm]
        wqT = np.ascontiguousarray(wq_g.T).astype(BF16)   # [D, NQ*HD]
        wkT = np.ascontiguousarray(wk_g.T).astype(BF16)
        wvT = np.ascontiguousarray(wv_g.T).astype(BF16)
        woT = np.ascontiguousarray(wo[:, qrows].T).astype(BF16)  # [NQ*HD, D]
        wshard[g] = (wqT, wkT, wvT, woT)

    in_maps = []
    for b in range(n_batch_shards):
        for g in range(G):
            wqT, wkT, wvT, woT = wshard[g]
            in_maps.append({
                "xt": xtb[b],
                "wqp": wqT, "wkp": wkT, "wvp": wvT, "woh": woT,
                "csT": csT,
                "masks": masks,
            })
    return in_maps


def combine_outputs(outTs, B, G):
    """outTs: list of [D, S] partials, core order b*G+g. Returns [B, S, D]."""
    outs = []
    for b in range(B):
        acc = outTs[b * G].astype(np.float32).copy()
        for g in range(1, G):
            acc += outTs[b * G + g]
        outs.append(acc.T)  # [S, D]
    return np.stack(outs)


_NC_CACHE = {}


def _get_nc(S, D, NQ, NKV, HD, TC):
    key = (S, D, NQ, NKV, HD, TC)
    if key not in _NC_CACHE:
        nc = build_attention_nc(S=S, D=D, NQ=NQ, NKV=NKV, HD=HD, TC=TC)
        split_multiwait_insts(nc)
        _NC_CACHE[key] = nc
    return _NC_CACHE[key]


def kernel(**inputs):
    x = np.asarray(inputs["x"], dtype=np.float32)
    wq = np.asarray(inputs["wq"], dtype=np.float32)
    wk = np.asarray(inputs["wk"], dtype=np.float32)
    wv = np.asarray(inputs["wv"], dtype=np.float32)
    wo = np.asarray(inputs["wo"], dtype=np.float32)

    B, S, D = x.shape          # (2, 2048, 2048)
    NQ_TOT = wq.shape[0] // 128
    NKV_TOT = wk.shape[0] // 128
    HD = 128
    TC = 512
    G = 4                      # head shards
    NQ, NKV = NQ_TOT // G, NKV_TOT // G

    nc = _get_nc(S, D, NQ, NKV, HD, TC)
    in_maps = make_in_maps(
        x, wq, wk, wv, wo,
        n_batch_shards=B, n_head_shards=G,
        NQ_TOT=NQ_TOT, NKV_TOT=NKV_TOT, HD=HD, TC=TC,
    )

    from concourse.bass_utils import run_bass_kernel_spmd

    trace = os.environ.get("BASS_ATTN_TRACE") == "1"
    res = run_bass_kernel_spmd(nc, in_maps, list(range(len(in_maps))), trace=trace)
    kernel.last_results = res
    outTs = [r["outT"] for r in res.results]
    return combine_outputs(outTs, B, G).astype(np.float32)


# revision 23
# speedup vs baseline: 2.5140x; 1.0123x over previous
"""Trainium2 Bass kernel for nn_Attention_77043123355775.

Sharded GQA causal attention with RoPE: 8 NeuronCores as 2-way data
parallel (batch) x 4-way tensor parallel (heads). Each core computes its
4 Q heads / 2 KV heads for one batch entry and a partial output
projection (x[b] @ W)^T; the host sums the 4 partials per batch.

All matmuls run in plain bf16 with fp32 PSUM accumulation (the 2e-2
rel-err budget has ~20x headroom over bf16 rounding noise). Weights are
fully SBUF-resident. The PE instruction stream interleaves next-chunk
QKV/V projection chains and prev-chunk output-projection groups between
attention blocks, so the PE never waits on the scalar-engine exp chain
and stays in its high DVFS p-state.
"""
import math
import os
import sys

for _p in ("/opt/trn_rl_repo",):
    if _p not in sys.path:
        sys.path.insert(0, _p)

import ml_dtypes
import numpy as np

import concourse.bass as bass
import concourse.mybir as mybir
import concourse.tile as tile

dt = mybir.dt
AF = mybir.ActivationFunctionType


def build_attention_nc(S=2048, D=2048, NQ=4, NKV=2, HD=128, TC=512):
    assert HD == 128
    C = D // 128          # contraction chunks over features
    TB = S // 128         # 128-token blocks
    NTC = S // TC         # token chunks
    DB = D // 128         # output feature blocks
    CO = NQ * HD // 128   # contraction chunks for wo (= NQ)
    REP = NQ // NKV
    CQ = C // 4           # c-chunks per x quarter-tile
    NTB = TC // 128       # token blocks per chunk
    scale = 1.0 / math.sqrt(HD)

    nc = bass.Bass()

    xt = nc.dram_tensor("xt", [D, S], dt.bfloat16, kind="ExternalInput")
    wqp = nc.dram_tensor("wqp", [D, NQ * HD], dt.bfloat16, kind="ExternalInput")
    wkp = nc.dram_tensor("wkp", [D, NKV * HD], dt.bfloat16, kind="ExternalInput")
    wvp = nc.dram_tensor("wvp", [D, NKV * HD], dt.bfloat16, kind="ExternalInput")
    woh = nc.dram_tensor("woh", [NQ * HD, D], dt.bfloat16, kind="ExternalInput")
    csT = nc.dram_tensor("csT", [HD, S], dt.bfloat16, kind="ExternalInput")
    masks = nc.dram_tensor("masks", [4 * 128, TC], dt.bfloat16, kind="ExternalInput")
    outT = nc.dram_tensor("outT", [D, S], dt.bfloat16, kind="ExternalOutput")

    with tile.TileContext(nc) as tc:
        with (
            tc.tile_pool(name="const", bufs=1) as constp,
            tc.tile_pool(name="tabs", bufs=1) as tabp,
            tc.tile_pool(name="wts", bufs=1) as wtp,
            tc.tile_pool(name="acts", bufs=1) as actp,
            tc.tile_pool(name="chunkacts", bufs=1) as cap,
            tc.tile_pool(name="xstream", bufs=8) as xsp,
            tc.tile_pool(name="scratch", bufs=3) as scr,
            tc.tile_pool(name="psum", bufs=1, space="PSUM") as psp,
        ):
            ones_t = constp.tile([128, 1], dt.bfloat16, tag="ones")
            nc.vector.memset(ones_t[:], 1.0)
            ones_row = constp.tile([1, 128], dt.bfloat16, tag="ones_row")
            nc.vector.memset(ones_row[:], 1.0)

            # ---- resident tables / weights (c-quartered for fine deps) ----
            def emit_w_dma(tiles, src, g, ring):
                cq = C // len(tiles)
                rs = slice(g * cq * 128, (g + 1) * cq * 128)
                ring.dma_start(
                    tiles[g].rearrange("p (c n) -> p c n", c=cq),
                    src[rs, :].rearrange("(c p) n -> p c n", p=128),
                )

            wq_ts = [wtp.tile([128, CQ * NQ * HD], dt.bfloat16, tag=f"wq{g}", name=f"wq{g}") for g in range(4)]
            wk_ts = [wtp.tile([128, (C // 2) * NKV * HD], dt.bfloat16, tag=f"wk{g}", name=f"wk{g}") for g in range(2)]
            wv_ts = [wtp.tile([128, (C // 2) * NKV * HD], dt.bfloat16, tag=f"wv{g}", name=f"wv{g}") for g in range(2)]
            wo_t = wtp.tile([128, CO * D], dt.bfloat16, tag="wo")

            def wq_sl(c, h):
                return wq_ts[c // CQ][:, (c % CQ) * NQ * HD + h * HD:(c % CQ) * NQ * HD + (h + 1) * HD]

            def wk_sl(c, h):
                ch = C // 2
                return wk_ts[c // ch][:, (c % ch) * NKV * HD + h * HD:(c % ch) * NKV * HD + (h + 1) * HD]

            def wv_sl(c):
                ch = C // 2
                return wv_ts[c // ch][:, (c % ch) * NKV * HD:(c % ch + 1) * NKV * HD]

            cs_t = tabp.tile([HD, S], dt.bfloat16, tag="cs")
            cos_t = cs_t[0:HD // 2, :]
            sin_t = cs_t[HD // 2:HD, :]
            mask_t = [tabp.tile([128, TC], dt.bfloat16, tag=f"mask{i}", name=f"mask{i}") for i in range(4)]

            xq_tiles = {}

            def emit_x_dmas(tci, rings):
                ts_ = slice(tci * TC, (tci + 1) * TC)
                tiles = []
                for g in range(4):
                    rs = slice(g * CQ * 128, (g + 1) * CQ * 128)
                    t = xsp.tile([128, CQ * TC], dt.bfloat16, tag="xq",
                                 name=f"x_{tci}_{g}")
                    rings[g].dma_start(
                        t.rearrange("p (c n) -> p c n", c=CQ),
                        xt[rs, ts_].rearrange("(c p) n -> p c n", p=128),
                    )
                    tiles.append(t)
                xq_tiles[tci] = tiles

            # startup: the first QKV chain consumes (wq_g, x_g) pairs in
            # quarter order, so interleave them pairwise on the fast sync
            # ring; gpsimd carries the RoPE tables + K/V weights (needed
            # a few microseconds later).
            x0_tiles = []
            for g in range(4):
                emit_w_dma(wq_ts, wqp, g, nc.sync)
                rs = slice(g * CQ * 128, (g + 1) * CQ * 128)
                t = xsp.tile([128, CQ * TC], dt.bfloat16, tag="xq", name=f"x_0_{g}")
                nc.sync.dma_start(
                    t.rearrange("p (c n) -> p c n", c=CQ),
                    xt[rs, 0:TC].rearrange("(c p) n -> p c n", p=128),
                )
                x0_tiles.append(t)
            xq_tiles[0] = x0_tiles
            nc.gpsimd.dma_start(cs_t[:], csT[:])
            emit_w_dma(wk_ts, wkp, 0, nc.gpsimd)
            emit_w_dma(wk_ts, wkp, 1, nc.gpsimd)
            emit_w_dma(wv_ts, wvp, 0, nc.gpsimd)
            emit_w_dma(wv_ts, wvp, 1, nc.gpsimd)
            for i in range(4):
                nc.gpsimd.dma_start(mask_t[i][:], masks[i * 128:(i + 1) * 128, :])
            nc.sync.dma_start(
                wo_t.rearrange("p (c n) -> p c n", c=CO),
                woh.rearrange("(c p) n -> p c n", p=128),
            )

            # K/V persist per 512-chunk / 128-block (no cross-chunk tiles,
            # so interleaved next-chunk RoPE writes never alias attention
            # reads at the dep tracker's granularity)
            ktc = [[actp.tile([128, TC], dt.bfloat16, tag=f"kt{h}_{j}", name=f"kt{h}_{j}")
                    for j in range(NTC)] for h in range(NKV)]
            vt = [actp.tile([128, NKV * HD], dt.bfloat16, tag=f"vt{b}", name=f"vt{b}") for b in range(TB)]
            qt_all = {}
            ot_all = {}
            for tci in range(NTC):
                qt_all[tci] = [cap.tile([128, TC], dt.bfloat16, tag=f"qt{h}_{tci % 2}", name=f"qt{h}_{tci}") for h in range(NQ)]
                ot_all[tci] = [cap.tile([128, TC], dt.bfloat16, tag=f"ot{h}_{tci % 2}", name=f"ot{h}_{tci}") for h in range(NQ)]

            # ---------------- unit generators ----------------
            def rope_epilogue(tci, h, ps):
                ts_ = slice(tci * TC, (tci + 1) * TC)
                rot = scr.tile([128, TC], dt.bfloat16, tag="rope", bufs=2)
                t0 = scr.tile([128, TC], dt.bfloat16, tag="ropetmp", bufs=1)
                cs = cos_t[:, ts_]
                sn = sin_t[:, ts_]
                xr = ps[0:64, :]
                xi = ps[64:128, :]
                dsth = qt_all[tci][h][:] if h < NQ else ktc[h - NQ][tci][:]
                nc.vector.tensor_tensor(rot[0:64, :], xr, cs, mybir.AluOpType.mult)
                nc.vector.tensor_tensor(t0[0:64, :], xi, sn, mybir.AluOpType.mult)
                nc.vector.tensor_tensor(dsth[0:64, :], rot[0:64, :], t0[0:64, :], mybir.AluOpType.subtract)
                nc.vector.tensor_tensor(rot[64:128, :], xr, sn, mybir.AluOpType.mult)
                nc.vector.tensor_tensor(t0[64:128, :], xi, cs, mybir.AluOpType.mult)
                nc.vector.tensor_tensor(dsth[64:128, :], rot[64:128, :], t0[64:128, :], mybir.AluOpType.add)

            def qkv_units(tci, ptag, pbufs, group=4):
                """Closures emitting `group` matmuls of a QKV/V chain each
                (coarser units keep chain psum lifetimes short)."""
                units = []
                state = {}

                def x_c(c):
                    xg = xq_tiles[tci]
                    return xg[c // CQ][:, (c % CQ) * TC:(c % CQ + 1) * TC]

                def qk_seg(h, c0):
                    if c0 == 0:
                        state[h] = psp.tile([128, TC], dt.float32, tag=ptag,
                                            bufs=pbufs, name=f"qkv_{tci}_{h}")
                    ps = state[h]
                    for c in range(c0, c0 + group):
                        wsl = wq_sl(c, h) if h < NQ else wk_sl(c, h - NQ)
                        nc.tensor.matmul(ps[:], wsl, x_c(c),
                                         start=(c == 0), stop=(c == C - 1))
                    if c0 + group == C:
                        rope_epilogue(tci, h, ps)

                def v_seg(tb, c0):
                    key = "v", tb
                    if c0 == 0:
                        state[key] = psp.tile([128, NKV * HD], dt.float32,
                                              tag=ptag, bufs=pbufs,
                                              name=f"v_{tci}_{tb}")
                    ps = state[key]
                    for c in range(c0, c0 + group):
                        nc.tensor.matmul(ps[:], x_c(c)[:, tb * 128:(tb + 1) * 128],
                                         wv_sl(c),
                                         start=(c == 0), stop=(c == C - 1))
                    if c0 + group == C:
                        nc.scalar.copy(vt[tci * NTB + tb][:], ps[:])

                for h in range(NQ + NKV):
                    for c0 in range(0, C, group):
                        units.append(lambda h=h, c0=c0: qk_seg(h, c0))
                for tb in range(NTB):
                    for c0 in range(0, C, group):
                        units.append(lambda tb=tb, c0=c0: v_seg(tb, c0))
                return units

            def op_units(tci, ptag, pbufs):
                """One closure per output-projection db group (4 matmuls +
                copy + store)."""
                ts_ = slice(tci * TC, (tci + 1) * TC)
                ot = ot_all[tci]
                units = []
                for db in range(DB):
                    def u(db=db):
                        ps = psp.tile([128, TC], dt.float32, tag=ptag, bufs=pbufs,
                                      name=f"op_{tci}_{db}")
                        for c in range(CO):
                            nc.tensor.matmul(
                                ps[:], wo_t[:, c * D + db * 128:c * D + (db + 1) * 128],
                                ot[c][:],
                                start=(c == 0), stop=(c == CO - 1),
                            )
                        o3 = scr.tile([128, TC], dt.bfloat16, tag="o3", bufs=4)
                        nc.scalar.copy(o3[:], ps[:])
                        eng = nc.sync if db % 2 == 0 else nc.gpsimd
                        eng.dma_start(outT[db * 128:(db + 1) * 128, ts_], o3[:])
                    units.append(u)
                return units

            # ---------------- attention emission ----------------
            def emit_attention(tci, filler):
                """Emits scores/exp/PV blocks for q-chunk tci, draining
                `filler` closures between blocks to keep the PE busy."""
                qc = tci
                qt = qt_all[tci]
                ot = ot_all[tci]
                nkb = (qc + 1) * NTB
                blocks = [(h, kb) for h in range(NQ) for kb in range(nkb)]
                nb = len(blocks)
                head_ps = {}
                pending_a = []   # (h, ot_ps, sum_ps) awaiting norm part b
                LAG = 4

                def norm_a(h):
                    _, sum_ps = head_ps[h]
                    # copy psum->sbuf on the (lightly loaded) vector queue so
                    # the single sums bank frees before the next head needs it
                    sumc = scr.tile([1, TC], dt.float32, tag="sumc", bufs=2, name=f"sumc_{tci}_{h}")
                    nc.vector.tensor_copy(sumc[:], sum_ps[:])
                    lnt = scr.tile([1, TC], dt.float32, tag="lnt", bufs=2, name=f"ln_{tci}_{h}")
                    nc.scalar.activation(lnt[:], sumc[:], AF.Ln)
                    rec = scr.tile([1, TC], dt.float32, tag="rec", bufs=2, name=f"rec_{tci}_{h}")
                    nc.scalar.activation(rec[:], lnt[:], AF.Exp, bias=0.0, scale=-1.0)
                    rech = scr.tile([1, TC], dt.bfloat16, tag="rech", bufs=2, name=f"rech_{tci}_{h}")
                    recl = scr.tile([1, TC], dt.bfloat16, tag="recl", bufs=2, name=f"recl_{tci}_{h}")
                    nc.scalar.copy(rech[:], rec[:])
                    nc.vector.tensor_tensor(recl[:], rec[:], rech[:], mybir.AluOpType.subtract)
                    return rech, recl

                def norm_b(h, rech, recl):
                    ot_ps, _ = head_ps[h]
                    bc_ps = psp.tile([128, TC], dt.float32, tag="op", bufs=1, name=f"bc_{tci}_{h}")
                    nc.tensor.matmul(bc_ps[:], ones_row[:], rech[:], start=True, stop=False)
                    nc.tensor.matmul(bc_ps[:], ones_row[:], recl[:], start=False, stop=True)
                    recb = scr.tile([128, TC], dt.float32, tag="recb", bufs=2, name=f"recb_{tci}_{h}")
                    nc.scalar.copy(recb[:], bc_ps[:])
                    nc.vector.tensor_tensor(ot[h][:], ot_ps[:], recb[:], mybir.AluOpType.mult)

                def emit_scores(h, kb):
                    kv = h // REP
                    d = kb * 128 - qc * TC
                    q0 = max(d, 0)
                    sc_ps = psp.tile([128, TC], dt.float32, tag="sc", bufs=2,
                                     name=f"sc_{tci}_{h}_{kb}")
                    ksl = ktc[kv][kb // NTB][:, (kb % NTB) * 128:(kb % NTB + 1) * 128]
                    nc.tensor.matmul(sc_ps[:, q0:TC], ksl, qt[h][:, q0:TC],
                                     start=True, stop=True)
                    ph = scr.tile([128, TC], dt.bfloat16, tag="ph", bufs=LAG + 2,
                                  name=f"ph_{tci}_{h}_{kb}")
                    nc.scalar.activation(ph[:, q0:TC], sc_ps[:, q0:TC], AF.Exp,
                                         bias=0.0, scale=scale)
                    if d >= 0:
                        nc.vector.tensor_tensor(ph[:, q0:TC], ph[:, q0:TC],
                                                mask_t[d // 128][:, q0:TC],
                                                mybir.AluOpType.mult)
                    return ph

                def emit_pv(h, kb, ph):
                    kv = h // REP
                    vcol = kv * HD
                    d = kb * 128 - qc * TC
                    q0 = max(d, 0)
                    if kb == 0:
                        head_ps[h] = (
                            psp.tile([128, TC], dt.float32, tag="otps", bufs=2,
                                     name=f"ot_{tci}_{h}"),
                            psp.tile([1, TC], dt.float32, tag="sums", bufs=1,
                                     name=f"sum_{tci}_{h}"),
                        )
                    ot_ps, sum_ps = head_ps[h]
                    nc.tensor.matmul(
                        ot_ps[:, q0:TC], vt[kb][:, vcol:vcol + HD], ph[:, q0:TC],
                        start=(kb == 0), stop=(kb == nkb - 1),
                    )
                    nc.tensor.matmul(
                        sum_ps[:, q0:TC], ones_t[:], ph[:, q0:TC],
                        start=(kb == 0), stop=(kb == nkb - 1),
                    )
                    if kb == nkb - 1:
                        pending_a.append((h, *norm_a(h)))

                # filler pacing: spread all filler units evenly over blocks
                nf = len(filler)
                drained = 0
                probs_q = []
                for bi, (h, kb) in enumerate(blocks):
                    probs_q.append((h, kb, emit_scores(h, kb)))
                    # norms deferred by one block so scalar/vector prep is
                    # ready before the PE broadcast matmul
                    if len(pending_a) > 1 or (pending_a and kb >= 2):
                        norm_b(*pending_a.pop(0))
                    want = (bi + 1) * nf // nb
                    while drained < want:
                        filler[drained]()
                        drained += 1
                    if len(probs_q) > LAG:
                        hh, kk, ph = probs_q.pop(0)
                        emit_pv(hh, kk, ph)
                for hh, kk, ph in probs_q:
                    emit_pv(hh, kk, ph)
                while drained < nf:
                    filler[drained]()
                    drained += 1
                for args in pending_a:
                    norm_b(*args)

            # ---------------- schedule ----------------
            # QKV0 standalone; attn(t) interleaves QKV(t+1) and outproj(t-1);
            # outproj(NTC-1) standalone.
            for u in qkv_units(0, "sc", 2):
                u()
            for tci in range(NTC):
                if tci + 1 < NTC:
                    emit_x_dmas(tci + 1, [nc.sync, nc.sync, nc.gpsimd, nc.gpsimd])
                filler = []
                if tci > 0:
                    filler += op_units(tci - 1, "op", 1)
                if tci + 1 < NTC:
                    filler += qkv_units(tci + 1, "fill", 2)
                emit_attention(tci, filler)
                xq_tiles.pop(tci)
            for u in op_units(NTC - 1, "sc", 2):
                u()

    return nc


# ---------------------------------------------------------------------------
# walrus in this container refuses >1 sem wait per instruction ("Too many
# sync wait commands"). Hoist excess waits onto same-engine NoOps inserted
# immediately before the instruction - program order on the engine queue
# preserves the sync semantics.
def split_multiwait_insts(nc, max_waits=1):
    n_split = 0
    for bb in nc.main_func.blocks:
        insts = bb.instructions
        i = 0
        while i < len(insts):
            ins = insts[i]
            si = getattr(ins, "sync_info", None)
            if si is not None and si.on_wait and len(si.on_wait) > max_waits:
                waits = list(si.on_wait)
                head, tail = waits[:-max_waits], waits[-max_waits:]
                nops = []
                for j in range(0, len(head), max_waits):
                    nop = mybir.InstNoOp(name=f"{ins.name}-ws{j}", ins=[], outs=[])
                    nop.engine = ins.engine
                    nop.sync_info = mybir.SyncInfo(
                        on_wait=head[j:j + max_waits], on_update=[])
                    nops.append(nop)
                ins.sync_info = mybir.SyncInfo(
                    on_wait=tail, on_update=list(si.on_update or []))
                insts[i:i] = nops
                i += len(nops)
                n_split += 1
            i += 1
    return n_split


# ---------------------------------------------------------------------------
# Host-side shard preparation / gather
BF16 = ml_dtypes.bfloat16


def rope_tables(S, HD):
    inv = 1.0 / (10000.0 ** (np.arange(0, HD, 2, dtype=np.float32) / HD))
    t = np.arange(S, dtype=np.float32)
    f = np.outer(t, inv).astype(np.float32)  # [S, HD//2]
    return np.ascontiguousarray(np.cos(f).T), np.ascontiguousarray(np.sin(f).T)


def causal_masks(TC):
    # masks[dd][k, qrel] = 1 if k + dd*128 <= qrel else 0
    out = np.zeros((4 * 128, TC), BF16)
    k = np.arange(128)[:, None]
    q = np.arange(TC)[None, :]
    for dd in range(4):
        out[dd * 128:(dd + 1) * 128] = (k + dd * 128 <= q).astype(BF16)
    return out


def rope_perm(HD):
    # new row i (i < HD//2) = old 2i; new row HD//2+i = old 2i+1
    return np.concatenate([np.arange(0, HD, 2), np.arange(1, HD, 2)])


def make_in_maps(x, wq, wk, wv, wo, *, n_batch_shards, n_head_shards,
                 NQ_TOT, NKV_TOT, HD, TC):
    """Returns list of in_maps, one per core (batch-major: core = b*G + g)."""
    B, S, D = x.shape
    G = n_head_shards
    NQ = NQ_TOT // G
    NKV = NKV_TOT // G
    perm = rope_perm(HD)
    cosT, sinT = rope_tables(S, HD)
    csT = np.concatenate([cosT, sinT], axis=0).astype(BF16)  # [HD, S]
    masks = causal_masks(TC)

    # Per-batch xT (shared across head shards)
    xtb = {}
    for b in range(B):
        xtb[b] = np.ascontiguousarray(x[b].T).astype(BF16)  # [D, S]

    # Per-headgroup weight shards
    wshard = {}
    for g in range(G):
        qrows = slice(g * NQ * HD, (g + 1) * NQ * HD)
        kvrows = slice(g * NKV * HD, (g + 1) * NKV * HD)
        wq_g = wq[qrows, :].copy()      # [NQ*HD, D]
        wk_g = wk[kvrows, :].copy()
        wv_g = wv[kvrows, :].copy()
        # RoPE permutation of output rows, per head
        for hh in range(NQ):
            blk = wq_g[hh * HD:(hh + 1) * HD]
            wq_g[hh * HD:(hh + 1) * HD] = blk[perm]
        for hh in range(NKV):
            blk = wk_g[hh * HD:(hh + 1) * HD]
            wk_g[hh * HD:(hh + 1) * HD] = blk[perm]
        wqT = np.ascontiguousarray(wq_g.T).astype(BF16)   # [D, NQ*HD]
        wkT = np.ascontiguousarray(wk_g.T).astype(BF16)
        wvT = np.ascontiguousarray(wv_g.T).astype(BF16)
        woT = np.ascontiguousarray(wo[:, qrows].T).astype(BF16)  # [NQ*HD, D]
        wshard[g] = (wqT, wkT, wvT, woT)

    in_maps = []
    for b in range(n_batch_shards):
        for g in range(G):
            wqT, wkT, wvT, woT = wshard[g]
            in_maps.append({
                "xt": xtb[b],
                "wqp": wqT, "wkp": wkT, "wvp": wvT, "woh": woT,
                "csT": csT,
                "masks": masks,
            })
    return in_maps


def combine_outputs(outTs, B, G):
    """outTs: list of [D, S] partials, core order b*G+g. Returns [B, S, D]."""
    outs = []
    for b in range(B):
        acc = outTs[b * G].astype(np.float32).copy()
        for g in range(1, G):
            acc += outTs[b * G + g]
        outs.append(acc.T)  # [S, D]
    return np.stack(outs)


_NC_CACHE = {}


def _get_nc(S, D, NQ, NKV, HD, TC):
    key = (S, D, NQ, NKV, HD, TC)
    if key not in _NC_CACHE:
        nc = build_attention_nc(S=S, D=D, NQ=NQ, NKV=NKV, HD=HD, TC=TC)
        split_multiwait_insts(nc)
        _NC_CACHE[key] = nc
    return _NC_CACHE[key]


def kernel(**inputs):
    x = np.asarray(inputs["x"], dtype=np.float32)
    wq = np.asarray(inputs["wq"], dtype=np.float32)
    wk = np.asarray(inputs["wk"], dtype=np.float32)
    wv = np.asarray(inputs["wv"], dtype=np.float32)
    wo = np.asarray(inputs["wo"], dtype=np.float32)

    B, S, D = x.shape          # (2, 2048, 2048)
    NQ_TOT = wq.shape[0] // 128
    NKV_TOT = wk.shape[0] // 128
    HD = 128
    TC = 512
    G = 4                      # head shards
    NQ, NKV = NQ_TOT // G, NKV_TOT // G

    nc = _get_nc(S, D, NQ, NKV, HD, TC)
    in_maps = make_in_maps(
        x, wq, wk, wv, wo,
        n_batch_shards=B, n_head_shards=G,
        NQ_TOT=NQ_TOT, NKV_TOT=NKV_TOT, HD=HD, TC=TC,
    )

    from concourse.bass_utils import run_bass_kernel_spmd

    trace = os.environ.get("BASS_ATTN_TRACE") == "1"
    res = run_bass_kernel_spmd(nc, in_maps, list(range(len(in_maps))), trace=trace)
    kernel.last_results = res
    outTs = [r["outT"] for r in res.results]
    return combine_outputs(outTs, B, G).astype(np.float32)


# revision 35
# speedup vs baseline: 2.8477x; 1.1327x over previous
"""Trainium2 Bass kernel for nn_Attention_77043123355775.

Sharded GQA causal attention with RoPE: 8 NeuronCores as 2-way data
parallel (batch) x 4-way tensor parallel (heads). Each core computes its
4 Q heads / 2 KV heads for one batch entry and a partial output
projection (x[b] @ W)^T; the host sums the 4 partials per batch.

All matmuls run in plain bf16 with fp32 PSUM accumulation (the 2e-2
rel-err budget has ~20x headroom over bf16 rounding noise). Weights are
fully SBUF-resident. The PE instruction stream interleaves next-chunk
QKV/V projection chains and prev-chunk output-projection groups between
attention blocks, so the PE never waits on the scalar-engine exp chain
and stays in its high DVFS p-state.
"""
import math
import os
import sys

for _p in ("/opt/trn_rl_repo",):
    if _p not in sys.path:
        sys.path.insert(0, _p)

import ml_dtypes
import numpy as np

import concourse.bass as bass
import concourse.mybir as mybir
import concourse.tile as tile

dt = mybir.dt
AF = mybir.ActivationFunctionType


def build_attention_nc(S=2048, D=2048, NQ=4, NKV=2, HD=128, TC=512):
    assert HD == 128
    C = D // 128          # contraction chunks over features
    TB = S // 128         # 128-token blocks
    NTC = S // TC         # token chunks
    DB = D // 128         # output feature blocks
    CO = NQ * HD // 128   # contraction chunks for wo (= NQ)
    REP = NQ // NKV
    CQ = C // 4           # c-chunks per x quarter-tile
    NTB = TC // 128       # token blocks per chunk
    scale = 1.0 / math.sqrt(HD)

    nc = bass.Bass()

    xt = nc.dram_tensor("xt", [D, S], dt.bfloat16, kind="ExternalInput")
    ident = nc.dram_tensor("ident", [128, 128], dt.bfloat16, kind="ExternalInput")
    wqp = nc.dram_tensor("wqp", [D, NQ * HD], dt.bfloat16, kind="ExternalInput")
    wkp = nc.dram_tensor("wkp", [D, NKV * HD], dt.bfloat16, kind="ExternalInput")
    wvp = nc.dram_tensor("wvp", [D, NKV * HD], dt.bfloat16, kind="ExternalInput")
    woh = nc.dram_tensor("woh", [NQ * HD, D], dt.bfloat16, kind="ExternalInput")
    csT = nc.dram_tensor("csT", [HD, S], dt.bfloat16, kind="ExternalInput")
    masks = nc.dram_tensor("masks", [4 * 128, TC], dt.bfloat16, kind="ExternalInput")
    outT = nc.dram_tensor("outT", [D, S], dt.bfloat16, kind="ExternalOutput")

    with tile.TileContext(nc) as tc:
        with (
            tc.tile_pool(name="const", bufs=1) as constp,
            tc.tile_pool(name="tabs", bufs=1) as tabp,
            tc.tile_pool(name="wts", bufs=1) as wtp,
            tc.tile_pool(name="acts", bufs=1) as actp,
            tc.tile_pool(name="chunkacts", bufs=1) as cap,
            tc.tile_pool(name="xstream", bufs=8) as xsp,
            tc.tile_pool(name="scratch", bufs=3) as scr,
            tc.tile_pool(name="psum", bufs=1, space="PSUM") as psp,
        ):
            ident_t = constp.tile([128, 128], dt.bfloat16, tag="ident")

            # ---- resident tables / weights (c-quartered for fine deps) ----
            def emit_w_dma(tiles, src, g, ring):
                cq = C // len(tiles)
                rs = slice(g * cq * 128, (g + 1) * cq * 128)
                ring.dma_start(
                    tiles[g].rearrange("p (c n) -> p c n", c=cq),
                    src[rs, :].rearrange("(c p) n -> p c n", p=128),
                )

            wq_ts = [wtp.tile([128, CQ * NQ * HD], dt.bfloat16, tag=f"wq{g}", name=f"wq{g}") for g in range(4)]
            wk_ts = [wtp.tile([128, (C // 2) * NKV * HD], dt.bfloat16, tag=f"wk{g}", name=f"wk{g}") for g in range(2)]
            wv_ts = [wtp.tile([128, (C // 2) * NKV * HD], dt.bfloat16, tag=f"wv{g}", name=f"wv{g}") for g in range(2)]
            wo_t = wtp.tile([128, CO * D], dt.bfloat16, tag="wo")

            def wq_sl(c, h):
                return wq_ts[c // CQ][:, (c % CQ) * NQ * HD + h * HD:(c % CQ) * NQ * HD + (h + 1) * HD]

            def wk_sl(c, h):
                ch = C // 2
                return wk_ts[c // ch][:, (c % ch) * NKV * HD + h * HD:(c % ch) * NKV * HD + (h + 1) * HD]

            def wv_sl(c):
                ch = C // 2
                return wv_ts[c // ch][:, (c % ch) * NKV * HD:(c % ch + 1) * NKV * HD]

            cs_t = tabp.tile([HD, S], dt.bfloat16, tag="cs")
            cos_t = cs_t[0:HD // 2, :]
            sin_t = cs_t[HD // 2:HD, :]
            mask_t = [tabp.tile([128, TC], dt.bfloat16, tag=f"mask{i}", name=f"mask{i}") for i in range(4)]

            xq_tiles = {}

            def emit_x_dmas(tci, rings):
                ts_ = slice(tci * TC, (tci + 1) * TC)
                tiles = []
                for g in range(4):
                    rs = slice(g * CQ * 128, (g + 1) * CQ * 128)
                    t = xsp.tile([128, CQ * TC], dt.bfloat16, tag="xq",
                                 name=f"x_{tci}_{g}")
                    rings[g].dma_start(
                        t.rearrange("p (c n) -> p c n", c=CQ),
                        xt[rs, ts_].rearrange("(c p) n -> p c n", p=128),
                    )
                    tiles.append(t)
                xq_tiles[tci] = tiles

            # startup: the first QKV chain consumes (wq_g, x_g) pairs in
            # quarter order, so interleave them pairwise on the fast sync
            # ring; gpsimd carries the RoPE tables + K/V weights (needed
            # a few microseconds later).
            x0_tiles = []
            for g in range(4):
                emit_w_dma(wq_ts, wqp, g, nc.sync)
                rs = slice(g * CQ * 128, (g + 1) * CQ * 128)
                t = xsp.tile([128, CQ * TC], dt.bfloat16, tag="xq", name=f"x_0_{g}")
                nc.sync.dma_start(
                    t.rearrange("p (c n) -> p c n", c=CQ),
                    xt[rs, 0:TC].rearrange("(c p) n -> p c n", p=128),
                )
                x0_tiles.append(t)
            xq_tiles[0] = x0_tiles
            nc.gpsimd.dma_start(cs_t[:], csT[:])
            emit_w_dma(wk_ts, wkp, 0, nc.gpsimd)
            emit_w_dma(wk_ts, wkp, 1, nc.gpsimd)
            emit_w_dma(wv_ts, wvp, 0, nc.gpsimd)
            emit_w_dma(wv_ts, wvp, 1, nc.gpsimd)
            for i in range(4):
                nc.gpsimd.dma_start(mask_t[i][:], masks[i * 128:(i + 1) * 128, :])
            nc.gpsimd.dma_start(ident_t[:], ident[:])
            nc.sync.dma_start(
                wo_t.rearrange("p (c n) -> p c n", c=CO),
                woh.rearrange("(c p) n -> p c n", p=128),
            )

            # K/V persist per 512-chunk / 128-block (no cross-chunk tiles,
            # so interleaved next-chunk RoPE writes never alias attention
            # reads at the dep tracker's granularity)
            ktc = [[actp.tile([128, TC], dt.bfloat16, tag=f"kt{h}_{j}", name=f"kt{h}_{j}")
                    for j in range(NTC)] for h in range(NKV)]
            # V tiles carry a ones column per kv head (col kv*(HD+1)+HD) so
            # the flipped PV matmul emits softmax denominators for free
            vt = [actp.tile([128, NKV * (HD + 1)], dt.bfloat16, tag=f"vt{b}", name=f"vt{b}") for b in range(TB)]
            for b in range(TB):
                for kv in range(NKV):
                    nc.vector.memset(vt[b][:, kv * (HD + 1) + HD:(kv + 1) * (HD + 1)], 1.0)
            qt_all = {}
            ot_all = {}
            for tci in range(NTC):
                qt_all[tci] = [cap.tile([128, TC], dt.bfloat16, tag=f"qt{h}_{tci % 2}", name=f"qt{h}_{tci}") for h in range(NQ)]
                ot_all[tci] = [cap.tile([128, TC], dt.bfloat16, tag=f"ot{h}_{tci % 2}", name=f"ot{h}_{tci}") for h in range(NQ)]

            # ---------------- unit generators ----------------
            def rope_epilogue(tci, h, ps):
                ts_ = slice(tci * TC, (tci + 1) * TC)
                rot = scr.tile([128, TC], dt.bfloat16, tag="rope", bufs=2)
                t0 = scr.tile([128, TC], dt.bfloat16, tag="ropetmp", bufs=1)
                cs = cos_t[:, ts_]
                sn = sin_t[:, ts_]
                xr = ps[0:64, :]
                xi = ps[64:128, :]
                dsth = qt_all[tci][h][:] if h < NQ else ktc[h - NQ][tci][:]
                nc.vector.tensor_tensor(rot[0:64, :], xr, cs, mybir.AluOpType.mult)
                nc.vector.tensor_tensor(t0[0:64, :], xi, sn, mybir.AluOpType.mult)
                nc.vector.tensor_tensor(dsth[0:64, :], rot[0:64, :], t0[0:64, :], mybir.AluOpType.subtract)
                nc.vector.tensor_tensor(rot[64:128, :], xr, sn, mybir.AluOpType.mult)
                nc.vector.tensor_tensor(t0[64:128, :], xi, cs, mybir.AluOpType.mult)
                nc.vector.tensor_tensor(dsth[64:128, :], rot[64:128, :], t0[64:128, :], mybir.AluOpType.add)

            def qkv_units(tci, ptag, pbufs, group=4):
                """Closures emitting `group` matmuls of a QKV/V chain each
                (coarser units keep chain psum lifetimes short)."""
                units = []
                state = {}

                def x_c(c):
                    xg = xq_tiles[tci]
                    return xg[c // CQ][:, (c % CQ) * TC:(c % CQ + 1) * TC]

                def qk_seg(h, c0):
                    if c0 == 0:
                        state[h] = psp.tile([128, TC], dt.float32, tag=ptag,
                                            bufs=pbufs, name=f"qkv_{tci}_{h}")
                    ps = state[h]
                    for c in range(c0, c0 + group):
                        wsl = wq_sl(c, h) if h < NQ else wk_sl(c, h - NQ)
                        nc.tensor.matmul(ps[:], wsl, x_c(c),
                                         start=(c == 0), stop=(c == C - 1))
                    if c0 + group == C:
                        rope_epilogue(tci, h, ps)

                def v_seg(tb, c0):
                    key = "v", tb
                    if c0 == 0:
                        state[key] = psp.tile([128, NKV * HD], dt.float32,
                                              tag=ptag, bufs=pbufs,
                                              name=f"v_{tci}_{tb}")
                    ps = state[key]
                    for c in range(c0, c0 + group):
                        nc.tensor.matmul(ps[:], x_c(c)[:, tb * 128:(tb + 1) * 128],
                                         wv_sl(c),
                                         start=(c == 0), stop=(c == C - 1))
                    if c0 + group == C:
                        for kv in range(NKV):
                            nc.scalar.copy(
                                vt[tci * NTB + tb][:, kv * (HD + 1):kv * (HD + 1) + HD],
                                ps[:, kv * HD:(kv + 1) * HD])

                for h in range(NQ + NKV):
                    for c0 in range(0, C, group):
                        units.append(lambda h=h, c0=c0: qk_seg(h, c0))
                for tb in range(NTB):
                    for c0 in range(0, C, group):
                        units.append(lambda tb=tb, c0=c0: v_seg(tb, c0))
                return units

            def op_units(tci, ptag, pbufs):
                """One closure per output-projection db group (4 matmuls +
                copy + store)."""
                ts_ = slice(tci * TC, (tci + 1) * TC)
                ot = ot_all[tci]
                units = []
                for db in range(DB):
                    def u(db=db):
                        ps = psp.tile([128, TC], dt.float32, tag=ptag, bufs=pbufs,
                                      name=f"op_{tci}_{db}")
                        for c in range(CO):
                            nc.tensor.matmul(
                                ps[:], wo_t[:, c * D + db * 128:c * D + (db + 1) * 128],
                                ot[c][:],
                                start=(c == 0), stop=(c == CO - 1),
                            )
                        o3 = scr.tile([128, TC], dt.bfloat16, tag="o3", bufs=4)
                        nc.scalar.copy(o3[:], ps[:])
                        eng = nc.sync if db % 2 == 0 else nc.gpsimd
                        eng.dma_start(outT[db * 128:(db + 1) * 128, ts_], o3[:])
                    units.append(u)
                return units

            # ---------------- attention emission ----------------
            def emit_attention(tci, filler):
                """Scores + flipped PV for q-chunk tci, draining `filler`
                closures between steps to keep the PE busy. PV runs
                qsb-major: each (head, q-subtile) accumulates [q,HD+1]
                (output + denominator column) as a single group in its own
                PSUM bank -- interleaved groups in one bank corrupt."""
                qc = tci
                qt = qt_all[tci]
                ot = ot_all[tci]
                nkb = (qc + 1) * NTB
                nf = len(filler)
                total_steps = NQ * 3 * nkb
                state = {"step": 0, "drained": 0}

                def drain():
                    state["step"] += 1
                    want = state["step"] * nf // total_steps
                    while state["drained"] < want:
                        filler[state["drained"]]()
                        state["drained"] += 1

                def emit_scores(h, kb):
                    kv = h // REP
                    d = kb * 128 - qc * TC
                    q0 = max(d, 0)
                    sc_ps = psp.tile([128, TC], dt.float32, tag="sc", bufs=3,
                                     name=f"sc_{tci}_{h}_{kb}")
                    ksl = ktc[kv][kb // NTB][:, (kb % NTB) * 128:(kb % NTB + 1) * 128]
                    nc.tensor.matmul(sc_ps[:, q0:TC], ksl, qt[h][:, q0:TC],
                                     start=True, stop=True)
                    ph = scr.tile([128, TC], dt.bfloat16, tag="ph", bufs=20,
                                  name=f"ph_{tci}_{h}_{kb}")
                    nc.scalar.activation(ph[:, q0:TC], sc_ps[:, q0:TC], AF.Exp,
                                         bias=0.0, scale=scale)
                    if d >= 0:
                        nc.vector.tensor_tensor(ph[:, q0:TC], ph[:, q0:TC],
                                                mask_t[d // 128][:, q0:TC],
                                                mybir.AluOpType.mult)
                    return ph

                def norm_one(h, qsb, t_):
                    rec = scr.tile([128, 1], dt.float32, tag="recq", bufs=4,
                                   name=f"rec_{tci}_{h}_{qsb}")
                    nc.vector.reciprocal(rec[:], t_[:, HD:HD + 1])
                    otn = scr.tile([128, HD], dt.bfloat16, tag="otn", bufs=4,
                                   name=f"otn_{tci}_{h}_{qsb}")
                    nc.vector.tensor_scalar(otn[:], t_[:, 0:HD], rec[:], None,
                                            mybir.AluOpType.mult)
                    tp = psp.tile([128, 128], dt.bfloat16, tag="sc", bufs=3,
                                  name=f"tp_{tci}_{h}_{qsb}")
                    nc.tensor.transpose(tp[:], otn[:], ident_t[:])
                    nc.scalar.copy(ot[h][:, qsb * 128:(qsb + 1) * 128], tp[:])

                for h in range(NQ):
                    kv = h // REP
                    phs = []
                    for kb in range(nkb):
                        phs.append(emit_scores(h, kb))
                        drain()
                    for qsb0 in (0, 2):
                        chains = []
                        for qsb in (qsb0, qsb0 + 1):
                            t_ = psp.tile([128, HD + 1], dt.float32, tag="otq",
                                          bufs=2, name=f"otq_{tci}_{h}_{qsb}")
                            chains.append((qsb, t_))
                        for kb in range(nkb):
                            d = kb * 128 - qc * TC
                            q0 = max(d, 0)
                            vsl = vt[kb][:, kv * (HD + 1):(kv + 1) * (HD + 1)]
                            for qsb, t_ in chains:
                                if kb > qc * NTB + qsb or qsb < q0 // 128:
                                    continue
                                nc.tensor.matmul(
                                    t_[:], phs[kb][:, qsb * 128:(qsb + 1) * 128], vsl,
                                    start=(kb == 0), stop=(kb == qc * NTB + qsb),
                                )
                            drain()
                        for qsb, t_ in chains:
                            norm_one(h, qsb, t_)
                while state["drained"] < nf:
                    filler[state["drained"]]()
                    state["drained"] += 1

            # ---------------- schedule ----------------
            # QKV0 standalone; attn(t) interleaves QKV(t+1) and outproj(t-1);
            # outproj(NTC-1) standalone.
            for u in qkv_units(0, "sc", 3):
                u()
            for tci in range(NTC):
                if tci + 1 < NTC:
                    emit_x_dmas(tci + 1, [nc.sync, nc.sync, nc.gpsimd, nc.gpsimd])
                filler = []
                if tci > 0:
                    filler += op_units(tci - 1, "op", 1)
                if tci + 1 < NTC:
                    filler += qkv_units(tci + 1, "fill", 2)
                emit_attention(tci, filler)
                xq_tiles.pop(tci)
            for u in op_units(NTC - 1, "sc", 3):
                u()

    return nc


# ---------------------------------------------------------------------------
# walrus in this container refuses >1 sem wait per instruction ("Too many
# sync wait commands"). Hoist excess waits onto same-engine NoOps inserted
# immediately before the instruction - program order on the engine queue
# preserves the sync semantics.
def split_multiwait_insts(nc, max_waits=1):
    n_split = 0
    for bb in nc.main_func.blocks:
        insts = bb.instructions
        i = 0
        while i < len(insts):
            ins = insts[i]
            si = getattr(ins, "sync_info", None)
            if si is not None and si.on_wait and len(si.on_wait) > max_waits:
                waits = list(si.on_wait)
                head, tail = waits[:-max_waits], waits[-max_waits:]
                nops = []
                for j in range(0, len(head), max_waits):
                    nop = mybir.InstNoOp(name=f"{ins.name}-ws{j}", ins=[], outs=[])
                    nop.engine = ins.engine
                    nop.sync_info = mybir.SyncInfo(
                        on_wait=head[j:j + max_waits], on_update=[])
                    nops.append(nop)
                ins.sync_info = mybir.SyncInfo(
                    on_wait=tail, on_update=list(si.on_update or []))
                insts[i:i] = nops
                i += len(nops)
                n_split += 1
            i += 1
    return n_split


# ---------------------------------------------------------------------------
# Host-side shard preparation / gather
BF16 = ml_dtypes.bfloat16


def rope_tables(S, HD):
    inv = 1.0 / (10000.0 ** (np.arange(0, HD, 2, dtype=np.float32) / HD))
    t = np.arange(S, dtype=np.float32)
    f = np.outer(t, inv).astype(np.float32)  # [S, HD//2]
    return np.ascontiguousarray(np.cos(f).T), np.ascontiguousarray(np.sin(f).T)


def causal_masks(TC):
    # masks[dd][k, qrel] = 1 if k + dd*128 <= qrel else 0
    out = np.zeros((4 * 128, TC), BF16)
    k = np.arange(128)[:, None]
    q = np.arange(TC)[None, :]
    for dd in range(4):
        out[dd * 128:(dd + 1) * 128] = (k + dd * 128 <= q).astype(BF16)
    return out


def rope_perm(HD):
    # new row i (i < HD//2) = old 2i; new row HD//2+i = old 2i+1
    return np.concatenate([np.arange(0, HD, 2), np.arange(1, HD, 2)])


def make_in_maps(x, wq, wk, wv, wo, *, n_batch_shards, n_head_shards,
                 NQ_TOT, NKV_TOT, HD, TC):
    """Returns list of in_maps, one per core (batch-major: core = b*G + g)."""
    B, S, D = x.shape
    G = n_head_shards
    NQ = NQ_TOT // G
    NKV = NKV_TOT // G
    perm = rope_perm(HD)
    cosT, sinT = rope_tables(S, HD)
    csT = np.concatenate([cosT, sinT], axis=0).astype(BF16)  # [HD, S]
    masks = causal_masks(TC)

    # Per-batch xT (shared across head shards)
    xtb = {}
    for b in range(B):
        xtb[b] = np.ascontiguousarray(x[b].T).astype(BF16)  # [D, S]

    # Per-headgroup weight shards
    wshard = {}
    for g in range(G):
        qrows = slice(g * NQ * HD, (g + 1) * NQ * HD)
        kvrows = slice(g * NKV * HD, (g + 1) * NKV * HD)
        wq_g = wq[qrows, :].copy()      # [NQ*HD, D]
        wk_g = wk[kvrows, :].copy()
        wv_g = wv[kvrows, :].copy()
        # RoPE permutation of output rows, per head
        for hh in range(NQ):
            blk = wq_g[hh * HD:(hh + 1) * HD]
            wq_g[hh * HD:(hh + 1) * HD] = blk[perm]
        for hh in range(NKV):
            blk = wk_g[hh * HD:(hh + 1) * HD]
            wk_g[hh * HD:(hh + 1) * HD] = blk[perm]
        wqT = np.ascontiguousarray(wq_g.T).astype(BF16)   # [D, NQ*HD]
        wkT = np.ascontiguousarray(wk_g.T).astype(BF16)
        wvT = np.ascontiguousarray(wv_g.T).astype(BF16)
        woT = np.ascontiguousarray(wo[:, qrows].T).astype(BF16)  # [NQ*HD, D]
        wshard[g] = (wqT, wkT, wvT, woT)

    ident = np.eye(128, dtype=BF16)
    in_maps = []
    for b in range(n_batch_shards):
        for g in range(G):
            wqT, wkT, wvT, woT = wshard[g]
            in_maps.append({
                "xt": xtb[b],
                "wqp": wqT, "wkp": wkT, "wvp": wvT, "woh": woT,
                "csT": csT,
                "masks": masks,
                "ident": ident,
            })
    return in_maps


def combine_outputs(outTs, B, G):
    """outTs: list of [D, S] partials, core order b*G+g. Returns [B, S, D]."""
    outs = []
    for b in range(B):
        acc = outTs[b * G].astype(np.float32).copy()
        for g in range(1, G):
            acc += outTs[b * G + g]
        outs.append(acc.T)  # [S, D]
    return np.stack(outs)


_NC_CACHE = {}


def _get_nc(S, D, NQ, NKV, HD, TC):
    key = (S, D, NQ, NKV, HD, TC)
    if key not in _NC_CACHE:
        nc = build_attention_nc(S=S, D=D, NQ=NQ, NKV=NKV, HD=HD, TC=TC)
        split_multiwait_insts(nc)
        _NC_CACHE[key] = nc
    return _NC_CACHE[key]


def kernel(**inputs):
    x = np.asarray(inputs["x"], dtype=np.float32)
    wq = np.asarray(inputs["wq"], dtype=np.float32)
    wk = np.asarray(inputs["wk"], dtype=np.float32)
    wv = np.asarray(inputs["wv"], dtype=np.float32)
    wo = np.asarray(inputs["wo"], dtype=np.float32)

    B, S, D = x.shape          # (2, 2048, 2048)
    NQ_TOT = wq.shape[0] // 128
    NKV_TOT = wk.shape[0] // 128
    HD = 128
    TC = 512
    G = 4                      # head shards
    NQ, NKV = NQ_TOT // G, NKV_TOT // G

    nc = _get_nc(S, D, NQ, NKV, HD, TC)
    in_maps = make_in_maps(
        x, wq, wk, wv, wo,
        n_batch_shards=B, n_head_shards=G,
        NQ_TOT=NQ_TOT, NKV_TOT=NKV_TOT, HD=HD, TC=TC,
    )

    from concourse.bass_utils import run_bass_kernel_spmd

    trace = os.environ.get("BASS_ATTN_TRACE") == "1"
    res = run_bass_kernel_spmd(nc, in_maps, list(range(len(in_maps))), trace=trace)
    kernel.last_results = res
    outTs = [r["outT"] for r in res.results]
    return combine_outputs(outTs, B, G).astype(np.float32)


# revision 36
# speedup vs baseline: 2.9004x; 1.0185x over previous
"""Trainium2 Bass kernel for nn_Attention_77043123355775.

Sharded GQA causal attention with RoPE: 8 NeuronCores as 2-way data
parallel (batch) x 4-way tensor parallel (heads). Each core computes its
4 Q heads / 2 KV heads for one batch entry and a partial output
projection (x[b] @ W)^T; the host sums the 4 partials per batch.

All matmuls run in plain bf16 with fp32 PSUM accumulation (the 2e-2
rel-err budget has ~20x headroom over bf16 rounding noise). Weights are
fully SBUF-resident. The PE instruction stream interleaves next-chunk
QKV/V projection chains and prev-chunk output-projection groups between
attention blocks, so the PE never waits on the scalar-engine exp chain
and stays in its high DVFS p-state.
"""
import math
import os
import sys

for _p in ("/opt/trn_rl_repo",):
    if _p not in sys.path:
        sys.path.insert(0, _p)

import ml_dtypes
import numpy as np

import concourse.bass as bass
import concourse.mybir as mybir
import concourse.tile as tile

dt = mybir.dt
AF = mybir.ActivationFunctionType


def build_attention_nc(S=2048, D=2048, NQ=4, NKV=2, HD=128, TC=512):
    assert HD == 128
    C = D // 128          # contraction chunks over features
    TB = S // 128         # 128-token blocks
    NTC = S // TC         # token chunks
    DB = D // 128         # output feature blocks
    CO = NQ * HD // 128   # contraction chunks for wo (= NQ)
    REP = NQ // NKV
    CQ = C // 4           # c-chunks per x quarter-tile
    NTB = TC // 128       # token blocks per chunk
    scale = 1.0 / math.sqrt(HD)

    nc = bass.Bass()

    xt = nc.dram_tensor("xt", [D, S], dt.bfloat16, kind="ExternalInput")
    ident = nc.dram_tensor("ident", [128, 128], dt.bfloat16, kind="ExternalInput")
    wqp = nc.dram_tensor("wqp", [D, NQ * HD], dt.bfloat16, kind="ExternalInput")
    wkp = nc.dram_tensor("wkp", [D, NKV * HD], dt.bfloat16, kind="ExternalInput")
    wvp = nc.dram_tensor("wvp", [D, NKV * HD], dt.bfloat16, kind="ExternalInput")
    woh = nc.dram_tensor("woh", [NQ * HD, D], dt.bfloat16, kind="ExternalInput")
    csT = nc.dram_tensor("csT", [HD, S], dt.bfloat16, kind="ExternalInput")
    masks = nc.dram_tensor("masks", [4 * 128, TC], dt.bfloat16, kind="ExternalInput")
    outT = nc.dram_tensor("outT", [D, S], dt.bfloat16, kind="ExternalOutput")

    with tile.TileContext(nc) as tc:
        with (
            tc.tile_pool(name="const", bufs=1) as constp,
            tc.tile_pool(name="tabs", bufs=1) as tabp,
            tc.tile_pool(name="wts", bufs=1) as wtp,
            tc.tile_pool(name="acts", bufs=1) as actp,
            tc.tile_pool(name="chunkacts", bufs=1) as cap,
            tc.tile_pool(name="xstream", bufs=8) as xsp,
            tc.tile_pool(name="scratch", bufs=3) as scr,
            tc.tile_pool(name="psum", bufs=1, space="PSUM") as psp,
        ):
            ident_t = constp.tile([128, 128], dt.bfloat16, tag="ident")

            # ---- resident tables / weights (c-quartered for fine deps) ----
            def emit_w_dma(tiles, src, g, ring):
                cq = C // len(tiles)
                rs = slice(g * cq * 128, (g + 1) * cq * 128)
                ring.dma_start(
                    tiles[g].rearrange("p (c n) -> p c n", c=cq),
                    src[rs, :].rearrange("(c p) n -> p c n", p=128),
                )

            wq_ts = [wtp.tile([128, CQ * NQ * HD], dt.bfloat16, tag=f"wq{g}", name=f"wq{g}") for g in range(4)]
            wk_ts = [wtp.tile([128, (C // 2) * NKV * HD], dt.bfloat16, tag=f"wk{g}", name=f"wk{g}") for g in range(2)]
            wv_ts = [wtp.tile([128, (C // 2) * NKV * HD], dt.bfloat16, tag=f"wv{g}", name=f"wv{g}") for g in range(2)]
            wo_t = wtp.tile([128, CO * D], dt.bfloat16, tag="wo")

            def wq_sl(c, h):
                return wq_ts[c // CQ][:, (c % CQ) * NQ * HD + h * HD:(c % CQ) * NQ * HD + (h + 1) * HD]

            def wk_sl(c, h):
                ch = C // 2
                return wk_ts[c // ch][:, (c % ch) * NKV * HD + h * HD:(c % ch) * NKV * HD + (h + 1) * HD]

            def wv_sl(c):
                ch = C // 2
                return wv_ts[c // ch][:, (c % ch) * NKV * HD:(c % ch + 1) * NKV * HD]

            cs_t = tabp.tile([HD, S], dt.bfloat16, tag="cs")
            cos_t = cs_t[0:HD // 2, :]
            sin_t = cs_t[HD // 2:HD, :]
            mask_t = [tabp.tile([128, TC], dt.bfloat16, tag=f"mask{i}", name=f"mask{i}") for i in range(4)]

            xq_tiles = {}

            def emit_x_dmas(tci, rings):
                ts_ = slice(tci * TC, (tci + 1) * TC)
                tiles = []
                for g in range(4):
                    rs = slice(g * CQ * 128, (g + 1) * CQ * 128)
                    t = xsp.tile([128, CQ * TC], dt.bfloat16, tag="xq",
                                 name=f"x_{tci}_{g}")
                    rings[g].dma_start(
                        t.rearrange("p (c n) -> p c n", c=CQ),
                        xt[rs, ts_].rearrange("(c p) n -> p c n", p=128),
                    )
                    tiles.append(t)
                xq_tiles[tci] = tiles

            # startup: the first QKV chain consumes (wq_g, x_g) pairs in
            # quarter order, so interleave them pairwise on the fast sync
            # ring; gpsimd carries the RoPE tables + K/V weights (needed
            # a few microseconds later).
            x0_tiles = []
            for g in range(4):
                emit_w_dma(wq_ts, wqp, g, nc.sync)
                rs = slice(g * CQ * 128, (g + 1) * CQ * 128)
                t = xsp.tile([128, CQ * TC], dt.bfloat16, tag="xq", name=f"x_0_{g}")
                nc.sync.dma_start(
                    t.rearrange("p (c n) -> p c n", c=CQ),
                    xt[rs, 0:TC].rearrange("(c p) n -> p c n", p=128),
                )
                x0_tiles.append(t)
            xq_tiles[0] = x0_tiles
            nc.gpsimd.dma_start(cs_t[:], csT[:])
            emit_w_dma(wk_ts, wkp, 0, nc.gpsimd)
            emit_w_dma(wk_ts, wkp, 1, nc.gpsimd)
            emit_w_dma(wv_ts, wvp, 0, nc.gpsimd)
            emit_w_dma(wv_ts, wvp, 1, nc.gpsimd)
            for i in range(4):
                nc.gpsimd.dma_start(mask_t[i][:], masks[i * 128:(i + 1) * 128, :])
            nc.gpsimd.dma_start(ident_t[:], ident[:])
            nc.sync.dma_start(
                wo_t.rearrange("p (c n) -> p c n", c=CO),
                woh.rearrange("(c p) n -> p c n", p=128),
            )

            # K/V persist per 512-chunk / 128-block (no cross-chunk tiles,
            # so interleaved next-chunk RoPE writes never alias attention
            # reads at the dep tracker's granularity)
            ktc = [[actp.tile([128, TC], dt.bfloat16, tag=f"kt{h}_{j}", name=f"kt{h}_{j}")
                    for j in range(NTC)] for h in range(NKV)]
            # V tiles carry a ones column per kv head (col kv*(HD+1)+HD) so
            # the flipped PV matmul emits softmax denominators for free
            vt = [actp.tile([128, NKV * (HD + 1)], dt.bfloat16, tag=f"vt{b}", name=f"vt{b}") for b in range(TB)]
            for b in range(TB):
                for kv in range(NKV):
                    nc.vector.memset(vt[b][:, kv * (HD + 1) + HD:(kv + 1) * (HD + 1)], 1.0)
            qt_all = {}
            ot_all = {}
            for tci in range(NTC):
                qt_all[tci] = [cap.tile([128, TC], dt.bfloat16, tag=f"qt{h}_{tci % 2}", name=f"qt{h}_{tci}") for h in range(NQ)]
                ot_all[tci] = [cap.tile([128, TC], dt.bfloat16, tag=f"ot{h}_{tci % 2}", name=f"ot{h}_{tci}") for h in range(NQ)]

            # ---------------- unit generators ----------------
            def rope_epilogue(tci, h, ps):
                ts_ = slice(tci * TC, (tci + 1) * TC)
                rot = scr.tile([128, TC], dt.bfloat16, tag="rope", bufs=2)
                t0 = scr.tile([128, TC], dt.bfloat16, tag="ropetmp", bufs=1)
                cs = cos_t[:, ts_]
                sn = sin_t[:, ts_]
                xr = ps[0:64, :]
                xi = ps[64:128, :]
                dsth = qt_all[tci][h][:] if h < NQ else ktc[h - NQ][tci][:]
                nc.vector.tensor_tensor(rot[0:64, :], xr, cs, mybir.AluOpType.mult)
                nc.vector.tensor_tensor(t0[0:64, :], xi, sn, mybir.AluOpType.mult)
                nc.vector.tensor_tensor(dsth[0:64, :], rot[0:64, :], t0[0:64, :], mybir.AluOpType.subtract)
                nc.vector.tensor_tensor(rot[64:128, :], xr, sn, mybir.AluOpType.mult)
                nc.vector.tensor_tensor(t0[64:128, :], xi, cs, mybir.AluOpType.mult)
                nc.vector.tensor_tensor(dsth[64:128, :], rot[64:128, :], t0[64:128, :], mybir.AluOpType.add)

            def qkv_units(tci, ptag, pbufs, group=4):
                """Closures emitting `group` matmuls of a QKV/V chain each
                (coarser units keep chain psum lifetimes short)."""
                units = []
                state = {}

                def x_c(c):
                    xg = xq_tiles[tci]
                    return xg[c // CQ][:, (c % CQ) * TC:(c % CQ + 1) * TC]

                def qk_seg(h, c0):
                    if c0 == 0:
                        state[h] = psp.tile([128, TC], dt.float32, tag=ptag,
                                            bufs=pbufs, name=f"qkv_{tci}_{h}")
                    ps = state[h]
                    for c in range(c0, c0 + group):
                        wsl = wq_sl(c, h) if h < NQ else wk_sl(c, h - NQ)
                        nc.tensor.matmul(ps[:], wsl, x_c(c),
                                         start=(c == 0), stop=(c == C - 1))
                    if c0 + group == C:
                        rope_epilogue(tci, h, ps)

                def v_seg(tb, c0):
                    key = "v", tb
                    if c0 == 0:
                        state[key] = psp.tile([128, NKV * HD], dt.float32,
                                              tag=ptag, bufs=pbufs,
                                              name=f"v_{tci}_{tb}")
                    ps = state[key]
                    for c in range(c0, c0 + group):
                        nc.tensor.matmul(ps[:], x_c(c)[:, tb * 128:(tb + 1) * 128],
                                         wv_sl(c),
                                         start=(c == 0), stop=(c == C - 1))
                    if c0 + group == C:
                        for kv in range(NKV):
                            nc.scalar.copy(
                                vt[tci * NTB + tb][:, kv * (HD + 1):kv * (HD + 1) + HD],
                                ps[:, kv * HD:(kv + 1) * HD])

                for h in range(NQ + NKV):
                    for c0 in range(0, C, group):
                        units.append(lambda h=h, c0=c0: qk_seg(h, c0))
                for tb in range(NTB):
                    for c0 in range(0, C, group):
                        units.append(lambda tb=tb, c0=c0: v_seg(tb, c0))
                return units

            def op_units(tci, ptag, pbufs):
                """One closure per output-projection db group (4 matmuls +
                copy + store)."""
                ts_ = slice(tci * TC, (tci + 1) * TC)
                ot = ot_all[tci]
                units = []
                for db in range(DB):
                    def u(db=db):
                        ps = psp.tile([128, TC], dt.float32, tag=ptag, bufs=pbufs,
                                      name=f"op_{tci}_{db}")
                        for c in range(CO):
                            nc.tensor.matmul(
                                ps[:], wo_t[:, c * D + db * 128:c * D + (db + 1) * 128],
                                ot[c][:],
                                start=(c == 0), stop=(c == CO - 1),
                            )
                        o3 = scr.tile([128, TC], dt.bfloat16, tag="o3", bufs=4)
                        if tci == NTC - 1 and db % 2 == 1:
                            nc.vector.tensor_copy(o3[:], ps[:])
                        else:
                            nc.scalar.copy(o3[:], ps[:])
                        eng = nc.sync if db % 2 == 0 else nc.gpsimd
                        eng.dma_start(outT[db * 128:(db + 1) * 128, ts_], o3[:])
                    units.append(u)
                return units

            # ---------------- attention emission ----------------
            def emit_attention(tci, filler):
                """Scores + flipped PV for q-chunk tci, draining `filler`
                closures between steps to keep the PE busy. PV runs
                qsb-major: each (head, q-subtile) accumulates [q,HD+1]
                (output + denominator column) as a single group in its own
                PSUM bank -- interleaved groups in one bank corrupt."""
                qc = tci
                qt = qt_all[tci]
                ot = ot_all[tci]
                nkb = (qc + 1) * NTB
                nf = len(filler)
                total_steps = (2 * NQ + 1) * nkb
                state = {"step": 0, "drained": 0}

                def drain():
                    state["step"] += 1
                    want = state["step"] * nf // total_steps
                    while state["drained"] < want:
                        filler[state["drained"]]()
                        state["drained"] += 1

                def emit_scores(h, kb):
                    kv = h // REP
                    d = kb * 128 - qc * TC
                    q0 = max(d, 0)
                    sc_ps = psp.tile([128, TC], dt.float32, tag="sc", bufs=3,
                                     name=f"sc_{tci}_{h}_{kb}")
                    ksl = ktc[kv][kb // NTB][:, (kb % NTB) * 128:(kb % NTB + 1) * 128]
                    nc.tensor.matmul(sc_ps[:, q0:TC], ksl, qt[h][:, q0:TC],
                                     start=True, stop=True)
                    ph = scr.tile([128, TC], dt.bfloat16, tag="ph", bufs=36,
                                  name=f"ph_{tci}_{h}_{kb}")
                    nc.scalar.activation(ph[:, q0:TC], sc_ps[:, q0:TC], AF.Exp,
                                         bias=0.0, scale=scale)
                    if d >= 0:
                        nc.vector.tensor_tensor(ph[:, q0:TC], ph[:, q0:TC],
                                                mask_t[d // 128][:, q0:TC],
                                                mybir.AluOpType.mult)
                    return ph

                def norm_one(h, qsb, t_):
                    rec = scr.tile([128, 1], dt.float32, tag="recq", bufs=4,
                                   name=f"rec_{tci}_{h}_{qsb}")
                    nc.vector.reciprocal(rec[:], t_[:, HD:HD + 1])
                    otn = scr.tile([128, HD], dt.bfloat16, tag="otn", bufs=4,
                                   name=f"otn_{tci}_{h}_{qsb}")
                    nc.vector.tensor_scalar(otn[:], t_[:, 0:HD], rec[:], None,
                                            mybir.AluOpType.mult)
                    tp = psp.tile([128, 128], dt.bfloat16, tag="sc", bufs=3,
                                  name=f"tp_{tci}_{h}_{qsb}")
                    nc.tensor.transpose(tp[:], otn[:], ident_t[:])
                    nc.scalar.copy(ot[h][:, qsb * 128:(qsb + 1) * 128], tp[:])

                # head software pipeline: scores of head h+1 are emitted
                # between PV chain steps of head h, so the scalar exp chain
                # for the next head runs under the current head's PE work
                all_phs = {}

                def score_units(h):
                    def u(h=h, kb=None):
                        pass
                    us = []
                    for kb in range(nkb):
                        def uu(h=h, kb=kb):
                            all_phs.setdefault(h, []).append(emit_scores(h, kb))
                        us.append(uu)
                    return us

                def chains_for(h, next_scores):
                    kv = h // REP
                    phs = all_phs[h]
                    nsteps = 2 * nkb
                    si = 0
                    step = 0
                    for qsb0 in (0, 2):
                        chains = []
                        for qsb in (qsb0, qsb0 + 1):
                            t_ = psp.tile([128, HD + 1], dt.float32, tag="otq",
                                          bufs=2, name=f"otq_{tci}_{h}_{qsb}")
                            chains.append((qsb, t_))
                        for kb in range(nkb):
                            d = kb * 128 - qc * TC
                            q0 = max(d, 0)
                            vsl = vt[kb][:, kv * (HD + 1):(kv + 1) * (HD + 1)]
                            for qsb, t_ in chains:
                                if kb > qc * NTB + qsb or qsb < q0 // 128:
                                    continue
                                nc.tensor.matmul(
                                    t_[:], phs[kb][:, qsb * 128:(qsb + 1) * 128], vsl,
                                    start=(kb == 0), stop=(kb == qc * NTB + qsb),
                                )
                            step += 1
                            want_s = step * len(next_scores) // nsteps
                            while si < want_s:
                                next_scores[si]()
                                si += 1
                            drain()
                        for qsb, t_ in chains:
                            norm_one(h, qsb, t_)
                    while si < len(next_scores):
                        next_scores[si]()
                        si += 1

                for u in score_units(0):
                    u()
                    drain()
                for h in range(NQ):
                    chains_for(h, score_units(h + 1) if h + 1 < NQ else [])
                while state["drained"] < nf:
                    filler[state["drained"]]()
                    state["drained"] += 1

            # ---------------- schedule ----------------
            # QKV0 standalone; attn(t) interleaves QKV(t+1) and outproj(t-1);
            # outproj(NTC-1) standalone.
            for u in qkv_units(0, "sc", 3):
                u()
            for tci in range(NTC):
                if tci + 1 < NTC:
                    emit_x_dmas(tci + 1, [nc.sync, nc.sync, nc.gpsimd, nc.gpsimd])
                filler = []
                if tci > 0:
                    filler += op_units(tci - 1, "op", 1)
                if tci + 1 < NTC:
                    filler += qkv_units(tci + 1, "fill", 2)
                emit_attention(tci, filler)
                xq_tiles.pop(tci)
            for u in op_units(NTC - 1, "sc", 3):
                u()

    return nc


# ---------------------------------------------------------------------------
# walrus in this container refuses >1 sem wait per instruction ("Too many
# sync wait commands"). Hoist excess waits onto same-engine NoOps inserted
# immediately before the instruction - program order on the engine queue
# preserves the sync semantics.
def split_multiwait_insts(nc, max_waits=1):
    n_split = 0
    for bb in nc.main_func.blocks:
        insts = bb.instructions
        i = 0
        while i < len(insts):
            ins = insts[i]
            si = getattr(ins, "sync_info", None)
            if si is not None and si.on_wait and len(si.on_wait) > max_waits:
                waits = list(si.on_wait)
                head, tail = waits[:-max_waits], waits[-max_waits:]
                nops = []
                for j in range(0, len(head), max_waits):
                    nop = mybir.InstNoOp(name=f"{ins.name}-ws{j}", ins=[], outs=[])
                    nop.engine = ins.engine
                    nop.sync_info = mybir.SyncInfo(
                        on_wait=head[j:j + max_waits], on_update=[])
                    nops.append(nop)
                ins.sync_info = mybir.SyncInfo(
                    on_wait=tail, on_update=list(si.on_update or []))
                insts[i:i] = nops
                i += len(nops)
                n_split += 1
            i += 1
    return n_split


# ---------------------------------------------------------------------------
# Host-side shard preparation / gather
BF16 = ml_dtypes.bfloat16


def rope_tables(S, HD):
    inv = 1.0 / (10000.0 ** (np.arange(0, HD, 2, dtype=np.float32) / HD))
    t = np.arange(S, dtype=np.float32)
    f = np.outer(t, inv).astype(np.float32)  # [S, HD//2]
    return np.ascontiguousarray(np.cos(f).T), np.ascontiguousarray(np.sin(f).T)


def causal_masks(TC):
    # masks[dd][k, qrel] = 1 if k + dd*128 <= qrel else 0
    out = np.zeros((4 * 128, TC), BF16)
    k = np.arange(128)[:, None]
    q = np.arange(TC)[None, :]
    for dd in range(4):
        out[dd * 128:(dd + 1) * 128] = (k + dd * 128 <= q).astype(BF16)
    return out


def rope_perm(HD):
    # new row i (i < HD//2) = old 2i; new row HD//2+i = old 2i+1
    return np.concatenate([np.arange(0, HD, 2), np.arange(1, HD, 2)])


def make_in_maps(x, wq, wk, wv, wo, *, n_batch_shards, n_head_shards,
                 NQ_TOT, NKV_TOT, HD, TC):
    """Returns list of in_maps, one per core (batch-major: core = b*G + g)."""
    B, S, D = x.shape
    G = n_head_shards
    NQ = NQ_TOT // G
    NKV = NKV_TOT // G
    perm = rope_perm(HD)
    cosT, sinT = rope_tables(S, HD)
    csT = np.concatenate([cosT, sinT], axis=0).astype(BF16)  # [HD, S]
    masks = causal_masks(TC)

    # Per-batch xT (shared across head shards)
    xtb = {}
    for b in range(B):
        xtb[b] = np.ascontiguousarray(x[b].T).astype(BF16)  # [D, S]

    # Per-headgroup weight shards
    wshard = {}
    for g in range(G):
        qrows = slice(g * NQ * HD, (g + 1) * NQ * HD)
        kvrows = slice(g * NKV * HD, (g + 1) * NKV * HD)
        wq_g = wq[qrows, :].copy()      # [NQ*HD, D]
        wk_g = wk[kvrows, :].copy()
        wv_g = wv[kvrows, :].copy()
        # RoPE permutation of output rows, per head
        for hh in range(NQ):
            blk = wq_g[hh * HD:(hh + 1) * HD]
            wq_g[hh * HD:(hh + 1) * HD] = blk[perm]
        for hh in range(NKV):
            blk = wk_g[hh * HD:(hh + 1) * HD]
            wk_g[hh * HD:(hh + 1) * HD] = blk[perm]
        wqT = np.ascontiguousarray(wq_g.T).astype(BF16)   # [D, NQ*HD]
        wkT = np.ascontiguousarray(wk_g.T).astype(BF16)
        wvT = np.ascontiguousarray(wv_g.T).astype(BF16)
        woT = np.ascontiguousarray(wo[:, qrows].T).astype(BF16)  # [NQ*HD, D]
        wshard[g] = (wqT, wkT, wvT, woT)

    ident = np.eye(128, dtype=BF16)
    in_maps = []
    for b in range(n_batch_shards):
        for g in range(G):
            wqT, wkT, wvT, woT = wshard[g]
            in_maps.append({
                "xt": xtb[b],
                "wqp": wqT, "wkp": wkT, "wvp": wvT, "woh": woT,
                "csT": csT,
                "masks": masks,
                "ident": ident,
            })
    return in_maps


def combine_outputs(outTs, B, G):
    """outTs: list of [D, S] partials, core order b*G+g. Returns [B, S, D]."""
    outs = []
    for b in range(B):
        acc = outTs[b * G].astype(np.float32).copy()
        for g in range(1, G):
            acc += outTs[b * G + g]
        outs.append(acc.T)  # [S, D]
    return np.stack(outs)


_NC_CACHE = {}


def _get_nc(S, D, NQ, NKV, HD, TC):
    key = (S, D, NQ, NKV, HD, TC)
    if key not in _NC_CACHE:
        nc = build_attention_nc(S=S, D=D, NQ=NQ, NKV=NKV, HD=HD, TC=TC)
        split_multiwait_insts(nc)
        _NC_CACHE[key] = nc
    return _NC_CACHE[key]


def kernel(**inputs):
    x = np.asarray(inputs["x"], dtype=np.float32)
    wq = np.asarray(inputs["wq"], dtype=np.float32)
    wk = np.asarray(inputs["wk"], dtype=np.float32)
    wv = np.asarray(inputs["wv"], dtype=np.float32)
    wo = np.asarray(inputs["wo"], dtype=np.float32)

    B, S, D = x.shape          # (2, 2048, 2048)
    NQ_TOT = wq.shape[0] // 128
    NKV_TOT = wk.shape[0] // 128
    HD = 128
    TC = 512
    G = 4                      # head shards
    NQ, NKV = NQ_TOT // G, NKV_TOT // G

    nc = _get_nc(S, D, NQ, NKV, HD, TC)
    in_maps = make_in_maps(
        x, wq, wk, wv, wo,
        n_batch_shards=B, n_head_shards=G,
        NQ_TOT=NQ_TOT, NKV_TOT=NKV_TOT, HD=HD, TC=TC,
    )

    from concourse.bass_utils import run_bass_kernel_spmd

    trace = os.environ.get("BASS_ATTN_TRACE") == "1"
    res = run_bass_kernel_spmd(nc, in_maps, list(range(len(in_maps))), trace=trace)
    kernel.last_results = res
    outTs = [r["outT"] for r in res.results]
    return combine_outputs(outTs, B, G).astype(np.float32)


# revision 37
# speedup vs baseline: 2.9109x; 1.0036x over previous
"""Trainium2 Bass kernel for nn_Attention_77043123355775.

Sharded GQA causal attention with RoPE: 8 NeuronCores as 2-way data
parallel (batch) x 4-way tensor parallel (heads). Each core computes its
4 Q heads / 2 KV heads for one batch entry and a partial output
projection (x[b] @ W)^T; the host sums the 4 partials per batch.

All matmuls run in plain bf16 with fp32 PSUM accumulation (the 2e-2
rel-err budget has ~20x headroom over bf16 rounding noise). Weights are
fully SBUF-resident. The PE instruction stream interleaves next-chunk
QKV/V projection chains and prev-chunk output-projection groups between
attention blocks, so the PE never waits on the scalar-engine exp chain
and stays in its high DVFS p-state.
"""
import math
import os
import sys

for _p in ("/opt/trn_rl_repo",):
    if _p not in sys.path:
        sys.path.insert(0, _p)

import ml_dtypes
import numpy as np

import concourse.bass as bass
import concourse.mybir as mybir
import concourse.tile as tile

from concourse.tile import add_dep_helper

dt = mybir.dt
AF = mybir.ActivationFunctionType


def build_attention_nc(S=2048, D=2048, NQ=4, NKV=2, HD=128, TC=512):
    assert HD == 128
    C = D // 128          # contraction chunks over features
    TB = S // 128         # 128-token blocks
    NTC = S // TC         # token chunks
    DB = D // 128         # output feature blocks
    CO = NQ * HD // 128   # contraction chunks for wo (= NQ)
    REP = NQ // NKV
    CQ = C // 4           # c-chunks per x quarter-tile
    NTB = TC // 128       # token blocks per chunk
    scale = 1.0 / math.sqrt(HD)

    nc = bass.Bass()

    xt = nc.dram_tensor("xt", [D, S], dt.bfloat16, kind="ExternalInput")
    ident = nc.dram_tensor("ident", [128, 128], dt.bfloat16, kind="ExternalInput")
    wqp = nc.dram_tensor("wqp", [D, NQ * HD], dt.bfloat16, kind="ExternalInput")
    wkp = nc.dram_tensor("wkp", [D, NKV * HD], dt.bfloat16, kind="ExternalInput")
    wvp = nc.dram_tensor("wvp", [D, NKV * HD], dt.bfloat16, kind="ExternalInput")
    woh = nc.dram_tensor("woh", [NQ * HD, D], dt.bfloat16, kind="ExternalInput")
    csT = nc.dram_tensor("csT", [HD, S], dt.bfloat16, kind="ExternalInput")
    masks = nc.dram_tensor("masks", [4 * 128, TC], dt.bfloat16, kind="ExternalInput")
    outT = nc.dram_tensor("outT", [D, S], dt.bfloat16, kind="ExternalOutput")

    with tile.TileContext(nc) as tc:
        with (
            tc.tile_pool(name="const", bufs=1) as constp,
            tc.tile_pool(name="tabs", bufs=1) as tabp,
            tc.tile_pool(name="wts", bufs=1) as wtp,
            tc.tile_pool(name="acts", bufs=1) as actp,
            tc.tile_pool(name="chunkacts", bufs=1) as cap,
            tc.tile_pool(name="xstream", bufs=8) as xsp,
            tc.tile_pool(name="scratch", bufs=3) as scr,
            tc.tile_pool(name="psum", bufs=1, space="PSUM") as psp,
        ):
            ident_t = constp.tile([128, 128], dt.bfloat16, tag="ident")

            # ---- resident tables / weights (c-quartered for fine deps) ----
            def emit_w_dma(tiles, src, g, ring):
                cq = C // len(tiles)
                rs = slice(g * cq * 128, (g + 1) * cq * 128)
                return ring.dma_start(
                    tiles[g].rearrange("p (c n) -> p c n", c=cq),
                    src[rs, :].rearrange("(c p) n -> p c n", p=128),
                )

            wq_ts = [wtp.tile([128, CQ * NQ * HD], dt.bfloat16, tag=f"wq{g}", name=f"wq{g}") for g in range(4)]
            wk_ts = [wtp.tile([128, (C // 2) * NKV * HD], dt.bfloat16, tag=f"wk{g}", name=f"wk{g}") for g in range(2)]
            wv_ts = [wtp.tile([128, (C // 2) * NKV * HD], dt.bfloat16, tag=f"wv{g}", name=f"wv{g}") for g in range(2)]
            wo_t = wtp.tile([128, CO * D], dt.bfloat16, tag="wo")

            def wq_sl(c, h):
                return wq_ts[c // CQ][:, (c % CQ) * NQ * HD + h * HD:(c % CQ) * NQ * HD + (h + 1) * HD]

            def wk_sl(c, h):
                ch = C // 2
                return wk_ts[c // ch][:, (c % ch) * NKV * HD + h * HD:(c % ch) * NKV * HD + (h + 1) * HD]

            def wv_sl(c):
                ch = C // 2
                return wv_ts[c // ch][:, (c % ch) * NKV * HD:(c % ch + 1) * NKV * HD]

            cs_t = tabp.tile([HD, S], dt.bfloat16, tag="cs")
            cos_t = cs_t[0:HD // 2, :]
            sin_t = cs_t[HD // 2:HD, :]
            mask_t = [tabp.tile([128, TC], dt.bfloat16, tag=f"mask{i}", name=f"mask{i}") for i in range(4)]

            xq_tiles = {}

            def emit_x_dmas(tci, rings):
                ts_ = slice(tci * TC, (tci + 1) * TC)
                tiles = []
                for g in range(4):
                    rs = slice(g * CQ * 128, (g + 1) * CQ * 128)
                    t = xsp.tile([128, CQ * TC], dt.bfloat16, tag="xq",
                                 name=f"x_{tci}_{g}")
                    rings[g].dma_start(
                        t.rearrange("p (c n) -> p c n", c=CQ),
                        xt[rs, ts_].rearrange("(c p) n -> p c n", p=128),
                    )
                    tiles.append(t)
                xq_tiles[tci] = tiles

            # startup: the first QKV chain consumes (wq_g, x_g) pairs in
            # quarter order, so interleave them pairwise on the fast sync
            # ring; gpsimd carries the RoPE tables + K/V weights (needed
            # a few microseconds later).
            x0_tiles = []
            for g in range(4):
                emit_w_dma(wq_ts, wqp, g, nc.sync)
                rs = slice(g * CQ * 128, (g + 1) * CQ * 128)
                t = xsp.tile([128, CQ * TC], dt.bfloat16, tag="xq", name=f"x_0_{g}")
                nc.sync.dma_start(
                    t.rearrange("p (c n) -> p c n", c=CQ),
                    xt[rs, 0:TC].rearrange("(c p) n -> p c n", p=128),
                )
                x0_tiles.append(t)
            xq_tiles[0] = x0_tiles
            # The secondary loads (cs/wk/wv/masks) would otherwise compete
            # with the critical wq/x burst for chip HBM bandwidth at t=0
            # (all 8 cores slurp simultaneously); throttle them behind
            # early Q-chain matmuls via explicit deps filled in later.
            startup_dmas = []
            startup_dmas.append(nc.gpsimd.dma_start(cs_t[:], csT[:]))
            startup_dmas.append(emit_w_dma(wk_ts, wkp, 0, nc.gpsimd))
            startup_dmas.append(emit_w_dma(wk_ts, wkp, 1, nc.gpsimd))
            startup_dmas.append(emit_w_dma(wv_ts, wvp, 0, nc.gpsimd))
            startup_dmas.append(emit_w_dma(wv_ts, wvp, 1, nc.gpsimd))
            for i in range(4):
                nc.gpsimd.dma_start(mask_t[i][:], masks[i * 128:(i + 1) * 128, :])
            nc.gpsimd.dma_start(ident_t[:], ident[:])
            nc.sync.dma_start(
                wo_t.rearrange("p (c n) -> p c n", c=CO),
                woh.rearrange("(c p) n -> p c n", p=128),
            )
            startup_mms = []

            # K/V persist per 512-chunk / 128-block (no cross-chunk tiles,
            # so interleaved next-chunk RoPE writes never alias attention
            # reads at the dep tracker's granularity)
            ktc = [[actp.tile([128, TC], dt.bfloat16, tag=f"kt{h}_{j}", name=f"kt{h}_{j}")
                    for j in range(NTC)] for h in range(NKV)]
            # V tiles carry a ones column per kv head (col kv*(HD+1)+HD) so
            # the flipped PV matmul emits softmax denominators for free
            vt = [actp.tile([128, NKV * (HD + 1)], dt.bfloat16, tag=f"vt{b}", name=f"vt{b}") for b in range(TB)]
            for b in range(TB):
                for kv in range(NKV):
                    nc.vector.memset(vt[b][:, kv * (HD + 1) + HD:(kv + 1) * (HD + 1)], 1.0)
            qt_all = {}
            ot_all = {}
            for tci in range(NTC):
                qt_all[tci] = [cap.tile([128, TC], dt.bfloat16, tag=f"qt{h}_{tci % 2}", name=f"qt{h}_{tci}") for h in range(NQ)]
                ot_all[tci] = [cap.tile([128, TC], dt.bfloat16, tag=f"ot{h}_{tci % 2}", name=f"ot{h}_{tci}") for h in range(NQ)]

            # ---------------- unit generators ----------------
            def rope_epilogue(tci, h, ps):
                ts_ = slice(tci * TC, (tci + 1) * TC)
                rot = scr.tile([128, TC], dt.bfloat16, tag="rope", bufs=2)
                t0 = scr.tile([128, TC], dt.bfloat16, tag="ropetmp", bufs=1)
                cs = cos_t[:, ts_]
                sn = sin_t[:, ts_]
                xr = ps[0:64, :]
                xi = ps[64:128, :]
                dsth = qt_all[tci][h][:] if h < NQ else ktc[h - NQ][tci][:]
                nc.vector.tensor_tensor(rot[0:64, :], xr, cs, mybir.AluOpType.mult)
                nc.vector.tensor_tensor(t0[0:64, :], xi, sn, mybir.AluOpType.mult)
                nc.vector.tensor_tensor(dsth[0:64, :], rot[0:64, :], t0[0:64, :], mybir.AluOpType.subtract)
                nc.vector.tensor_tensor(rot[64:128, :], xr, sn, mybir.AluOpType.mult)
                nc.vector.tensor_tensor(t0[64:128, :], xi, cs, mybir.AluOpType.mult)
                nc.vector.tensor_tensor(dsth[64:128, :], rot[64:128, :], t0[64:128, :], mybir.AluOpType.add)

            def qkv_units(tci, ptag, pbufs, group=4):
                """Closures emitting `group` matmuls of a QKV/V chain each
                (coarser units keep chain psum lifetimes short)."""
                units = []
                state = {}

                def x_c(c):
                    xg = xq_tiles[tci]
                    return xg[c // CQ][:, (c % CQ) * TC:(c % CQ + 1) * TC]

                def qk_seg(h, c0):
                    if c0 == 0:
                        state[h] = psp.tile([128, TC], dt.float32, tag=ptag,
                                            bufs=pbufs, name=f"qkv_{tci}_{h}")
                    ps = state[h]
                    for c in range(c0, c0 + group):
                        wsl = wq_sl(c, h) if h < NQ else wk_sl(c, h - NQ)
                        mm = nc.tensor.matmul(ps[:], wsl, x_c(c),
                                              start=(c == 0), stop=(c == C - 1))
                        if tci == 0 and h < 2:
                            startup_mms.append(mm)
                    if c0 + group == C:
                        rope_epilogue(tci, h, ps)

                def v_seg(tb, c0):
                    key = "v", tb
                    if c0 == 0:
                        state[key] = psp.tile([128, NKV * HD], dt.float32,
                                              tag=ptag, bufs=pbufs,
                                              name=f"v_{tci}_{tb}")
                    ps = state[key]
                    for c in range(c0, c0 + group):
                        nc.tensor.matmul(ps[:], x_c(c)[:, tb * 128:(tb + 1) * 128],
                                         wv_sl(c),
                                         start=(c == 0), stop=(c == C - 1))
                    if c0 + group == C:
                        for kv in range(NKV):
                            nc.scalar.copy(
                                vt[tci * NTB + tb][:, kv * (HD + 1):kv * (HD + 1) + HD],
                                ps[:, kv * HD:(kv + 1) * HD])

                for h in range(NQ + NKV):
                    for c0 in range(0, C, group):
                        units.append(lambda h=h, c0=c0: qk_seg(h, c0))
                for tb in range(NTB):
                    for c0 in range(0, C, group):
                        units.append(lambda tb=tb, c0=c0: v_seg(tb, c0))
                return units

            def op_units(tci, ptag, pbufs):
                """One closure per output-projection db group (4 matmuls +
                copy + store)."""
                ts_ = slice(tci * TC, (tci + 1) * TC)
                ot = ot_all[tci]
                units = []
                for db in range(DB):
                    def u(db=db):
                        ps = psp.tile([128, TC], dt.float32, tag=ptag, bufs=pbufs,
                                      name=f"op_{tci}_{db}")
                        for c in range(CO):
                            nc.tensor.matmul(
                                ps[:], wo_t[:, c * D + db * 128:c * D + (db + 1) * 128],
                                ot[c][:],
                                start=(c == 0), stop=(c == CO - 1),
                            )
                        o3 = scr.tile([128, TC], dt.bfloat16, tag="o3", bufs=4)
                        if tci == NTC - 1 and db % 2 == 1:
                            nc.vector.tensor_copy(o3[:], ps[:])
                        else:
                            nc.scalar.copy(o3[:], ps[:])
                        eng = nc.sync if db % 2 == 0 else nc.gpsimd
                        eng.dma_start(outT[db * 128:(db + 1) * 128, ts_], o3[:])
                    units.append(u)
                return units

            # ---------------- attention emission ----------------
            def emit_attention(tci, filler):
                """Scores + flipped PV for q-chunk tci, draining `filler`
                closures between steps to keep the PE busy. PV runs
                qsb-major: each (head, q-subtile) accumulates [q,HD+1]
                (output + denominator column) as a single group in its own
                PSUM bank -- interleaved groups in one bank corrupt."""
                qc = tci
                qt = qt_all[tci]
                ot = ot_all[tci]
                nkb = (qc + 1) * NTB
                nf = len(filler)
                total_steps = (2 * NQ + 1) * nkb
                state = {"step": 0, "drained": 0}

                def drain():
                    state["step"] += 1
                    want = state["step"] * nf // total_steps
                    while state["drained"] < want:
                        filler[state["drained"]]()
                        state["drained"] += 1

                def emit_scores(h, kb):
                    kv = h // REP
                    d = kb * 128 - qc * TC
                    q0 = max(d, 0)
                    sc_ps = psp.tile([128, TC], dt.float32, tag="sc", bufs=3,
                                     name=f"sc_{tci}_{h}_{kb}")
                    ksl = ktc[kv][kb // NTB][:, (kb % NTB) * 128:(kb % NTB + 1) * 128]
                    nc.tensor.matmul(sc_ps[:, q0:TC], ksl, qt[h][:, q0:TC],
                                     start=True, stop=True)
                    ph = scr.tile([128, TC], dt.bfloat16, tag="ph", bufs=36,
                                  name=f"ph_{tci}_{h}_{kb}")
                    nc.scalar.activation(ph[:, q0:TC], sc_ps[:, q0:TC], AF.Exp,
                                         bias=0.0, scale=scale)
                    if d >= 0:
                        nc.vector.tensor_tensor(ph[:, q0:TC], ph[:, q0:TC],
                                                mask_t[d // 128][:, q0:TC],
                                                mybir.AluOpType.mult)
                    return ph

                def norm_one(h, qsb, t_):
                    rec = scr.tile([128, 1], dt.float32, tag="recq", bufs=4,
                                   name=f"rec_{tci}_{h}_{qsb}")
                    nc.vector.reciprocal(rec[:], t_[:, HD:HD + 1])
                    otn = scr.tile([128, HD], dt.bfloat16, tag="otn", bufs=4,
                                   name=f"otn_{tci}_{h}_{qsb}")
                    nc.vector.tensor_scalar(otn[:], t_[:, 0:HD], rec[:], None,
                                            mybir.AluOpType.mult)
                    tp = psp.tile([128, 128], dt.bfloat16, tag="sc", bufs=3,
                                  name=f"tp_{tci}_{h}_{qsb}")
                    nc.tensor.transpose(tp[:], otn[:], ident_t[:])
                    nc.scalar.copy(ot[h][:, qsb * 128:(qsb + 1) * 128], tp[:])

                # head software pipeline: scores of head h+1 are emitted
                # between PV chain steps of head h, so the scalar exp chain
                # for the next head runs under the current head's PE work
                all_phs = {}

                def score_units(h):
                    def u(h=h, kb=None):
                        pass
                    us = []
                    for kb in range(nkb):
                        def uu(h=h, kb=kb):
                            all_phs.setdefault(h, []).append(emit_scores(h, kb))
                        us.append(uu)
                    return us

                def chains_for(h, next_scores):
                    kv = h // REP
                    phs = all_phs[h]
                    nsteps = 2 * nkb
                    si = 0
                    step = 0
                    for qsb0 in (0, 2):
                        chains = []
                        for qsb in (qsb0, qsb0 + 1):
                            t_ = psp.tile([128, HD + 1], dt.float32, tag="otq",
                                          bufs=2, name=f"otq_{tci}_{h}_{qsb}")
                            chains.append((qsb, t_))
                        for kb in range(nkb):
                            d = kb * 128 - qc * TC
                            q0 = max(d, 0)
                            vsl = vt[kb][:, kv * (HD + 1):(kv + 1) * (HD + 1)]
                            for qsb, t_ in chains:
                                if kb > qc * NTB + qsb or qsb < q0 // 128:
                                    continue
                                nc.tensor.matmul(
                                    t_[:], phs[kb][:, qsb * 128:(qsb + 1) * 128], vsl,
                                    start=(kb == 0), stop=(kb == qc * NTB + qsb),
                                )
                            step += 1
                            want_s = step * len(next_scores) // nsteps
                            while si < want_s:
                                next_scores[si]()
                                si += 1
                            drain()
                        for qsb, t_ in chains:
                            norm_one(h, qsb, t_)
                    while si < len(next_scores):
                        next_scores[si]()
                        si += 1

                for u in score_units(0):
                    u()
                    drain()
                for h in range(NQ):
                    chains_for(h, score_units(h + 1) if h + 1 < NQ else [])
                while state["drained"] < nf:
                    filler[state["drained"]]()
                    state["drained"] += 1

            # ---------------- schedule ----------------
            # QKV0 standalone; attn(t) interleaves QKV(t+1) and outproj(t-1);
            # outproj(NTC-1) standalone.
            for u in qkv_units(0, "sc", 3):
                u()
            # cs waits Q0-chain c2; wk halves wait Q0 end / Q1 mid; wv later
            _anchors = [2, 14, 18, 22, 26]
            for _d, _a in zip(startup_dmas, _anchors):
                if _d is not None and _a < len(startup_mms):
                    add_dep_helper(_d.ins, startup_mms[_a].ins,
                                   reason="startup HBM burst throttle")
            for tci in range(NTC):
                if tci + 1 < NTC:
                    emit_x_dmas(tci + 1, [nc.sync, nc.sync, nc.gpsimd, nc.gpsimd])
                filler = []
                if tci > 0:
                    filler += op_units(tci - 1, "op", 1)
                if tci + 1 < NTC:
                    filler += qkv_units(tci + 1, "fill", 2)
                emit_attention(tci, filler)
                xq_tiles.pop(tci)
            for u in op_units(NTC - 1, "sc", 3):
                u()

    return nc


# ---------------------------------------------------------------------------
# walrus in this container refuses >1 sem wait per instruction ("Too many
# sync wait commands"). Hoist excess waits onto same-engine NoOps inserted
# immediately before the instruction - program order on the engine queue
# preserves the sync semantics.
def split_multiwait_insts(nc, max_waits=1):
    n_split = 0
    for bb in nc.main_func.blocks:
        insts = bb.instructions
        i = 0
        while i < len(insts):
            ins = insts[i]
            si = getattr(ins, "sync_info", None)
            if si is not None and si.on_wait and len(si.on_wait) > max_waits:
                waits = list(si.on_wait)
                head, tail = waits[:-max_waits], waits[-max_waits:]
                nops = []
                for j in range(0, len(head), max_waits):
                    nop = mybir.InstNoOp(name=f"{ins.name}-ws{j}", ins=[], outs=[])
                    nop.engine = ins.engine
                    nop.sync_info = mybir.SyncInfo(
                        on_wait=head[j:j + max_waits], on_update=[])
                    nops.append(nop)
                ins.sync_info = mybir.SyncInfo(
                    on_wait=tail, on_update=list(si.on_update or []))
                insts[i:i] = nops
                i += len(nops)
                n_split += 1
            i += 1
    return n_split


# ---------------------------------------------------------------------------
# Host-side shard preparation / gather
BF16 = ml_dtypes.bfloat16


def rope_tables(S, HD):
    inv = 1.0 / (10000.0 ** (np.arange(0, HD, 2, dtype=np.float32) / HD))
    t = np.arange(S, dtype=np.float32)
    f = np.outer(t, inv).astype(np.float32)  # [S, HD//2]
    return np.ascontiguousarray(np.cos(f).T), np.ascontiguousarray(np.sin(f).T)


def causal_masks(TC):
    # masks[dd][k, qrel] = 1 if k + dd*128 <= qrel else 0
    out = np.zeros((4 * 128, TC), BF16)
    k = np.arange(128)[:, None]
    q = np.arange(TC)[None, :]
    for dd in range(4):
        out[dd * 128:(dd + 1) * 128] = (k + dd * 128 <= q).astype(BF16)
    return out


def rope_perm(HD):
    # new row i (i < HD//2) = old 2i; new row HD//2+i = old 2i+1
    return np.concatenate([np.arange(0, HD, 2), np.arange(1, HD, 2)])


def make_in_maps(x, wq, wk, wv, wo, *, n_batch_shards, n_head_shards,
                 NQ_TOT, NKV_TOT, HD, TC):
    """Returns list of in_maps, one per core (batch-major: core = b*G + g)."""
    B, S, D = x.shape
    G = n_head_shards
    NQ = NQ_TOT // G
    NKV = NKV_TOT // G
    perm = rope_perm(HD)
    cosT, sinT = rope_tables(S, HD)
    csT = np.concatenate([cosT, sinT], axis=0).astype(BF16)  # [HD, S]
    masks = causal_masks(TC)

    # Per-batch xT (shared across head shards)
    xtb = {}
    for b in range(B):
        xtb[b] = np.ascontiguousarray(x[b].T).astype(BF16)  # [D, S]

    # Per-headgroup weight shards
    wshard = {}
    for g in range(G):
        qrows = slice(g * NQ * HD, (g + 1) * NQ * HD)
        kvrows = slice(g * NKV * HD, (g + 1) * NKV * HD)
        wq_g = wq[qrows, :].copy()      # [NQ*HD, D]
        wk_g = wk[kvrows, :].copy()
        wv_g = wv[kvrows, :].copy()
        # RoPE permutation of output rows, per head
        for hh in range(NQ):
            blk = wq_g[hh * HD:(hh + 1) * HD]
            wq_g[hh * HD:(hh + 1) * HD] = blk[perm]
        for hh in range(NKV):
            blk = wk_g[hh * HD:(hh + 1) * HD]
            wk_g[hh * HD:(hh + 1) * HD] = blk[perm]
        wqT = np.ascontiguousarray(wq_g.T).astype(BF16)   # [D, NQ*HD]
        wkT = np.ascontiguousarray(wk_g.T).astype(BF16)
        wvT = np.ascontiguousarray(wv_g.T).astype(BF16)
        woT = np.ascontiguousarray(wo[:, qrows].T).astype(BF16)  # [NQ*HD, D]
        wshard[g] = (wqT, wkT, wvT, woT)

    ident = np.eye(128, dtype=BF16)
    in_maps = []
    for b in range(n_batch_shards):
        for g in range(G):
            wqT, wkT, wvT, woT = wshard[g]
            in_maps.append({
                "xt": xtb[b],
                "wqp": wqT, "wkp": wkT, "wvp": wvT, "woh": woT,
                "csT": csT,
                "masks": masks,
                "ident": ident,
            })
    return in_maps


def combine_outputs(outTs, B, G):
    """outTs: list of [D, S] partials, core order b*G+g. Returns [B, S, D]."""
    outs = []
    for b in range(B):
        acc = outTs[b * G].astype(np.float32).copy()
        for g in range(1, G):
            acc += outTs[b * G + g]
        outs.append(acc.T)  # [S, D]
    return np.stack(outs)


_NC_CACHE = {}


def _get_nc(S, D, NQ, NKV, HD, TC):
    key = (S, D, NQ, NKV, HD, TC)
    if key not in _NC_CACHE:
        nc = build_attention_nc(S=S, D=D, NQ=NQ, NKV=NKV, HD=HD, TC=TC)
        split_multiwait_insts(nc)
        _NC_CACHE[key] = nc
    return _NC_CACHE[key]


def kernel(**inputs):
    x = np.asarray(inputs["x"], dtype=np.float32)
    wq = np.asarray(inputs["wq"], dtype=np.float32)
    wk = np.asarray(inputs["wk"], dtype=np.float32)
    wv = np.asarray(inputs["wv"], dtype=np.float32)
    wo = np.asarray(inputs["wo"], dtype=np.float32)

    B, S, D = x.shape          # (2, 2048, 2048)
    NQ_TOT = wq.shape[0] // 128
    NKV_TOT = wk.shape[0] // 128
    HD = 128
    TC = 512
    G = 4                      # head shards
    NQ, NKV = NQ_TOT // G, NKV_TOT // G

    nc = _get_nc(S, D, NQ, NKV, HD, TC)
    in_maps = make_in_maps(
        x, wq, wk, wv, wo,
        n_batch_shards=B, n_head_shards=G,
        NQ_TOT=NQ_TOT, NKV_TOT=NKV_TOT, HD=HD, TC=TC,
    )

    from concourse.bass_utils import run_bass_kernel_spmd

    trace = os.environ.get("BASS_ATTN_TRACE") == "1"
    res = run_bass_kernel_spmd(nc, in_maps, list(range(len(in_maps))), trace=trace)
    kernel.last_results = res
    outTs = [r["outT"] for r in res.results]
    return combine_outputs(outTs, B, G).astype(np.float32)


# revision 38
# speedup vs baseline: 2.9126x; 1.0006x over previous
"""Trainium2 Bass kernel for nn_Attention_77043123355775.

Sharded GQA causal attention with RoPE: 8 NeuronCores as 2-way data
parallel (batch) x 4-way tensor parallel (heads). Each core computes its
4 Q heads / 2 KV heads for one batch entry and a partial output
projection (x[b] @ W)^T; the host sums the 4 partials per batch.

All matmuls run in plain bf16 with fp32 PSUM accumulation (the 2e-2
rel-err budget has ~20x headroom over bf16 rounding noise). Weights are
fully SBUF-resident. The PE instruction stream interleaves next-chunk
QKV/V projection chains and prev-chunk output-projection groups between
attention blocks, so the PE never waits on the scalar-engine exp chain
and stays in its high DVFS p-state.
"""
import math
import os
import sys

for _p in ("/opt/trn_rl_repo",):
    if _p not in sys.path:
        sys.path.insert(0, _p)

import ml_dtypes
import numpy as np

import concourse.bass as bass
import concourse.mybir as mybir
import concourse.tile as tile

from concourse.tile import add_dep_helper

dt = mybir.dt
AF = mybir.ActivationFunctionType


def build_attention_nc(S=2048, D=2048, NQ=4, NKV=2, HD=128, TC=512):
    assert HD == 128
    C = D // 128          # contraction chunks over features
    TB = S // 128         # 128-token blocks
    NTC = S // TC         # token chunks
    DB = D // 128         # output feature blocks
    CO = NQ * HD // 128   # contraction chunks for wo (= NQ)
    REP = NQ // NKV
    CQ = C // 4           # c-chunks per x quarter-tile
    NTB = TC // 128       # token blocks per chunk
    scale = 1.0 / math.sqrt(HD)

    nc = bass.Bass()

    xt = nc.dram_tensor("xt", [D, S], dt.bfloat16, kind="ExternalInput")
    ident = nc.dram_tensor("ident", [128, 128], dt.bfloat16, kind="ExternalInput")
    wqp = nc.dram_tensor("wqp", [D, NQ * HD], dt.bfloat16, kind="ExternalInput")
    wkp = nc.dram_tensor("wkp", [D, NKV * HD], dt.bfloat16, kind="ExternalInput")
    wvp = nc.dram_tensor("wvp", [D, NKV * HD], dt.bfloat16, kind="ExternalInput")
    woh = nc.dram_tensor("woh", [NQ * HD, D], dt.bfloat16, kind="ExternalInput")
    csT = nc.dram_tensor("csT", [HD, S], dt.bfloat16, kind="ExternalInput")
    masks = nc.dram_tensor("masks", [4 * 128, TC], dt.bfloat16, kind="ExternalInput")
    outT = nc.dram_tensor("outT", [D, S], dt.bfloat16, kind="ExternalOutput")

    with tile.TileContext(nc) as tc:
        with (
            tc.tile_pool(name="const", bufs=1) as constp,
            tc.tile_pool(name="tabs", bufs=1) as tabp,
            tc.tile_pool(name="wts", bufs=1) as wtp,
            tc.tile_pool(name="acts", bufs=1) as actp,
            tc.tile_pool(name="chunkacts", bufs=1) as cap,
            tc.tile_pool(name="xstream", bufs=8) as xsp,
            tc.tile_pool(name="scratch", bufs=3) as scr,
            tc.tile_pool(name="psum", bufs=1, space="PSUM") as psp,
        ):
            ident_t = constp.tile([128, 128], dt.bfloat16, tag="ident")

            # ---- resident tables / weights (c-quartered for fine deps) ----
            def emit_w_dma(tiles, src, g, ring):
                cq = C // len(tiles)
                rs = slice(g * cq * 128, (g + 1) * cq * 128)
                return ring.dma_start(
                    tiles[g].rearrange("p (c n) -> p c n", c=cq),
                    src[rs, :].rearrange("(c p) n -> p c n", p=128),
                )

            wq_ts = [wtp.tile([128, CQ * NQ * HD], dt.bfloat16, tag=f"wq{g}", name=f"wq{g}") for g in range(4)]
            wk_ts = [wtp.tile([128, (C // 2) * NKV * HD], dt.bfloat16, tag=f"wk{g}", name=f"wk{g}") for g in range(2)]
            wv_ts = [wtp.tile([128, (C // 2) * NKV * HD], dt.bfloat16, tag=f"wv{g}", name=f"wv{g}") for g in range(2)]
            wo_t = wtp.tile([128, CO * D], dt.bfloat16, tag="wo")

            def wq_sl(c, h):
                return wq_ts[c // CQ][:, (c % CQ) * NQ * HD + h * HD:(c % CQ) * NQ * HD + (h + 1) * HD]

            def wk_sl(c, h):
                ch = C // 2
                return wk_ts[c // ch][:, (c % ch) * NKV * HD + h * HD:(c % ch) * NKV * HD + (h + 1) * HD]

            def wv_sl(c):
                ch = C // 2
                return wv_ts[c // ch][:, (c % ch) * NKV * HD:(c % ch + 1) * NKV * HD]

            cs_t = tabp.tile([HD, S], dt.bfloat16, tag="cs")
            cos_t = cs_t[0:HD // 2, :]
            sin_t = cs_t[HD // 2:HD, :]
            mask_t = [tabp.tile([128, TC], dt.bfloat16, tag=f"mask{i}", name=f"mask{i}") for i in range(4)]

            xq_tiles = {}

            def emit_x_dmas(tci, rings):
                ts_ = slice(tci * TC, (tci + 1) * TC)
                tiles = []
                for g in range(4):
                    rs = slice(g * CQ * 128, (g + 1) * CQ * 128)
                    t = xsp.tile([128, CQ * TC], dt.bfloat16, tag="xq",
                                 name=f"x_{tci}_{g}")
                    rings[g].dma_start(
                        t.rearrange("p (c n) -> p c n", c=CQ),
                        xt[rs, ts_].rearrange("(c p) n -> p c n", p=128),
                    )
                    tiles.append(t)
                xq_tiles[tci] = tiles

            # startup: the first QKV chain consumes (wq_g, x_g) pairs in
            # quarter order, so interleave them pairwise on the fast sync
            # ring; gpsimd carries the RoPE tables + K/V weights (needed
            # a few microseconds later).
            x0_tiles = []
            for g in range(4):
                emit_w_dma(wq_ts, wqp, g, nc.sync)
                rs = slice(g * CQ * 128, (g + 1) * CQ * 128)
                t = xsp.tile([128, CQ * TC], dt.bfloat16, tag="xq", name=f"x_0_{g}")
                nc.sync.dma_start(
                    t.rearrange("p (c n) -> p c n", c=CQ),
                    xt[rs, 0:TC].rearrange("(c p) n -> p c n", p=128),
                )
                x0_tiles.append(t)
            xq_tiles[0] = x0_tiles
            # The secondary loads (cs/wk/wv/masks) would otherwise compete
            # with the critical wq/x burst for chip HBM bandwidth at t=0
            # (all 8 cores slurp simultaneously); throttle them behind
            # early Q-chain matmuls via explicit deps filled in later.
            startup_dmas = []
            startup_dmas.append(nc.gpsimd.dma_start(cs_t[:], csT[:]))
            startup_dmas.append(emit_w_dma(wk_ts, wkp, 0, nc.gpsimd))
            startup_dmas.append(emit_w_dma(wk_ts, wkp, 1, nc.gpsimd))
            startup_dmas.append(emit_w_dma(wv_ts, wvp, 0, nc.gpsimd))
            startup_dmas.append(emit_w_dma(wv_ts, wvp, 1, nc.gpsimd))
            for i in range(4):
                nc.gpsimd.dma_start(mask_t[i][:], masks[i * 128:(i + 1) * 128, :])
            nc.gpsimd.dma_start(ident_t[:], ident[:])
            nc.sync.dma_start(
                wo_t.rearrange("p (c n) -> p c n", c=CO),
                woh.rearrange("(c p) n -> p c n", p=128),
            )
            startup_mms = []

            # K/V persist per 512-chunk / 128-block (no cross-chunk tiles,
            # so interleaved next-chunk RoPE writes never alias attention
            # reads at the dep tracker's granularity)
            ktc = [[actp.tile([128, TC], dt.bfloat16, tag=f"kt{h}_{j}", name=f"kt{h}_{j}")
                    for j in range(NTC)] for h in range(NKV)]
            # V tiles carry a ones column per kv head (col kv*(HD+1)+HD) so
            # the flipped PV matmul emits softmax denominators for free
            vt = [actp.tile([128, NKV * (HD + 1)], dt.bfloat16, tag=f"vt{b}", name=f"vt{b}") for b in range(TB)]
            for b in range(TB):
                for kv in range(NKV):
                    nc.vector.memset(vt[b][:, kv * (HD + 1) + HD:(kv + 1) * (HD + 1)], 1.0)
            qt_all = {}
            ot_all = {}
            for tci in range(NTC):
                qt_all[tci] = [cap.tile([128, TC], dt.bfloat16, tag=f"qt{h}_{tci % 2}", name=f"qt{h}_{tci}") for h in range(NQ)]
                ot_all[tci] = [cap.tile([128, TC], dt.bfloat16, tag=f"ot{h}_{tci % 2}", name=f"ot{h}_{tci}") for h in range(NQ)]

            # ---------------- unit generators ----------------
            def rope_epilogue(tci, h, ps):
                ts_ = slice(tci * TC, (tci + 1) * TC)
                rot = scr.tile([128, TC], dt.bfloat16, tag="rope", bufs=2)
                t0 = scr.tile([128, TC], dt.bfloat16, tag="ropetmp", bufs=1)
                cs = cos_t[:, ts_]
                sn = sin_t[:, ts_]
                xr = ps[0:64, :]
                xi = ps[64:128, :]
                dsth = qt_all[tci][h][:] if h < NQ else ktc[h - NQ][tci][:]
                nc.vector.tensor_tensor(rot[0:64, :], xr, cs, mybir.AluOpType.mult)
                nc.vector.tensor_tensor(t0[0:64, :], xi, sn, mybir.AluOpType.mult)
                nc.vector.tensor_tensor(dsth[0:64, :], rot[0:64, :], t0[0:64, :], mybir.AluOpType.subtract)
                nc.vector.tensor_tensor(rot[64:128, :], xr, sn, mybir.AluOpType.mult)
                nc.vector.tensor_tensor(t0[64:128, :], xi, cs, mybir.AluOpType.mult)
                nc.vector.tensor_tensor(dsth[64:128, :], rot[64:128, :], t0[64:128, :], mybir.AluOpType.add)

            def qkv_units(tci, ptag, pbufs, group=4):
                """Closures emitting `group` matmuls of a QKV/V chain each
                (coarser units keep chain psum lifetimes short)."""
                units = []
                state = {}

                def x_c(c):
                    xg = xq_tiles[tci]
                    return xg[c // CQ][:, (c % CQ) * TC:(c % CQ + 1) * TC]

                def qk_seg(h, c0):
                    if c0 == 0:
                        state[h] = psp.tile([128, TC], dt.float32, tag=ptag,
                                            bufs=pbufs, name=f"qkv_{tci}_{h}")
                    ps = state[h]
                    for c in range(c0, c0 + group):
                        wsl = wq_sl(c, h) if h < NQ else wk_sl(c, h - NQ)
                        mm = nc.tensor.matmul(ps[:], wsl, x_c(c),
                                              start=(c == 0), stop=(c == C - 1))
                        if tci == 0 and h < 2:
                            startup_mms.append(mm)
                    if c0 + group == C:
                        rope_epilogue(tci, h, ps)

                def v_seg(tb, c0):
                    key = "v", tb
                    if c0 == 0:
                        state[key] = psp.tile([128, NKV * HD], dt.float32,
                                              tag=ptag, bufs=pbufs,
                                              name=f"v_{tci}_{tb}")
                    ps = state[key]
                    for c in range(c0, c0 + group):
                        nc.tensor.matmul(ps[:], x_c(c)[:, tb * 128:(tb + 1) * 128],
                                         wv_sl(c),
                                         start=(c == 0), stop=(c == C - 1))
                    if c0 + group == C:
                        for kv in range(NKV):
                            nc.scalar.copy(
                                vt[tci * NTB + tb][:, kv * (HD + 1):kv * (HD + 1) + HD],
                                ps[:, kv * HD:(kv + 1) * HD])

                for h in range(NQ + NKV):
                    for c0 in range(0, C, group):
                        units.append(lambda h=h, c0=c0: qk_seg(h, c0))
                for tb in range(NTB):
                    for c0 in range(0, C, group):
                        units.append(lambda tb=tb, c0=c0: v_seg(tb, c0))
                return units

            def op_units(tci, ptag, pbufs):
                """One closure per output-projection db group (4 matmuls +
                copy + store)."""
                ts_ = slice(tci * TC, (tci + 1) * TC)
                ot = ot_all[tci]
                units = []
                for db in range(DB):
                    def u(db=db):
                        ps = psp.tile([128, TC], dt.float32, tag=ptag, bufs=pbufs,
                                      name=f"op_{tci}_{db}")
                        for c in range(CO):
                            nc.tensor.matmul(
                                ps[:], wo_t[:, c * D + db * 128:c * D + (db + 1) * 128],
                                ot[c][:],
                                start=(c == 0), stop=(c == CO - 1),
                            )
                        o3 = scr.tile([128, TC], dt.bfloat16, tag="o3", bufs=4)
                        if tci == NTC - 1 and db % 2 == 1:
                            nc.vector.tensor_copy(o3[:], ps[:])
                        else:
                            nc.scalar.copy(o3[:], ps[:])
                        eng = nc.sync if db % 2 == 0 else nc.gpsimd
                        eng.dma_start(outT[db * 128:(db + 1) * 128, ts_], o3[:])
                    units.append(u)
                return units

            # ---------------- attention emission ----------------
            def emit_attention(tci, filler):
                """Scores + flipped PV for q-chunk tci, draining `filler`
                closures between steps to keep the PE busy. PV runs
                qsb-major: each (head, q-subtile) accumulates [q,HD+1]
                (output + denominator column) as a single group in its own
                PSUM bank -- interleaved groups in one bank corrupt."""
                qc = tci
                qt = qt_all[tci]
                ot = ot_all[tci]
                nkb = (qc + 1) * NTB
                nf = len(filler)
                total_steps = (2 * NQ + 1) * nkb
                state = {"step": 0, "drained": 0}

                def drain():
                    state["step"] += 1
                    want = state["step"] * nf // total_steps
                    while state["drained"] < want:
                        filler[state["drained"]]()
                        state["drained"] += 1

                def emit_scores(h, kb):
                    kv = h // REP
                    d = kb * 128 - qc * TC
                    q0 = max(d, 0)
                    sc_ps = psp.tile([128, TC], dt.float32, tag="sc", bufs=3,
                                     name=f"sc_{tci}_{h}_{kb}")
                    ksl = ktc[kv][kb // NTB][:, (kb % NTB) * 128:(kb % NTB + 1) * 128]
                    nc.tensor.matmul(sc_ps[:, q0:TC], ksl, qt[h][:, q0:TC],
                                     start=True, stop=True)
                    ph = scr.tile([128, TC], dt.bfloat16, tag="ph", bufs=36,
                                  name=f"ph_{tci}_{h}_{kb}")
                    nc.scalar.activation(ph[:, q0:TC], sc_ps[:, q0:TC], AF.Exp,
                                         bias=0.0, scale=scale)
                    if d >= 0:
                        nc.vector.tensor_tensor(ph[:, q0:TC], ph[:, q0:TC],
                                                mask_t[d // 128][:, q0:TC],
                                                mybir.AluOpType.mult)
                    return ph

                def norm_one(h, qsb, t_):
                    rec = scr.tile([128, 1], dt.float32, tag="recq", bufs=4,
                                   name=f"rec_{tci}_{h}_{qsb}")
                    nc.vector.reciprocal(rec[:], t_[:, HD:HD + 1])
                    otn = scr.tile([128, HD], dt.bfloat16, tag="otn", bufs=4,
                                   name=f"otn_{tci}_{h}_{qsb}")
                    nc.vector.tensor_scalar(otn[:], t_[:, 0:HD], rec[:], None,
                                            mybir.AluOpType.mult)
                    tp = psp.tile([128, 128], dt.bfloat16, tag="sc", bufs=3,
                                  name=f"tp_{tci}_{h}_{qsb}")
                    nc.tensor.transpose(tp[:], otn[:], ident_t[:])
                    nc.scalar.copy(ot[h][:, qsb * 128:(qsb + 1) * 128], tp[:])

                # head software pipeline: scores of head h+1 are emitted
                # between PV chain steps of head h, so the scalar exp chain
                # for the next head runs under the current head's PE work
                all_phs = {}

                def score_units(h):
                    def u(h=h, kb=None):
                        pass
                    us = []
                    for kb in range(nkb):
                        def uu(h=h, kb=kb):
                            all_phs.setdefault(h, []).append(emit_scores(h, kb))
                        us.append(uu)
                    return us

                def chains_for(h, next_scores):
                    kv = h // REP
                    phs = all_phs[h]
                    nsteps = 2 * nkb
                    si = 0
                    step = 0
                    for qsb0 in (0, 2):
                        chains = []
                        for qsb in (qsb0, qsb0 + 1):
                            t_ = psp.tile([128, HD + 1], dt.float32, tag="otq",
                                          bufs=2, name=f"otq_{tci}_{h}_{qsb}")
                            chains.append((qsb, t_))
                        for kb in range(nkb):
                            d = kb * 128 - qc * TC
                            q0 = max(d, 0)
                            vsl = vt[kb][:, kv * (HD + 1):(kv + 1) * (HD + 1)]
                            for qsb, t_ in chains:
                                if kb > qc * NTB + qsb or qsb < q0 // 128:
                                    continue
                                nc.tensor.matmul(
                                    t_[:], phs[kb][:, qsb * 128:(qsb + 1) * 128], vsl,
                                    start=(kb == 0), stop=(kb == qc * NTB + qsb),
                                )
                            step += 1
                            want_s = step * len(next_scores) // nsteps
                            while si < want_s:
                                next_scores[si]()
                                si += 1
                            drain()
                        for qsb, t_ in chains:
                            norm_one(h, qsb, t_)
                    while si < len(next_scores):
                        next_scores[si]()
                        si += 1

                for u in score_units(0):
                    u()
                    drain()
                for h in range(NQ):
                    chains_for(h, score_units(h + 1) if h + 1 < NQ else [])
                while state["drained"] < nf:
                    filler[state["drained"]]()
                    state["drained"] += 1

            # ---------------- schedule ----------------
            # QKV0 standalone; attn(t) interleaves QKV(t+1) and outproj(t-1);
            # outproj(NTC-1) standalone.
            # chunk-0 wavefront: Q0..Q2 advance quarter-by-quarter as x/wq
            # land (3 concurrent chains = the 3 "sc" banks); later chains
            # run after their data has arrived
            _u0 = qkv_units(0, "sc", 3)
            _ng = C // 4
            _wave = []
            for g in range(_ng):
                for h in range(3):
                    _wave.append(_u0[h * _ng + g])
            for i in range(3 * _ng, len(_u0)):
                _wave.append(_u0[i])
            for u in _wave:
                u()
            # cs waits Q0-chain c2; wk halves wait Q0 end / Q1 mid; wv later
            _anchors = [2, 14, 18, 22, 26]
            for _d, _a in zip(startup_dmas, _anchors):
                if _d is not None and _a < len(startup_mms):
                    add_dep_helper(_d.ins, startup_mms[_a].ins,
                                   reason="startup HBM burst throttle")
            for tci in range(NTC):
                if tci + 1 < NTC:
                    emit_x_dmas(tci + 1, [nc.sync, nc.sync, nc.gpsimd, nc.gpsimd])
                filler = []
                if tci > 0:
                    filler += op_units(tci - 1, "op", 1)
                if tci + 1 < NTC:
                    filler += qkv_units(tci + 1, "fill", 2)
                emit_attention(tci, filler)
                xq_tiles.pop(tci)
            for u in op_units(NTC - 1, "sc", 3):
                u()

    return nc


# ---------------------------------------------------------------------------
# walrus in this container refuses >1 sem wait per instruction ("Too many
# sync wait commands"). Hoist excess waits onto same-engine NoOps inserted
# immediately before the instruction - program order on the engine queue
# preserves the sync semantics.
def split_multiwait_insts(nc, max_waits=1):
    n_split = 0
    for bb in nc.main_func.blocks:
        insts = bb.instructions
        i = 0
        while i < len(insts):
            ins = insts[i]
            si = getattr(ins, "sync_info", None)
            if si is not None and si.on_wait and len(si.on_wait) > max_waits:
                waits = list(si.on_wait)
                head, tail = waits[:-max_waits], waits[-max_waits:]
                nops = []
                for j in range(0, len(head), max_waits):
                    nop = mybir.InstNoOp(name=f"{ins.name}-ws{j}", ins=[], outs=[])
                    nop.engine = ins.engine
                    nop.sync_info = mybir.SyncInfo(
                        on_wait=head[j:j + max_waits], on_update=[])
                    nops.append(nop)
                ins.sync_info = mybir.SyncInfo(
                    on_wait=tail, on_update=list(si.on_update or []))
                insts[i:i] = nops
                i += len(nops)
                n_split += 1
            i += 1
    return n_split


# ---------------------------------------------------------------------------
# Host-side shard preparation / gather
BF16 = ml_dtypes.bfloat16


def rope_tables(S, HD):
    inv = 1.0 / (10000.0 ** (np.arange(0, HD, 2, dtype=np.float32) / HD))
    t = np.arange(S, dtype=np.float32)
    f = np.outer(t, inv).astype(np.float32)  # [S, HD//2]
    return np.ascontiguousarray(np.cos(f).T), np.ascontiguousarray(np.sin(f).T)


def causal_masks(TC):
    # masks[dd][k, qrel] = 1 if k + dd*128 <= qrel else 0
    out = np.zeros((4 * 128, TC), BF16)
    k = np.arange(128)[:, None]
    q = np.arange(TC)[None, :]
    for dd in range(4):
        out[dd * 128:(dd + 1) * 128] = (k + dd * 128 <= q).astype(BF16)
    return out


def rope_perm(HD):
    # new row i (i < HD//2) = old 2i; new row HD//2+i = old 2i+1
    return np.concatenate([np.arange(0, HD, 2), np.arange(1, HD, 2)])


def make_in_maps(x, wq, wk, wv, wo, *, n_batch_shards, n_head_shards,
                 NQ_TOT, NKV_TOT, HD, TC):
    """Returns list of in_maps, one per core (batch-major: core = b*G + g)."""
    B, S, D = x.shape
    G = n_head_shards
    NQ = NQ_TOT // G
    NKV = NKV_TOT // G
    perm = rope_perm(HD)
    cosT, sinT = rope_tables(S, HD)
    csT = np.concatenate([cosT, sinT], axis=0).astype(BF16)  # [HD, S]
    masks = causal_masks(TC)

    # Per-batch xT (shared across head shards)
    xtb = {}
    for b in range(B):
        xtb[b] = np.ascontiguousarray(x[b].T).astype(BF16)  # [D, S]

    # Per-headgroup weight shards
    wshard = {}
    for g in range(G):
        qrows = slice(g * NQ * HD, (g + 1) * NQ * HD)
        kvrows = slice(g * NKV * HD, (g + 1) * NKV * HD)
        wq_g = wq[qrows, :].copy()      # [NQ*HD, D]
        wk_g = wk[kvrows, :].copy()
        wv_g = wv[kvrows, :].copy()
        # RoPE permutation of output rows, per head
        for hh in range(NQ):
            blk = wq_g[hh * HD:(hh + 1) * HD]
            wq_g[hh * HD:(hh + 1) * HD] = blk[perm]
        for hh in range(NKV):
            blk = wk_g[hh * HD:(hh + 1) * HD]
            wk_g[hh * HD:(hh + 1) * HD] = blk[perm]
        wqT = np.ascontiguousarray(wq_g.T).astype(BF16)   # [D, NQ*HD]
        wkT = np.ascontiguousarray(wk_g.T).astype(BF16)
        wvT = np.ascontiguousarray(wv_g.T).astype(BF16)
        woT = np.ascontiguousarray(wo[:, qrows].T).astype(BF16)  # [NQ*HD, D]
        wshard[g] = (wqT, wkT, wvT, woT)

    ident = np.eye(128, dtype=BF16)
    in_maps = []
    for b in range(n_batch_shards):
        for g in range(G):
            wqT, wkT, wvT, woT = wshard[g]
            in_maps.append({
                "xt": xtb[b],
                "wqp": wqT, "wkp": wkT, "wvp": wvT, "woh": woT,
                "csT": csT,
                "masks": masks,
                "ident": ident,
            })
    return in_maps


def combine_outputs(outTs, B, G):
    """outTs: list of [D, S] partials, core order b*G+g. Returns [B, S, D]."""
    outs = []
    for b in range(B):
        acc = outTs[b * G].astype(np.float32).copy()
        for g in range(1, G):
            acc += outTs[b * G + g]
        outs.append(acc.T)  # [S, D]
    return np.stack(outs)


_NC_CACHE = {}


def _get_nc(S, D, NQ, NKV, HD, TC):
    key = (S, D, NQ, NKV, HD, TC)
    if key not in _NC_CACHE:
        nc = build_attention_nc(S=S, D=D, NQ=NQ, NKV=NKV, HD=HD, TC=TC)
        split_multiwait_insts(nc)
        _NC_CACHE[key] = nc
    return _NC_CACHE[key]


def kernel(**inputs):
    x = np.asarray(inputs["x"], dtype=np.float32)
    wq = np.asarray(inputs["wq"], dtype=np.float32)
    wk = np.asarray(inputs["wk"], dtype=np.float32)
    wv = np.asarray(inputs["wv"], dtype=np.float32)
    wo = np.asarray(inputs["wo"], dtype=np.float32)

    B, S, D = x.shape          # (2, 2048, 2048)
    NQ_TOT = wq.shape[0] // 128
    NKV_TOT = wk.shape[0] // 128
    HD = 128
    TC = 512
    G = 4                      # head shards
    NQ, NKV = NQ_TOT // G, NKV_TOT // G

    nc = _get_nc(S, D, NQ, NKV, HD, TC)
    in_maps = make_in_maps(
        x, wq, wk, wv, wo,
        n_batch_shards=B, n_head_shards=G,
        NQ_TOT=NQ_TOT, NKV_TOT=NKV_TOT, HD=HD, TC=TC,
    )

    from concourse.bass_utils import run_bass_kernel_spmd

    trace = os.environ.get("BASS_ATTN_TRACE") == "1"
    res = run_bass_kernel_spmd(nc, in_maps, list(range(len(in_maps))), trace=trace)
    kernel.last_results = res
    outTs = [r["outT"] for r in res.results]
    return combine_outputs(outTs, B, G).astype(np.float32)


# revision 40
# speedup vs baseline: 2.9279x; 1.0052x over previous
"""Trainium2 Bass kernel for nn_Attention_77043123355775.

Sharded GQA causal attention with RoPE: 8 NeuronCores as 2-way data
parallel (batch) x 4-way tensor parallel (heads). Each core computes its
4 Q heads / 2 KV heads for one batch entry and a partial output
projection (x[b] @ W)^T; the host sums the 4 partials per batch.

All matmuls run in plain bf16 with fp32 PSUM accumulation (the 2e-2
rel-err budget has ~20x headroom over bf16 rounding noise). Weights are
fully SBUF-resident. The PE instruction stream interleaves next-chunk
QKV/V projection chains and prev-chunk output-projection groups between
attention blocks, so the PE never waits on the scalar-engine exp chain
and stays in its high DVFS p-state.
"""
import math
import os
import sys

for _p in ("/opt/trn_rl_repo",):
    if _p not in sys.path:
        sys.path.insert(0, _p)

import ml_dtypes
import numpy as np

import concourse.bass as bass
import concourse.mybir as mybir
import concourse.tile as tile

from concourse.tile import add_dep_helper

dt = mybir.dt
AF = mybir.ActivationFunctionType


def build_attention_nc(S=2048, D=2048, NQ=4, NKV=2, HD=128, TC=512):
    assert HD == 128
    C = D // 128          # contraction chunks over features
    TB = S // 128         # 128-token blocks
    NTC = S // TC         # token chunks
    DB = D // 128         # output feature blocks
    CO = NQ * HD // 128   # contraction chunks for wo (= NQ)
    REP = NQ // NKV
    CQ = C // 4           # c-chunks per x quarter-tile
    NTB = TC // 128       # token blocks per chunk
    scale = 1.0 / math.sqrt(HD)

    nc = bass.Bass()

    xt = nc.dram_tensor("xt", [D, S], dt.bfloat16, kind="ExternalInput")
    ident = nc.dram_tensor("ident", [128, 128], dt.bfloat16, kind="ExternalInput")
    wqp = nc.dram_tensor("wqp", [D, NQ * HD], dt.bfloat16, kind="ExternalInput")
    wkp = nc.dram_tensor("wkp", [D, NKV * HD], dt.bfloat16, kind="ExternalInput")
    wvp = nc.dram_tensor("wvp", [D, NKV * HD], dt.bfloat16, kind="ExternalInput")
    woh = nc.dram_tensor("woh", [NQ * HD, D], dt.bfloat16, kind="ExternalInput")
    csT = nc.dram_tensor("csT", [HD, S], dt.bfloat16, kind="ExternalInput")
    masks = nc.dram_tensor("masks", [4 * 128, TC], dt.bfloat16, kind="ExternalInput")
    outT = nc.dram_tensor("outT", [D, S], dt.bfloat16, kind="ExternalOutput")

    with tile.TileContext(nc) as tc:
        with (
            tc.tile_pool(name="const", bufs=1) as constp,
            tc.tile_pool(name="tabs", bufs=1) as tabp,
            tc.tile_pool(name="wts", bufs=1) as wtp,
            tc.tile_pool(name="acts", bufs=1) as actp,
            tc.tile_pool(name="chunkacts", bufs=1) as cap,
            tc.tile_pool(name="xstream", bufs=8) as xsp,
            tc.tile_pool(name="scratch", bufs=3) as scr,
            tc.tile_pool(name="psum", bufs=1, space="PSUM") as psp,
        ):
            ident_t = constp.tile([128, 128], dt.bfloat16, tag="ident")

            # ---- resident tables / weights (c-quartered for fine deps) ----
            def emit_w_dma(tiles, src, g, ring):
                cq = C // len(tiles)
                rs = slice(g * cq * 128, (g + 1) * cq * 128)
                return ring.dma_start(
                    tiles[g].rearrange("p (c n) -> p c n", c=cq),
                    src[rs, :].rearrange("(c p) n -> p c n", p=128),
                )

            wq_ts = [wtp.tile([128, CQ * NQ * HD], dt.bfloat16, tag=f"wq{g}", name=f"wq{g}") for g in range(4)]
            wk_ts = [wtp.tile([128, (C // 2) * NKV * HD], dt.bfloat16, tag=f"wk{g}", name=f"wk{g}") for g in range(2)]
            wv_ts = [wtp.tile([128, (C // 2) * NKV * HD], dt.bfloat16, tag=f"wv{g}", name=f"wv{g}") for g in range(2)]
            wo_t = wtp.tile([128, CO * D], dt.bfloat16, tag="wo")

            def wq_sl(c, h):
                return wq_ts[c // CQ][:, (c % CQ) * NQ * HD + h * HD:(c % CQ) * NQ * HD + (h + 1) * HD]

            def wk_sl(c, h):
                ch = C // 2
                return wk_ts[c // ch][:, (c % ch) * NKV * HD + h * HD:(c % ch) * NKV * HD + (h + 1) * HD]

            def wv_sl(c):
                ch = C // 2
                return wv_ts[c // ch][:, (c % ch) * NKV * HD:(c % ch + 1) * NKV * HD]

            cs_t = tabp.tile([HD, S], dt.bfloat16, tag="cs")
            cos_t = cs_t[0:HD // 2, :]
            sin_t = cs_t[HD // 2:HD, :]
            mask_t = [tabp.tile([128, TC], dt.bfloat16, tag=f"mask{i}", name=f"mask{i}") for i in range(4)]

            xq_tiles = {}

            def emit_x_dmas(tci, rings):
                ts_ = slice(tci * TC, (tci + 1) * TC)
                tiles = []
                for g in range(4):
                    rs = slice(g * CQ * 128, (g + 1) * CQ * 128)
                    t = xsp.tile([128, CQ * TC], dt.bfloat16, tag="xq",
                                 name=f"x_{tci}_{g}")
                    rings[g].dma_start(
                        t.rearrange("p (c n) -> p c n", c=CQ),
                        xt[rs, ts_].rearrange("(c p) n -> p c n", p=128),
                    )
                    tiles.append(t)
                xq_tiles[tci] = tiles

            # startup: the first QKV chain consumes (wq_g, x_g) pairs in
            # quarter order, so interleave them pairwise on the fast sync
            # ring; gpsimd carries the RoPE tables + K/V weights (needed
            # a few microseconds later).
            x0_tiles = []
            for g in range(4):
                emit_w_dma(wq_ts, wqp, g, nc.sync)
                rs = slice(g * CQ * 128, (g + 1) * CQ * 128)
                t = xsp.tile([128, CQ * TC], dt.bfloat16, tag="xq", name=f"x_0_{g}")
                nc.sync.dma_start(
                    t.rearrange("p (c n) -> p c n", c=CQ),
                    xt[rs, 0:TC].rearrange("(c p) n -> p c n", p=128),
                )
                x0_tiles.append(t)
            xq_tiles[0] = x0_tiles
            # The secondary loads (cs/wk/wv/masks) would otherwise compete
            # with the critical wq/x burst for chip HBM bandwidth at t=0
            # (all 8 cores slurp simultaneously); throttle them behind
            # early Q-chain matmuls via explicit deps filled in later.
            startup_dmas = []
            startup_dmas.append(nc.gpsimd.dma_start(cs_t[:], csT[:]))
            startup_dmas.append(emit_w_dma(wk_ts, wkp, 0, nc.gpsimd))
            startup_dmas.append(emit_w_dma(wk_ts, wkp, 1, nc.gpsimd))
            startup_dmas.append(emit_w_dma(wv_ts, wvp, 0, nc.gpsimd))
            startup_dmas.append(emit_w_dma(wv_ts, wvp, 1, nc.gpsimd))
            for i in range(4):
                nc.gpsimd.dma_start(mask_t[i][:], masks[i * 128:(i + 1) * 128, :])
            nc.gpsimd.dma_start(ident_t[:], ident[:])
            nc.sync.dma_start(
                wo_t.rearrange("p (c n) -> p c n", c=CO),
                woh.rearrange("(c p) n -> p c n", p=128),
            )
            startup_mms = []

            # K/V persist per 512-chunk / 128-block (no cross-chunk tiles,
            # so interleaved next-chunk RoPE writes never alias attention
            # reads at the dep tracker's granularity)
            ktc = [[actp.tile([128, TC], dt.bfloat16, tag=f"kt{h}_{j}", name=f"kt{h}_{j}")
                    for j in range(NTC)] for h in range(NKV)]
            # V tiles carry a ones column per kv head (col kv*(HD+1)+HD) so
            # the flipped PV matmul emits softmax denominators for free
            vt = [actp.tile([128, NKV * (HD + 1)], dt.bfloat16, tag=f"vt{b}", name=f"vt{b}") for b in range(TB)]
            for b in range(TB):
                for kv in range(NKV):
                    nc.vector.memset(vt[b][:, kv * (HD + 1) + HD:(kv + 1) * (HD + 1)], 1.0)
            qt_all = {}
            ot_all = {}
            for tci in range(NTC):
                qt_all[tci] = [cap.tile([128, TC], dt.bfloat16, tag=f"qt{h}_{tci % 2}", name=f"qt{h}_{tci}") for h in range(NQ)]
                ot_all[tci] = [cap.tile([128, TC], dt.bfloat16, tag=f"ot{h}_{tci % 2}", name=f"ot{h}_{tci}") for h in range(NQ)]

            # ---------------- unit generators ----------------
            def rope_epilogue(tci, h, ps):
                ts_ = slice(tci * TC, (tci + 1) * TC)
                rot = scr.tile([128, TC], dt.bfloat16, tag="rope", bufs=2)
                t0 = scr.tile([128, TC], dt.bfloat16, tag="ropetmp", bufs=1)
                cs = cos_t[:, ts_]
                sn = sin_t[:, ts_]
                xr = ps[0:64, :]
                xi = ps[64:128, :]
                dsth = qt_all[tci][h][:] if h < NQ else ktc[h - NQ][tci][:]
                nc.vector.tensor_tensor(rot[0:64, :], xr, cs, mybir.AluOpType.mult)
                nc.vector.tensor_tensor(t0[0:64, :], xi, sn, mybir.AluOpType.mult)
                nc.vector.tensor_tensor(dsth[0:64, :], rot[0:64, :], t0[0:64, :], mybir.AluOpType.subtract)
                nc.vector.tensor_tensor(rot[64:128, :], xr, sn, mybir.AluOpType.mult)
                nc.vector.tensor_tensor(t0[64:128, :], xi, cs, mybir.AluOpType.mult)
                nc.vector.tensor_tensor(dsth[64:128, :], rot[64:128, :], t0[64:128, :], mybir.AluOpType.add)

            def qkv_units(tci, ptag, pbufs, group=4):
                """Closures emitting `group` matmuls of a QKV/V chain each
                (coarser units keep chain psum lifetimes short)."""
                units = []
                state = {}

                def x_c(c):
                    xg = xq_tiles[tci]
                    return xg[c // CQ][:, (c % CQ) * TC:(c % CQ + 1) * TC]

                def qk_seg(h, c0):
                    if c0 == 0:
                        state[h] = psp.tile([128, TC], dt.float32, tag=ptag,
                                            bufs=pbufs, name=f"qkv_{tci}_{h}")
                    ps = state[h]
                    for c in range(c0, c0 + group):
                        wsl = wq_sl(c, h) if h < NQ else wk_sl(c, h - NQ)
                        mm = nc.tensor.matmul(ps[:], wsl, x_c(c),
                                              start=(c == 0), stop=(c == C - 1))
                        if tci == 0 and h < 2:
                            startup_mms.append(mm)
                    if c0 + group == C:
                        rope_epilogue(tci, h, ps)

                def v_seg(tb, c0):
                    key = "v", tb
                    if c0 == 0:
                        state[key] = psp.tile([128, NKV * HD], dt.float32,
                                              tag=ptag, bufs=pbufs,
                                              name=f"v_{tci}_{tb}")
                    ps = state[key]
                    for c in range(c0, c0 + group):
                        nc.tensor.matmul(ps[:], x_c(c)[:, tb * 128:(tb + 1) * 128],
                                         wv_sl(c),
                                         start=(c == 0), stop=(c == C - 1))
                    if c0 + group == C:
                        for kv in range(NKV):
                            nc.scalar.copy(
                                vt[tci * NTB + tb][:, kv * (HD + 1):kv * (HD + 1) + HD],
                                ps[:, kv * HD:(kv + 1) * HD])

                for h in range(NQ + NKV):
                    for c0 in range(0, C, group):
                        units.append(lambda h=h, c0=c0: qk_seg(h, c0))
                for tb in range(NTB):
                    for c0 in range(0, C, group):
                        units.append(lambda tb=tb, c0=c0: v_seg(tb, c0))
                return units

            def op_units(tci, ptag, pbufs):
                """One closure per output-projection db group (4 matmuls +
                copy + store)."""
                ts_ = slice(tci * TC, (tci + 1) * TC)
                ot = ot_all[tci]
                units = []
                for db in range(DB):
                    def u(db=db):
                        ps = psp.tile([128, TC], dt.float32, tag=ptag, bufs=pbufs,
                                      name=f"op_{tci}_{db}")
                        for c in range(CO):
                            nc.tensor.matmul(
                                ps[:], wo_t[:, c * D + db * 128:c * D + (db + 1) * 128],
                                ot[c][:],
                                start=(c == 0), stop=(c == CO - 1),
                            )
                        o3 = scr.tile([128, TC], dt.bfloat16, tag="o3", bufs=4)
                        if tci == NTC - 1 and db % 2 == 1:
                            nc.vector.tensor_copy(o3[:], ps[:])
                        else:
                            nc.scalar.copy(o3[:], ps[:])
                        eng = nc.sync if db % 2 == 0 else nc.gpsimd
                        eng.dma_start(outT[db * 128:(db + 1) * 128, ts_], o3[:])
                    units.append(u)
                return units

            # ---------------- attention emission ----------------
            def emit_attention(tci, filler):
                """Scores + flipped PV for q-chunk tci, draining `filler`
                closures between steps to keep the PE busy. PV runs
                qsb-major: each (head, q-subtile) accumulates [q,HD+1]
                (output + denominator column) as a single group in its own
                PSUM bank -- interleaved groups in one bank corrupt."""
                qc = tci
                qt = qt_all[tci]
                ot = ot_all[tci]
                nkb = (qc + 1) * NTB
                nf = len(filler)
                total_steps = (2 * NQ + 1) * nkb
                state = {"step": 0, "drained": 0}

                def drain():
                    state["step"] += 1
                    want = state["step"] * nf // total_steps
                    while state["drained"] < want:
                        filler[state["drained"]]()
                        state["drained"] += 1

                def emit_scores(h, kb):
                    kv = h // REP
                    d = kb * 128 - qc * TC
                    q0 = max(d, 0)
                    sc_ps = psp.tile([128, TC], dt.float32, tag="sc", bufs=3,
                                     name=f"sc_{tci}_{h}_{kb}")
                    ksl = ktc[kv][kb // NTB][:, (kb % NTB) * 128:(kb % NTB + 1) * 128]
                    nc.tensor.matmul(sc_ps[:, q0:TC], ksl, qt[h][:, q0:TC],
                                     start=True, stop=True)
                    ph = scr.tile([128, TC], dt.bfloat16, tag="ph", bufs=36,
                                  name=f"ph_{tci}_{h}_{kb}")
                    nc.scalar.activation(ph[:, q0:TC], sc_ps[:, q0:TC], AF.Exp,
                                         bias=0.0, scale=scale)
                    if d >= 0:
                        nc.vector.tensor_tensor(ph[:, q0:TC], ph[:, q0:TC],
                                                mask_t[d // 128][:, q0:TC],
                                                mybir.AluOpType.mult)
                    return ph

                def norm_one(h, qsb, t_):
                    rec = scr.tile([128, 1], dt.float32, tag="recq", bufs=4,
                                   name=f"rec_{tci}_{h}_{qsb}")
                    nc.vector.reciprocal(rec[:], t_[:, HD:HD + 1])
                    otn = scr.tile([128, HD], dt.bfloat16, tag="otn", bufs=4,
                                   name=f"otn_{tci}_{h}_{qsb}")
                    nc.vector.tensor_scalar(otn[:], t_[:, 0:HD], rec[:], None,
                                            mybir.AluOpType.mult)
                    tp = psp.tile([128, 128], dt.bfloat16, tag="sc", bufs=3,
                                  name=f"tp_{tci}_{h}_{qsb}")
                    nc.tensor.transpose(tp[:], otn[:], ident_t[:])
                    nc.scalar.copy(ot[h][:, qsb * 128:(qsb + 1) * 128], tp[:])

                # head software pipeline: scores of head h+1 are emitted
                # between PV chain steps of head h, so the scalar exp chain
                # for the next head runs under the current head's PE work
                all_phs = {}

                def score_units(h):
                    def u(h=h, kb=None):
                        pass
                    us = []
                    for kb in range(nkb):
                        def uu(h=h, kb=kb):
                            all_phs.setdefault(h, []).append(emit_scores(h, kb))
                        us.append(uu)
                    return us

                def chains_for(h, next_scores):
                    kv = h // REP
                    phs = all_phs[h]
                    nsteps = 2 * nkb
                    si = 0
                    step = 0
                    for qsb0 in (0, 2):
                        chains = []
                        for qsb in (qsb0, qsb0 + 1):
                            t_ = psp.tile([128, HD + 1], dt.float32, tag="otq",
                                          bufs=2, name=f"otq_{tci}_{h}_{qsb}")
                            chains.append((qsb, t_))
                        for kb in range(nkb):
                            d = kb * 128 - qc * TC
                            q0 = max(d, 0)
                            vsl = vt[kb][:, kv * (HD + 1):(kv + 1) * (HD + 1)]
                            for qsb, t_ in chains:
                                if kb > qc * NTB + qsb or qsb < q0 // 128:
                                    continue
                                nc.tensor.matmul(
                                    t_[:], phs[kb][:, qsb * 128:(qsb + 1) * 128], vsl,
                                    start=(kb == 0), stop=(kb == qc * NTB + qsb),
                                )
                            step += 1
                            want_s = step * len(next_scores) // nsteps
                            while si < want_s:
                                next_scores[si]()
                                si += 1
                            drain()
                        for qsb, t_ in chains:
                            norm_one(h, qsb, t_)
                    while si < len(next_scores):
                        next_scores[si]()
                        si += 1

                for u in score_units(0):
                    u()
                    drain()
                for h in range(NQ):
                    chains_for(h, score_units(h + 1) if h + 1 < NQ else [])
                while state["drained"] < nf:
                    filler[state["drained"]]()
                    state["drained"] += 1

            # ---------------- schedule ----------------
            # QKV0 standalone; attn(t) interleaves QKV(t+1) and outproj(t-1);
            # outproj(NTC-1) standalone.
            # chunk-0 wavefront: Q0..Q2 advance quarter-by-quarter as x/wq
            # land (3 concurrent chains = the 3 "sc" banks); later chains
            # run after their data has arrived
            _u0 = qkv_units(0, "sc", 3)
            _ng = C // 4
            _wave = []
            for g in range(_ng):
                for h in range(3):
                    _wave.append(_u0[h * _ng + g])
            for i in range(3 * _ng, len(_u0)):
                _wave.append(_u0[i])
            for u in _wave:
                u()
            # cs waits Q0-chain c2; wk halves wait Q0 end / Q1 mid; wv later
            _anchors = [2, 14, 18, 22, 26]
            for _d, _a in zip(startup_dmas, _anchors):
                if _d is not None and _a < len(startup_mms):
                    add_dep_helper(_d.ins, startup_mms[_a].ins,
                                   reason="startup HBM burst throttle")
            for tci in range(NTC):
                if tci + 1 < NTC:
                    emit_x_dmas(tci + 1, [nc.sync, nc.sync, nc.gpsimd, nc.gpsimd])
                filler = []
                if tci > 0:
                    filler += op_units(tci - 1, "op", 1)
                if tci + 1 < NTC:
                    filler += qkv_units(tci + 1, "fill", 2)
                emit_attention(tci, filler)
                xq_tiles.pop(tci)
            for u in op_units(NTC - 1, "sc", 3):
                u()

    return nc


# ---------------------------------------------------------------------------
# walrus in this container refuses >1 sem wait per instruction ("Too many
# sync wait commands"). Hoist excess waits onto same-engine NoOps inserted
# immediately before the instruction - program order on the engine queue
# preserves the sync semantics.
def split_multiwait_insts(nc, max_waits=1):
    n_split = 0
    for bb in nc.main_func.blocks:
        insts = bb.instructions
        i = 0
        while i < len(insts):
            ins = insts[i]
            si = getattr(ins, "sync_info", None)
            if si is not None and si.on_wait and len(si.on_wait) > max_waits:
                waits = list(si.on_wait)
                head, tail = waits[:-max_waits], waits[-max_waits:]
                nops = []
                for j in range(0, len(head), max_waits):
                    nop = mybir.InstNoOp(name=f"{ins.name}-ws{j}", ins=[], outs=[])
                    nop.engine = ins.engine
                    nop.sync_info = mybir.SyncInfo(
                        on_wait=head[j:j + max_waits], on_update=[])
                    nops.append(nop)
                ins.sync_info = mybir.SyncInfo(
                    on_wait=tail, on_update=list(si.on_update or []))
                insts[i:i] = nops
                i += len(nops)
                n_split += 1
            i += 1
    return n_split


# ---------------------------------------------------------------------------
# Host-side shard preparation / gather
BF16 = ml_dtypes.bfloat16


def rope_tables(S, HD):
    inv = 1.0 / (10000.0 ** (np.arange(0, HD, 2, dtype=np.float32) / HD))
    t = np.arange(S, dtype=np.float32)
    f = np.outer(t, inv).astype(np.float32)  # [S, HD//2]
    return np.ascontiguousarray(np.cos(f).T), np.ascontiguousarray(np.sin(f).T)


def causal_masks(TC):
    # masks[dd][k, qrel] = 1 if k + dd*128 <= qrel else 0
    out = np.zeros((4 * 128, TC), BF16)
    k = np.arange(128)[:, None]
    q = np.arange(TC)[None, :]
    for dd in range(4):
        out[dd * 128:(dd + 1) * 128] = (k + dd * 128 <= q).astype(BF16)
    return out


def rope_perm(HD):
    # new row i (i < HD//2) = old 2i; new row HD//2+i = old 2i+1
    return np.concatenate([np.arange(0, HD, 2), np.arange(1, HD, 2)])


def make_in_maps(x, wq, wk, wv, wo, *, n_batch_shards, n_head_shards,
                 NQ_TOT, NKV_TOT, HD, TC):
    """Returns list of in_maps, one per core (batch-major: core = b*G + g)."""
    B, S, D = x.shape
    G = n_head_shards
    NQ = NQ_TOT // G
    NKV = NKV_TOT // G
    perm = rope_perm(HD)
    cosT, sinT = rope_tables(S, HD)
    csT = np.concatenate([cosT, sinT], axis=0).astype(BF16)  # [HD, S]
    masks = causal_masks(TC)

    # Per-batch xT (shared across head shards)
    xtb = {}
    for b in range(B):
        xtb[b] = np.ascontiguousarray(x[b].T).astype(BF16)  # [D, S]

    # Per-headgroup weight shards
    wshard = {}
    for g in range(G):
        qrows = slice(g * NQ * HD, (g + 1) * NQ * HD)
        kvrows = slice(g * NKV * HD, (g + 1) * NKV * HD)
        wq_g = wq[qrows, :].copy()      # [NQ*HD, D]
        wk_g = wk[kvrows, :].copy()
        wv_g = wv[kvrows, :].copy()
        # RoPE permutation of output rows, per head
        for hh in range(NQ):
            blk = wq_g[hh * HD:(hh + 1) * HD]
            wq_g[hh * HD:(hh + 1) * HD] = blk[perm]
        for hh in range(NKV):
            blk = wk_g[hh * HD:(hh + 1) * HD]
            wk_g[hh * HD:(hh + 1) * HD] = blk[perm]
        wqT = np.ascontiguousarray(wq_g.T).astype(BF16)   # [D, NQ*HD]
        wkT = np.ascontiguousarray(wk_g.T).astype(BF16)
        wvT = np.ascontiguousarray(wv_g.T).astype(BF16)
        woT = np.ascontiguousarray(wo[:, qrows].T).astype(BF16)  # [NQ*HD, D]
        wshard[g] = (wqT, wkT, wvT, woT)

    ident = np.eye(128, dtype=BF16)
    in_maps = []
    for b in range(n_batch_shards):
        for g in range(G):
            wqT, wkT, wvT, woT = wshard[g]
            in_maps.append({
                "xt": xtb[b],
                "wqp": wqT, "wkp": wkT, "wvp": wvT, "woh": woT,
                "csT": csT,
                "masks": masks,
                "ident": ident,
            })
    return in_maps


def combine_outputs(outTs, B, G):
    """outTs: list of [D, S] partials, core order b*G+g. Returns [B, S, D]."""
    outs = []
    for b in range(B):
        acc = outTs[b * G].astype(np.float32).copy()
        for g in range(1, G):
            acc += outTs[b * G + g]
        outs.append(acc.T)  # [S, D]
    return np.stack(outs)


_NC_CACHE = {}


def _get_nc(S, D, NQ, NKV, HD, TC):
    key = (S, D, NQ, NKV, HD, TC)
    if key not in _NC_CACHE:
        nc = build_attention_nc(S=S, D=D, NQ=NQ, NKV=NKV, HD=HD, TC=TC)
        split_multiwait_insts(nc)
        _NC_CACHE[key] = nc
    return _NC_CACHE[key]


def kernel(**inputs):
    x = np.asarray(inputs["x"], dtype=np.float32)
    wq = np.asarray(inputs["wq"], dtype=np.float32)
    wk = np.asarray(inputs["wk"], dtype=np.float32)
    wv = np.asarray(inputs["wv"], dtype=np.float32)
    wo = np.asarray(inputs["wo"], dtype=np.float32)

    B, S, D = x.shape          # (2, 2048, 2048)
    NQ_TOT = wq.shape[0] // 128
    NKV_TOT = wk.shape[0] // 128
    HD = 128
    TC = 512
    G = 4                      # head shards
    NQ, NKV = NQ_TOT // G, NKV_TOT // G

    nc = _get_nc(S, D, NQ, NKV, HD, TC)
    in_maps = make_in_maps(
        x, wq, wk, wv, wo,
        n_batch_shards=B, n_head_shards=G,
        NQ_TOT=NQ_TOT, NKV_TOT=NKV_TOT, HD=HD, TC=TC,
    )

    from concourse.bass_utils import run_bass_kernel_spmd

    trace = os.environ.get("BASS_ATTN_TRACE") == "1"
    res = run_bass_kernel_spmd(nc, in_maps, list(range(len(in_maps))), trace=trace)
    kernel.last_results = res
    outTs = [r["outT"] for r in res.results]
    return combine_outputs(outTs, B, G).astype(np.float32)
